# revision 1
# baseline (speedup 1.0000x reference)
"""DiT block kernel for Trainium2 (Bass/Tile), 8-core data parallel.

Shapes (hardcoded from the problem spec):
  x: (8, 1024, 1152), t_emb: (8, 1152)
  w_qkv (1152, 3456), w_proj (1152, 1152), w_fc1 (1152, 4608),
  w_fc2 (4608, 1152), w_ada (1152, 6912) + biases.

Strategy: batch-parallel across 8 cores (one batch element each, no
collectives). Inside a core, activations live in "transposed" layout
[D on partitions, tokens on free] so every projection is
out_T = W.T @ x_T with lhsT = W exactly as stored in DRAM.
LayerNorm statistics reduce over the partition (D) axis via ones-vector
matmuls; softmax runs in transposed orientation (keys on partitions,
no max subtraction -- scores are bounded ~+-8 here) with sums collected
through a ones-column appended to V. Matmuls run in float32r (full PE
rate at N>=256) except attention/proj/fc2 which run in bf16.
"""

import os
import threading
from contextlib import ExitStack

import numpy as np

import concourse.bass as bass
import concourse.mybir as mybir
import concourse.tile as tile
from concourse import bacc
from concourse.bass_utils import run_bass_kernel_spmd
from concourse.masks import make_identity

F32 = mybir.dt.float32
F32R = mybir.dt.float32r
BF16 = mybir.dt.bfloat16
AF = mybir.ActivationFunctionType
ALU = mybir.AluOpType

NCORES = 8
D = 1152
NT = 1024          # tokens per core (batch element)
KT = D // 128      # 9 partition-tiles of D
H = 16
HD = 72
HID = 4 * D        # 4608
MQK = (2 * D) // 128   # 18 output tiles for q,k
MH = HID // 128        # 36
EPS = 1e-6
ISC = 1.0 / float(np.sqrt(HD))

# v output column slices aligned to head boundaries (each >=256 for f32r)
V_SLICES = [(0, 432, 0, 6), (432, 864, 6, 12), (864, 1152, 12, 16)]


def _r(ap):
    return ap.bitcast(F32R)


def _head_segs(d0, n):
    """Split logical rows [d0, d0+n) of a [*,128]-tiled stacked tensor into
    (ktile, part0, length, dst_offset) segments within 128-partition tiles."""
    segs = []
    off = 0
    while n > 0:
        kt_i, p0 = divmod(d0, 128)
        ln = min(n, 128 - p0)
        segs.append((kt_i, p0, ln, off))
        d0 += ln
        off += ln
        n -= ln
    return segs


def _build_program():
    nc = bacc.Bacc(
        "TRN2", target_bir_lowering=False, debug=False, enable_asserts=False
    )
    ins = {}
    ins["x"] = nc.dram_tensor("x", [NT, D], F32, kind="ExternalInput").ap()
    ins["t_emb"] = nc.dram_tensor("t_emb", [D], F32, kind="ExternalInput").ap()
    for name, shape in [
        ("w_qkv", [D, 3 * D]), ("b_qkv", [3 * D]),
        ("w_proj", [D, D]), ("b_proj", [D]),
        ("w_fc1", [D, HID]), ("b_fc1", [HID]),
        ("w_fc2", [HID, D]), ("b_fc2", [D]),
        ("w_ada", [D, 6 * D]), ("b_ada", [6 * D]),
    ]:
        ins[name] = nc.dram_tensor(name, shape, F32, kind="ExternalInput").ap()
    out_dram = nc.dram_tensor("out", [NT, D], F32, kind="ExternalOutput").ap()

    with tile.TileContext(nc) as tc:
        _body(tc, ins, out_dram)
    nc.compile()
    return nc


def _ln_stats_and_modulate(tc, nc, src, dst, ada_pp, shift_c, scale_c,
                           ones_col, pst, pln, ps_st):
    """dst[:,k,:] = ((src-mean)*rstd) * ada_pp[:,scale_c,k] + ada_pp[:,shift_c,k]
    (mean/rstd over the partition (D) axis per token; scale_c holds 1+scale;
    dst is bf16). Stats for both 512-token halves are emitted first so the
    PE finishes them early; applies follow per half so downstream matmuls
    on half 0 can start while half 1 is still being modulated."""
    ps_x, ps_q, st = {}, {}, {}
    for n in range(2):
        nsl = slice(n * 512, (n + 1) * 512)
        ps_x[n] = ps_st.tile([1, 512], F32, tag="st", name=f"psx{n}")
        ps_q[n] = ps_st.tile([1, 512], F32, tag="st", name=f"psq{n}")
        for k in range(KT):
            xb = pln.tile([128, 512], BF16, tag="xb", bufs=3, name="xb")
            nc.scalar.copy(xb[:, :], src[:, k, nsl])
            sq_b = pln.tile([128, 512], BF16, tag="sqb", bufs=3, name="sq_b")
            nc.vector.tensor_mul(sq_b[:, :], src[:, k, nsl], src[:, k, nsl])
            nc.tensor.matmul(
                ps_x[n][:, :], ones_col[:, :], xb[:, :],
                start=(k == 0), stop=(k == KT - 1), skip_group_check=True,
            )
            nc.tensor.matmul(
                ps_q[n][:, :], ones_col[:, :], sq_b[:, :],
                start=(k == 0), stop=(k == KT - 1), skip_group_check=True,
            )
    eps_sb = pst.tile([1, 1], F32, tag="eps", bufs=1, name="eps_sb")
    nc.vector.memset(eps_sb[:, :], EPS)
    for n in range(2):
        # rows: 0 = mean, 1 = E[x^2] -> rstd
        st[n] = pst.tile([1, 2, 512], F32, tag="lnst", bufs=2, name=f"st{n}")
        nc.vector.tensor_scalar_mul(st[n][:, 0, :], ps_x[n][:, :], 1.0 / D)
        work = pst.tile([1, 512], F32, tag="lnwork", bufs=2, name="work")
        nc.vector.tensor_mul(work[:, :], st[n][:, 0, :], st[n][:, 0, :])
        nc.vector.scalar_tensor_tensor(
            st[n][:, 1, :], ps_q[n][:, :], 1.0 / D, work[:, :],
            ALU.mult, ALU.subtract,
        )
        nc.scalar.activation(st[n][:, 1, :], st[n][:, 1, :], AF.Sqrt,
                             bias=eps_sb[:, :], scale=1.0)
        nc.vector.reciprocal(st[n][:, 1, :], st[n][:, 1, :])
    for n in range(2):
        nsl = slice(n * 512, (n + 1) * 512)
        meanB = pln.tile([128, 512], F32, tag="meanB", bufs=2, name="meanB")
        rstdB = pln.tile([128, 512], F32, tag="rstdB", bufs=2, name="rstdB")
        nc.gpsimd.partition_broadcast(meanB[:, :], st[n][:, 0, :])
        nc.gpsimd.partition_broadcast(rstdB[:, :], st[n][:, 1, :])
        for k in range(KT):
            tmp = pln.tile([128, 512], F32, tag="lnt", bufs=3, name="tmp")
            nc.vector.tensor_sub(tmp[:, :], src[:, k, nsl], meanB[:, :])
            nc.vector.tensor_mul(tmp[:, :], tmp[:, :], rstdB[:, :])
            nc.scalar.activation(
                dst[:, k, nsl], tmp[:, :], AF.Identity,
                bias=ada_pp[:, shift_c, k:k + 1],
                scale=ada_pp[:, scale_c, k:k + 1],
            )


def _truncate_out(tc, nc, out_dram):
    with tc.tile_pool(name="ptrunc", bufs=1) as p:
        z = p.tile([128, D], F32, name="z")
        nc.vector.memset(z[:, :], 0.0)
        for tt in range(NT // 128):
            nc.sync.dma_start(out_dram[tt * 128:(tt + 1) * 128, :], z[:, :])


def _body(tc, ins, out_dram):
    nc = tc.nc
    phase_limit = float(os.environ.get("BASS_PHASES", "6"))
    ctx = ExitStack()
    with ctx:
        dram = ctx.enter_context(tc.tile_pool(name="dram", bufs=1, space="DRAM"))
        ada_dr = dram.tile([6 * D], F32)
        w2_dr = dram.tile([HID, D], BF16)

        pers = ctx.enter_context(tc.tile_pool(name="pers", bufs=1))
        ident = pers.tile([128, 128], F32)
        make_identity(nc, ident[:, :])
        ones_col = pers.tile([128, 1], BF16)
        nc.vector.memset(ones_col[:, :], 1.0)
        t_pp = pers.tile([128, KT], F32)
        nc.sync.dma_start(t_pp[:, :], ins["t_emb"].rearrange("(k p) -> p k", p=128))
        t_pb = pers.tile([128, KT], BF16)
        nc.scalar.activation(t_pb[:, :], t_pp[:, :], AF.Silu)

        bqk_pp = pers.tile([128, MQK], F32)
        bv_pp = pers.tile([72, H], F32)
        bproj_pp = pers.tile([128, KT], F32)
        bfc1_pp = pers.tile([128, MH], F32)
        bfc2_pp = pers.tile([128, KT], F32)
        bada_pp = pers.tile([128, 6, KT], F32)
        ada_pp = pers.tile([128, 6, KT], F32)

        def emit_bias_loads():
            nc.sync.dma_start(
                bqk_pp[:, :],
                ins["b_qkv"][0:2 * D].rearrange("(m p) -> p m", p=128))
            nc.sync.dma_start(
                bv_pp[:, :],
                ins["b_qkv"][2 * D:3 * D].rearrange("(h p) -> p h", p=72))
            nc.sync.dma_start(
                bproj_pp[:, :], ins["b_proj"].rearrange("(m p) -> p m", p=128))
            nc.sync.dma_start(
                bfc1_pp[:, :], ins["b_fc1"].rearrange("(m p) -> p m", p=128))
            nc.sync.dma_start(
                bfc2_pp[:, :], ins["b_fc2"].rearrange("(m p) -> p m", p=128))
            nc.sync.dma_start(
                bada_pp[:, :, :],
                ins["b_ada"].rearrange("(c k p) -> p c k", k=KT, p=128))
        xT = pers.tile([128, KT, NT], F32)   # becomes x2T after residual 1
        # weight-stream pool spanning all phases: lets the scheduler prefetch
        # the next phase's weights across pool boundaries
        pw_s = ctx.enter_context(tc.tile_pool(name="pw_s", bufs=1))

        # ============ phase 1: ada, x load+transpose, LN1 ====================
        es_mod1 = ExitStack()
        pmod1 = es_mod1.enter_context(tc.tile_pool(name="pmod1", bufs=1))
        mod1T = pmod1.tile([128, KT, NT], BF16, name="mod1T")

        with tc.tile_pool(name="p1w", bufs=1) as p1w, \
             tc.tile_pool(name="pst", bufs=1) as pst, \
             tc.tile_pool(name="pln", bufs=1) as pln:
            with tc.tile_pool(name="ps_pro", bufs=2, space="PSUM") as ps_pro, \
                 tc.tile_pool(name="pxin", bufs=3) as pxin, \
                 tc.tile_pool(name="ps_tr", bufs=2, space="PSUM") as ps_tr:

                def emit_transpose_block(tt):
                    xin = pxin.tile([128, D], F32, tag="xin", name="xin")
                    nc.sync.dma_start(
                        xin[:, :], ins["x"][tt * 128:(tt + 1) * 128, :])
                    for kd in range(KT):
                        pt = ps_tr.tile([128, 128], F32, tag="ptr", name="pt")
                        nc.tensor.transpose(
                            pt[:, :], xin[:, kd * 128:(kd + 1) * 128],
                            ident[:, :],
                        )
                        tsl = slice(tt * 128, (tt + 1) * 128)
                        if kd % 2 == 0:
                            nc.vector.tensor_copy(xT[:, kd, tsl], pt[:, :])
                        else:
                            nc.scalar.copy(xT[:, kd, tsl], pt[:, :])

                def emit_ada_chunk(n):
                    pa = ps_pro.tile([1, 384], F32, tag="psada", name="pa")
                    for k in range(KT):
                        wada_t = p1w.tile([128, 384], F32, tag="wsk", bufs=4,
                                          name="wada_t")
                        nc.sync.dma_start(
                            wada_t[:, :],
                            ins["w_ada"][k * 128:(k + 1) * 128,
                                         n * 384:(n + 1) * 384],
                        )
                        wada_b = p1w.tile([128, 384], BF16, tag="wskb", bufs=4,
                                          name="wada_b")
                        if k % 2 == 0:
                            nc.gpsimd.tensor_copy(wada_b[:, :], wada_t[:, :])
                        else:
                            nc.vector.tensor_copy(wada_b[:, :], wada_t[:, :])
                        nc.tensor.matmul(
                            pa[:, :], t_pb[:, k:k + 1], wada_b[:, :],
                            start=(k == 0), stop=(k == KT - 1),
                        )
                    asb = pst.tile([1, 384], F32, tag="asb", bufs=2, name="asb")
                    nc.vector.tensor_copy(asb[:, :], pa[:, :])
                    nc.sync.dma_start(
                        ada_dr[n * 384:(n + 1) * 384]
                        .rearrange("(a b) -> a b", a=1),
                        asb[0:1, :],
                    )

                # interleave: ada chunk n and transpose block(s) alternate so
                # the PE fills DMA wait time of one with the other
                for i in range(8):
                    emit_transpose_block(i)
                    if i < 6:
                        emit_ada_chunk(i)
                    if i == 0:
                        emit_bias_loads()
                for c in range(2):
                    nc.sync.dma_start(
                        ada_pp[:, c, :],
                        ada_dr[c * D:(c + 1) * D].rearrange("(k p) -> p k", p=128),
                    )
                nc.vector.tensor_add(ada_pp[:, 0:2, :], ada_pp[:, 0:2, :],
                                     bada_pp[:, 0:2, :])
                nc.vector.tensor_scalar_add(ada_pp[:, 1, :], ada_pp[:, 1, :], 1.0)

            if phase_limit > 0.6:
              with tc.tile_pool(name="ps_st", bufs=4, space="PSUM") as ps_st, \
                 tc.tile_pool(name="ps_bc", bufs=2, space="PSUM") as ps_bc:
                _ln_stats_and_modulate(
                    tc, nc, xT, mod1T, ada_pp, 0, 1, ones_col,
                    pst, pln, ps_st,
                )

        if phase_limit <= 1:
            es_mod1.close()
            return _truncate_out(tc, nc, out_dram)

        # ============ phase 2: qkv ==========================================
        es_qkv = ExitStack()
        pqks = es_qkv.enter_context(tc.tile_pool(name="pqks", bufs=1, side="right"))
        qk_st = pqks.tile([128, MQK, NT], BF16, name="qk_st")
        pvaug = es_qkv.enter_context(
            tc.tile_pool(name="pvaug", bufs=1, side="right"))
        # per head: cols 0..72 = v, col 96 = ones (sum row lands on an
        # aligned PSUM partition), cols 72..96 zero padding
        v_aug = pvaug.tile([128, NT // 128, H, 97], BF16, name="v_aug")
        nc.gpsimd.memset(v_aug[:, :, :, HD:97], 0.0)
        nc.gpsimd.memset(v_aug[:, :, :, 96:97], 1.0)

        with tc.tile_pool(name="p2w", bufs=1) as p2w, \
             tc.tile_pool(name="ps_mm", bufs=4, space="PSUM") as ps_mm:
            for mo in range(MQK):
                wqk_t = pw_s.tile([128, KT, 128], F32, tag="ws", bufs=3,
                                  name="wqk_t")
                nc.sync.dma_start(
                    wqk_t[:, :, :],
                    ins["w_qkv"][:, mo * 128:(mo + 1) * 128]
                    .rearrange("(k p) m -> p k m", p=128),
                )
                wqk_b = pw_s.tile([128, KT, 128], BF16, tag="wsb", bufs=3,
                                  name="wqk_b")
                nc.gpsimd.tensor_copy(wqk_b[:, :, :], wqk_t[:, :, :])
                for n in range(2):
                    pm = ps_mm.tile([128, 512], F32, tag="mm", name="pm")
                    for k in range(KT):
                        nc.tensor.matmul(
                            pm[:, :], wqk_b[:, k, :],
                            mod1T[:, k, n * 512:(n + 1) * 512],
                            start=(k == 0), stop=(k == KT - 1),
                        )
                    nc.scalar.activation(
                        qk_st[:, mo, n * 512:(n + 1) * 512], pm[:, :],
                        AF.Identity, bias=bqk_pp[:, mo:mo + 1], scale=1.0,
                    )
            for (c0, c1, h0, h1) in V_SLICES:
                wv_t = p2w.tile([128, KT, 432], F32, tag="wv", bufs=2,
                                name="wv_t")
                nc.sync.dma_start(
                    wv_t[:, :, 0:c1 - c0],
                    ins["w_qkv"][:, 2 * D + c0:2 * D + c1]
                    .rearrange("(k p) m -> p k m", p=128),
                )
                wv_b = p2w.tile([128, KT, 432], BF16, tag="wvb", bufs=2,
                                name="wv_b")
                nc.gpsimd.tensor_copy(wv_b[:, :, 0:c1 - c0], wv_t[:, :, 0:c1 - c0])
                for tt in range(NT // 128):
                    pmv = ps_mm.tile([128, 512], F32, tag="mm", name="pmv")
                    for k in range(KT):
                        nc.tensor.matmul(
                            pmv[:, 0:c1 - c0],
                            mod1T[:, k, tt * 128:(tt + 1) * 128],
                            wv_b[:, k, 0:c1 - c0],
                            start=(k == 0), stop=(k == KT - 1),
                        )
                    for h in range(h0, h1):
                        nc.vector.tensor_copy(
                            v_aug[:, tt, h, 0:HD],
                            pmv[:, h * HD - c0:(h + 1) * HD - c0],
                        )
        es_mod1.close()
        if phase_limit <= 2:
            es_qkv.close()
            return _truncate_out(tc, nc, out_dram)

        # ============ phase 3: attention ====================================
        es_ao = ExitStack()
        pastk = es_ao.enter_context(tc.tile_pool(name="pastk", bufs=1))
        attn_st = pastk.tile([128, KT, NT], BF16, name="attn_st")

        with tc.tile_pool(name="pheads", bufs=2) as pheads, \
             tc.tile_pool(name="pexp", bufs=3) as pexp, \
             tc.tile_pool(name="pattn", bufs=2) as pattn, \
             tc.tile_pool(name="p3w", bufs=1) as p3w, \
             tc.tile_pool(name="ps_s", bufs=3, space="PSUM") as ps_s, \
             tc.tile_pool(name="ps_av", bufs=4, space="PSUM") as ps_av:
            ps_a3 = ps_s  # [1,384] ada tiles share the pool (own tag, 1 buf)

            def emit_late_ada_chunk(n):
                pa = ps_a3.tile([1, 384], F32, tag="psada3", bufs=1,
                                name="pa3")
                for k in range(KT):
                    wada_t = p3w.tile([128, 384], F32, tag="wsk", bufs=4,
                                      name="wada_t3")
                    nc.sync.dma_start(
                        wada_t[:, :],
                        ins["w_ada"][k * 128:(k + 1) * 128,
                                     n * 384:(n + 1) * 384],
                    )
                    wada_b = p3w.tile([128, 384], BF16, tag="wskb", bufs=3,
                                      name="wada_b3")
                    nc.gpsimd.tensor_copy(wada_b[:, :], wada_t[:, :])
                    nc.tensor.matmul(
                        pa[:, :], t_pb[:, k:k + 1], wada_b[:, :],
                        start=(k == 0), stop=(k == KT - 1),
                    )
                asb = p3w.tile([1, 384], F32, tag="asb", bufs=1, name="asb3")
                nc.vector.tensor_copy(asb[:, :], pa[:, :])
                nc.sync.dma_start(
                    ada_dr[n * 384:(n + 1) * 384]
                    .rearrange("(a b) -> a b", a=1),
                    asb[0:1, :],
                )

            def emit_w2_convert(k):
                w2src = p3w.tile([128, D], F32, tag="w2src", bufs=2,
                                 name="w2src")
                nc.sync.dma_start(
                    w2src[:, :], ins["w_fc2"][k * 128:(k + 1) * 128, :]
                )
                w2b = p3w.tile([128, D], BF16, tag="w2b", bufs=2, name="w2b")
                nc.vector.tensor_copy(w2b[:, :], w2src[:, :])
                nc.sync.dma_start(w2_dr[k * 128:(k + 1) * 128, :], w2b[:, :])

            def emit_filler(h):
                # spread late-ada (12 chunks) and w2 conversion (36 blocks)
                # across the 16 head iterations
                if h < 12:
                    emit_late_ada_chunk(6 + h)
                if h == 11:
                    for c in range(2, 6):
                        nc.sync.dma_start(
                            ada_pp[:, c, :],
                            ada_dr[c * D:(c + 1) * D]
                            .rearrange("(k p) -> p k", p=128),
                        )
                    nc.vector.tensor_add(ada_pp[:, 2:6, :], ada_pp[:, 2:6, :],
                                         bada_pp[:, 2:6, :])
                    nc.vector.tensor_scalar_add(ada_pp[:, 4, :],
                                                ada_pp[:, 4, :], 1.0)
                for k2 in range((h * 36) // H, ((h + 1) * 36) // H):
                    emit_w2_convert(k2)

            for h in range(H):
                emit_filler(h)
                q_h = pheads.tile([72, NT], BF16, tag="qh", name="q_h")
                k_h = pheads.tile([72, NT], BF16, tag="kh", name="k_h")
                for (kt_i, p0, ln, off) in _head_segs(h * HD, HD):
                    nc.sync.dma_start(
                        q_h[off:off + ln, :], qk_st[p0:p0 + ln, kt_i, :]
                    )
                for (kt_i, p0, ln, off) in _head_segs(D + h * HD, HD):
                    nc.sync.dma_start(
                        k_h[off:off + ln, :], qk_st[p0:p0 + ln, kt_i, :]
                    )
                attn_f = pattn.tile([72, NT], F32, tag="attnf", bufs=1,
                                    name="attn_f")
                attn_h = pattn.tile([72, NT], BF16, tag="attnh", name="attn_h")
                for n in range(2):
                    nsl = slice(n * 512, (n + 1) * 512)
                    exp_hn = pexp.tile([128, NT // 128, 512], BF16, tag="exp",
                                       bufs=3, name="exp_hn")
                    for kt_i in range(NT // 128):
                        pss = ps_s.tile([128, 512], F32, tag="s", name="pss")
                        nc.tensor.matmul(
                            pss[:, :], k_h[:, kt_i * 128:(kt_i + 1) * 128],
                            q_h[:, nsl], start=True, stop=True,
                        )
                        nc.scalar.activation(
                            exp_hn[:, kt_i, :], pss[:, :], AF.Exp, scale=ISC
                        )
                    pav = ps_av.tile([97, 512], F32, tag="av", name="pav")
                    for kt_i in range(NT // 128):
                        nc.tensor.matmul(
                            pav[:, :], v_aug[:, kt_i, h, :], exp_hn[:, kt_i, :],
                            start=(kt_i == 0), stop=(kt_i == NT // 128 - 1),
                        )
                    recip = pattn.tile([1, 512], F32, tag="recip", bufs=2,
                                       name="recip")
                    nc.vector.reciprocal(recip[:, :], pav[96:97, :])
                    bca = pattn.tile([72, 512], F32, tag="bca", name="bca")
                    nc.gpsimd.partition_broadcast(bca[:, :], recip[:, :])
                    nc.vector.tensor_mul(attn_f[:, nsl], pav[0:72, :], bca[:, :])
                    nc.vector.tensor_scalar_add(
                        attn_h[:, nsl], attn_f[:, nsl], bv_pp[:, h:h + 1]
                    )
                for (kt_i, p0, ln, off) in _head_segs(h * HD, HD):
                    nc.sync.dma_start(
                        attn_st[p0:p0 + ln, kt_i, :], attn_h[off:off + ln, :]
                    )
        es_qkv.close()
        if phase_limit <= 3:
            es_ao.close()
            return _truncate_out(tc, nc, out_dram)

        # ============ phase 4: proj + residual1 + LN2 ========================
        es_mod2 = ExitStack()
        pmod2 = es_mod2.enter_context(
            tc.tile_pool(name="pmod2", bufs=1, side="right"))
        mod2T = pmod2.tile([128, KT, NT], BF16, name="mod2T")

        with tc.tile_pool(name="p4w", bufs=1) as p4w, \
             tc.tile_pool(name="pst4", bufs=1) as pst4, \
             tc.tile_pool(name="pln4", bufs=1) as pln4:
            with tc.tile_pool(name="ps_mm2", bufs=4, space="PSUM") as ps_mm2:
                for mo in range(KT):
                    wp_f = pw_s.tile([128, KT, 128], F32, tag="ws", bufs=3,
                                     name="wp_f")
                    nc.sync.dma_start(
                        wp_f[:, :, :],
                        ins["w_proj"][:, mo * 128:(mo + 1) * 128]
                        .rearrange("(k p) m -> p k m", p=128),
                    )
                    wp_b = pw_s.tile([128, KT, 128], BF16, tag="wsb", bufs=3,
                                     name="wp_b")
                    nc.gpsimd.tensor_copy(wp_b[:, :, :], wp_f[:, :, :])
                    for n in range(2):
                        nsl = slice(n * 512, (n + 1) * 512)
                        pm2 = ps_mm2.tile([128, 512], F32, tag="mm2", name="pm2")
                        for k in range(KT):
                            nc.tensor.matmul(
                                pm2[:, :], wp_b[:, k, :], attn_st[:, k, nsl],
                                start=(k == 0), stop=(k == KT - 1),
                            )
                        t_sb = p4w.tile([128, 512], F32, tag="tsb", bufs=2,
                                        name="t_sb")
                        nc.scalar.activation(
                            t_sb[:, :], pm2[:, :], AF.Identity,
                            bias=bproj_pp[:, mo:mo + 1], scale=1.0,
                        )
                        nc.vector.scalar_tensor_tensor(
                            xT[:, mo, nsl], t_sb[:, :], ada_pp[:, 2, mo:mo + 1],
                            xT[:, mo, nsl], ALU.mult, ALU.add,
                        )


            with tc.tile_pool(name="ps_st2", bufs=4, space="PSUM") as ps_st2, \
                 tc.tile_pool(name="ps_bc2", bufs=2, space="PSUM") as ps_bc2:
                _ln_stats_and_modulate(
                    tc, nc, xT, mod2T, ada_pp, 3, 4, ones_col,
                    pst4, pln4, ps_st2,
                )
        es_ao.close()
        if phase_limit <= 4:
            es_mod2.close()
            return _truncate_out(tc, nc, out_dram)

        # ============ phase 5: FFN ==========================================
        es_o = ExitStack()
        po = es_o.enter_context(tc.tile_pool(name="po", bufs=1))
        o_full = po.tile([128, KT, NT], F32, name="o_full")

        with tc.tile_pool(name="p5w", bufs=1) as p5w, \
             tc.tile_pool(name="ph", bufs=1) as ph, \
             tc.tile_pool(name="p5h", bufs=1) as p5h, \
             tc.tile_pool(name="ps_f1", bufs=2, space="PSUM") as ps_f1, \
             tc.tile_pool(name="ps_f2", bufs=4, space="PSUM") as ps_f2:
            hT_sb = p5h.tile([128, MH, NT], BF16, name="hT_sb")
            for mo in range(MH):
                wf1_t = pw_s.tile([128, KT, 128], F32, tag="ws", bufs=3,
                                  name="wf1_t")
                nc.sync.dma_start(
                    wf1_t[:, :, :],
                    ins["w_fc1"][:, mo * 128:(mo + 1) * 128]
                    .rearrange("(k p) m -> p k m", p=128),
                )
                wf1_b = pw_s.tile([128, KT, 128], BF16, tag="wsb", bufs=3,
                                  name="wf1_b")
                nc.gpsimd.tensor_copy(wf1_b[:, :, :], wf1_t[:, :, :])
                for n in range(2):
                    pf1 = ps_f1.tile([128, 512], F32, tag="f1", name="pf1")
                    for k in range(KT):
                        nc.tensor.matmul(
                            pf1[:, :], wf1_b[:, k, :],
                            mod2T[:, k, n * 512:(n + 1) * 512],
                            start=(k == 0), stop=(k == KT - 1),
                        )
                    nc.scalar.activation(
                        hT_sb[:, mo, n * 512:(n + 1) * 512], pf1[:, :],
                        AF.Gelu_apprx_tanh,
                        bias=bfc1_pp[:, mo:mo + 1], scale=1.0,
                    )
            # fc2 in groups of 2 m-tiles (4 psum banks) so 2 banks remain
            # for output transposes interleaved right behind each group
            for ms in ([0, 1], [2, 3], [4, 5], [6, 7], [8]):
                pms = {}
                for m in ms:
                    for n in range(2):
                        pms[(m, n)] = ps_f2.tile(
                            [128, 512], F32, tag="f2", bufs=4,
                            name=f"f2_{m}_{n}"
                        )
                w = 128 * len(ms)
                for k in range(MH):
                    w2_rd = p5w.tile([128, 384], BF16, tag="w2rd", bufs=8,
                                     name="w2_rd")
                    nc.sync.dma_start(
                        w2_rd[:, 0:w],
                        w2_dr[k * 128:(k + 1) * 128,
                              ms[0] * 128:ms[0] * 128 + w],
                    )
                    for n in range(2):
                        for i, m in enumerate(ms):
                            nc.tensor.matmul(
                                pms[(m, n)][:, :],
                                w2_rd[:, i * 128:(i + 1) * 128],
                                hT_sb[:, k, n * 512:(n + 1) * 512],
                                start=(k == 0), stop=(k == MH - 1),
                                skip_group_check=True,
                            )
                for m in ms:
                    for n in range(2):
                        nsl = slice(n * 512, (n + 1) * 512)
                        t2 = p5w.tile([128, 512], F32, tag="tsb", bufs=3,
                                      name="t2")
                        nc.scalar.activation(
                            t2[:, :], pms[(m, n)][:, :], AF.Identity,
                            bias=bfc2_pp[:, m:m + 1], scale=1.0,
                        )
                        nc.vector.scalar_tensor_tensor(
                            o_full[:, m, nsl], t2[:, :],
                            ada_pp[:, 5, m:m + 1], xT[:, m, nsl],
                            ALU.mult, ALU.add,
                        )
                    for tt in range(NT // 128):
                        pt = ps_f2.tile([128, 128], F32, tag="tro", bufs=2,
                                        name="pt6")
                        nc.tensor.transpose(
                            pt[:, :], o_full[:, m, tt * 128:(tt + 1) * 128],
                            ident[:, :],
                        )
                        ot = ph.tile([128, 128], F32, tag="ot", bufs=4,
                                     name="ot")
                        if tt % 2 == 0:
                            nc.vector.tensor_copy(ot[:, :], pt[:, :])
                        else:
                            nc.scalar.copy(ot[:, :], pt[:, :])
                        nc.sync.dma_start(
                            out_dram[tt * 128:(tt + 1) * 128,
                                     m * 128:(m + 1) * 128],
                            ot[:, :],
                        )
        es_mod2.close()
        es_o.close()


_LOCK = threading.Lock()
_PROG = None


def _get_program():
    global _PROG
    with _LOCK:
        if _PROG is None:
            _PROG = _build_program()
    return _PROG


def _make_in_maps(inputs):
    arrs = {k: np.ascontiguousarray(np.asarray(v, dtype=np.float32))
            for k, v in inputs.items()}
    in_maps = []
    for c in range(NCORES):
        m = {k: v for k, v in arrs.items() if k not in ("x", "t_emb")}
        m["x"] = np.ascontiguousarray(arrs["x"][c])
        m["t_emb"] = np.ascontiguousarray(arrs["t_emb"][c])
        in_maps.append(m)
    return in_maps


def kernel(**inputs):
    nc = _get_program()
    res = run_bass_kernel_spmd(nc, _make_in_maps(inputs), core_ids=list(range(NCORES)))
    return np.stack([r["out"] for r in res.results], axis=0)


def kernel_traced(inputs, **kw):
    """test-harness helper: returns full BassKernelResults with trace."""
    nc = _get_program()
    return run_bass_kernel_spmd(
        nc, _make_in_maps(inputs), core_ids=list(range(NCORES)), trace=True, **kw
    )



# revision 60
# speedup vs baseline: 1.5562x; 1.5562x over previous
"""DiT block kernel for Trainium2 (Bass/Tile), 8-core data parallel.

Shapes (hardcoded from the problem spec):
  x: (8, 1024, 1152), t_emb: (8, 1152)
  w_qkv (1152, 3456), w_proj (1152, 1152), w_fc1 (1152, 4608),
  w_fc2 (4608, 1152), w_ada (1152, 6912) + biases.

Strategy: batch-parallel across 8 cores (one batch element each).
Activations live transposed [D on partitions, tokens free]; projections are
out_T = W.T @ x_T with lhsT = W as stored.  All large GEMMs run in fp8e4
DoubleRow (two 128-deep k-tiles per PE pass); weights are scaled by 32 into
fp8 to stay in the normal range, compensated in the PSUM epilogue scale.
LayerNorm stats use f32r ones-column matmuls; softmax runs transposed with
no max subtraction and a ones-column appended to V for the denominators.
adaLN is contraction-split across the 8 cores (each core loads 1/8 of
w_ada, computes partial rows for all batches, then one small AllToAll
redistributes) -- gated by BASS_ADA_SPLIT.
"""

import os
import threading
from contextlib import ExitStack

import numpy as np

import concourse.bass as bass
import concourse.mybir as mybir
import concourse.tile as tile
from concourse import bacc
from concourse.bass_utils import run_bass_kernel_spmd
from concourse.masks import make_identity

F32 = mybir.dt.float32
F32R = mybir.dt.float32r
BF16 = mybir.dt.bfloat16
FP8 = mybir.dt.float8e4
AF = mybir.ActivationFunctionType
ALU = mybir.AluOpType
DR = mybir.MatmulPerfMode.DoubleRow

NCORES = 8
D = 1152
NT = 1024          # tokens per core (batch element)
KT = D // 128      # 9 partition-tiles of D
H = 16
HD = 72
HID = 4 * D        # 4608
MH = HID // 128    # 36
EPS = 1e-6
ISC = 1.0 / float(np.sqrt(HD))
WS = 32.0          # weight upscale into fp8e4 (avoids subnormals)
WSI = 1.0 / WS

ADA_SPLIT = os.environ.get("BASS_ADA_SPLIT", "1") == "1"
ADA_COLS = 6 * D // NCORES   # 864 columns of w_ada per core when split

# v output column slices aligned to head boundaries
V_SLICES = [(0, 432, 0, 6), (432, 864, 6, 12), (864, 1152, 12, 16)]


def _r(ap):
    return ap.bitcast(F32R)


def _head_segs(d0, n):
    """Split logical rows [d0, d0+n) of a [*,128]-tiled stacked tensor into
    (ktile, part0, length, dst_offset) segments within 128-partition tiles."""
    segs = []
    off = 0
    while n > 0:
        kt_i, p0 = divmod(d0, 128)
        ln = min(n, 128 - p0)
        segs.append((kt_i, p0, ln, off))
        d0 += ln
        off += ln
        n -= ln
    return segs


def _build_program():
    nc = bacc.Bacc(
        "TRN2", target_bir_lowering=False, debug=False, enable_asserts=False,
        num_devices=NCORES,
    )
    ins = {}
    ins["x"] = nc.dram_tensor("x", [NT, D], F32, kind="ExternalInput").ap()
    if ADA_SPLIT:
        ins["t_emb"] = nc.dram_tensor(
            "t_emb", [NCORES, D], F32, kind="ExternalInput").ap()
        ins["w_ada"] = nc.dram_tensor(
            "w_ada", [D, ADA_COLS], F32, kind="ExternalInput").ap()
    else:
        ins["t_emb"] = nc.dram_tensor(
            "t_emb", [D], F32, kind="ExternalInput").ap()
        ins["w_ada"] = nc.dram_tensor(
            "w_ada", [D, 6 * D], F32, kind="ExternalInput").ap()
    for name, shape in [
        ("w_qkv", [D, 3 * D]), ("b_qkv", [3 * D]),
        ("w_proj", [D, D]), ("b_proj", [D]),
        ("w_fc1", [D, HID]), ("b_fc1", [HID]),
        ("w_fc2", [HID, D]), ("b_fc2", [D]),
        ("b_ada", [6 * D]),
    ]:
        ins[name] = nc.dram_tensor(name, shape, F32, kind="ExternalInput").ap()
    out_dram = nc.dram_tensor("out", [NT, D], F32, kind="ExternalOutput").ap()

    with tile.TileContext(nc) as tc:
        _body(tc, ins, out_dram)
    nc.compile()
    return nc


def _ln_mod(tc, nc, src, dst, ada_pp, shift_c, scale_c, ones_col,
            pst, pln, ps_st):
    """dst[:,k,:] = fp8((src-mean)*rstd * ada[scale_c] + ada[shift_c]).
    Stats over the partition (D) axis per token via f32r ones matmuls."""
    ps_x, ps_q, st = {}, {}, {}
    for n in range(2):
        nsl = slice(n * 512, (n + 1) * 512)
        ps_x[n] = ps_st.tile([1, 512], F32, tag="st", name=f"psx{n}")
        ps_q[n] = ps_st.tile([1, 512], F32, tag="st", name=f"psq{n}")
        for k in range(KT):
            xb = pln.tile([128, 512], BF16, tag="xb", bufs=2, name="xb")
            nc.scalar.copy(xb[:, :], src[:, k, nsl])
            sq = pln.tile([128, 512], BF16, tag="sqb", bufs=2, name="sq")
            nc.vector.tensor_mul(sq[:, :], xb[:, :], xb[:, :])
            nc.tensor.matmul(
                ps_x[n][:, :], ones_col[:, :], xb[:, :],
                start=(k == 0), stop=(k == KT - 1), skip_group_check=True,
            )
            nc.tensor.matmul(
                ps_q[n][:, :], ones_col[:, :], sq[:, :],
                start=(k == 0), stop=(k == KT - 1), skip_group_check=True,
            )
    eps_sb = pst.tile([1, 1], F32, tag="eps", bufs=1, name="eps_sb")
    nc.vector.memset(eps_sb[:, :], EPS)
    for n in range(2):
        # rows: 0 = mean, 1 = E[x^2] -> rstd
        st[n] = pst.tile([1, 2, 512], F32, tag="lnst", bufs=2, name=f"st{n}")
        nc.vector.tensor_scalar_mul(st[n][:, 0, :], ps_x[n][:, :], 1.0 / D)
        work = pst.tile([1, 512], F32, tag="lnwork", bufs=2, name="work")
        nc.vector.tensor_mul(work[:, :], st[n][:, 0, :], st[n][:, 0, :])
        nc.vector.scalar_tensor_tensor(
            st[n][:, 1, :], ps_q[n][:, :], 1.0 / D, work[:, :],
            ALU.mult, ALU.subtract,
        )
        nc.scalar.activation(st[n][:, 1, :], st[n][:, 1, :], AF.Sqrt,
                             bias=eps_sb[:, :], scale=1.0)
        nc.vector.reciprocal(st[n][:, 1, :], st[n][:, 1, :])
    for n in range(2):
        nsl = slice(n * 512, (n + 1) * 512)
        meanB = pln.tile([128, 512], F32, tag="meanB", bufs=2, name="meanB")
        rstdB = pln.tile([128, 512], F32, tag="rstdB", bufs=2, name="rstdB")
        nc.gpsimd.partition_broadcast(meanB[:, :], st[n][:, 0, :])
        nc.gpsimd.partition_broadcast(rstdB[:, :], st[n][:, 1, :])
        for k in range(KT):
            tmp = pln.tile([128, 512], F32, tag="lnt", bufs=2, name="tmp")
            nc.vector.tensor_sub(tmp[:, :], src[:, k, nsl], meanB[:, :])
            nc.vector.tensor_mul(tmp[:, :], tmp[:, :], rstdB[:, :])
            nc.gpsimd.tensor_scalar(
                dst[:, k, nsl], tmp[:, :],
                ada_pp[:, scale_c, k:k + 1], ada_pp[:, shift_c, k:k + 1],
                ALU.mult, ALU.add,
            )


def _truncate_out(tc, nc, out_dram):
    with tc.tile_pool(name="ptrunc", bufs=1) as p:
        z = p.tile([128, D], F32, name="z")
        nc.vector.memset(z[:, :], 0.0)
        for tt in range(NT // 128):
            nc.sync.dma_start(out_dram[tt * 128:(tt + 1) * 128, :], z[:, :])


def _body(tc, ins, out_dram):
    nc = tc.nc
    phase_limit = float(os.environ.get("BASS_PHASES", "6"))
    ctx = ExitStack()
    with ctx:
        dram = ctx.enter_context(tc.tile_pool(name="dram", bufs=1, space="DRAM"))
        if ADA_SPLIT:
            ada_part_dr = dram.tile([NCORES * ADA_COLS], F32)
            ada_gath_dr = dram.tile([NCORES * ADA_COLS], F32)
        else:
            ada_dr = dram.tile([6 * D], F32)

        pers = ctx.enter_context(tc.tile_pool(name="pers", bufs=1))
        ident = pers.tile([128, 128], F32)
        make_identity(nc, ident[:, :])
        ones_col = pers.tile([128, 1], BF16)
        nc.vector.memset(ones_col[:, :], 1.0)

        # silu(t_emb): split case holds all 8 batches, else just our own
        NB = NCORES if ADA_SPLIT else 1
        t_pp = pers.tile([128, KT, NB], F32)
        if ADA_SPLIT:
            for b in range(NCORES):
                nc.sync.dma_start(
                    t_pp[:, :, b],
                    ins["t_emb"][b, :].rearrange("(k p) -> p k", p=128))
        else:
            nc.sync.dma_start(
                t_pp[:, :, 0], ins["t_emb"].rearrange("(k p) -> p k", p=128))
        t_sb = pers.tile([128, KT, NB], BF16)
        nc.scalar.activation(t_sb[:, :, :], t_pp[:, :, :], AF.Silu)

        bqk72 = pers.tile([72, 32], F32)      # q,k bias per 72-head chunk
        bv_pp = pers.tile([72, H], F32)
        bproj_pp = pers.tile([128, KT], F32)
        bfc1_pp = pers.tile([128, MH], F32)
        bfc2_pp = pers.tile([128, KT], F32)
        bada_pp = pers.tile([128, 6, KT], F32)
        ada_pp = pers.tile([128, 6, KT], F32)

        def emit_bias_loads():
            nc.sync.dma_start(
                bqk72[:, :],
                ins["b_qkv"][0:2 * D].rearrange("(c p) -> p c", p=72))
            nc.sync.dma_start(
                bv_pp[:, :],
                ins["b_qkv"][2 * D:3 * D].rearrange("(h p) -> p h", p=72))
            nc.sync.dma_start(
                bproj_pp[:, :], ins["b_proj"].rearrange("(m p) -> p m", p=128))
            nc.sync.dma_start(
                bfc1_pp[:, :], ins["b_fc1"].rearrange("(m p) -> p m", p=128))
            nc.sync.dma_start(
                bfc2_pp[:, :], ins["b_fc2"].rearrange("(m p) -> p m", p=128))
            nc.sync.dma_start(
                bada_pp[:, :, :],
                ins["b_ada"].rearrange("(c k p) -> p c k", k=KT, p=128))

        xT = pers.tile([128, KT, NT], F32)   # becomes x2T, then out_T
        w2_sb = pers.tile([128, MH, D], FP8)  # fc2 weights, fp8*WS
        # weight-stream pool spanning phases 4-5 (proj/fc1 prefetch)
        pw_s = ctx.enter_context(tc.tile_pool(name="pw_s", bufs=1))
        # attn output store: allocated early so attention-phase pools do not
        # sit in freed qkv space (space-reuse would serialize the phases)
        es_ao = ExitStack()
        pastk = es_ao.enter_context(tc.tile_pool(name="pastk", bufs=1))
        attn_st = pastk.tile([128, KT, NT], FP8, name="attn_st")
        # qkv weight pool: lives through attention (closed at phase 4)
        es_qw = ExitStack()
        pqw = es_qw.enter_context(tc.tile_pool(name="pqw", bufs=1))

        # ============ phase 1: ada, x load+transpose, LN1 ====================
        es_mod1 = ExitStack()
        pmod1 = es_mod1.enter_context(tc.tile_pool(name="pmod1", bufs=1))
        mod1T = pmod1.tile([128, KT, NT], FP8, name="mod1T")

        with tc.tile_pool(name="p1w", bufs=1) as p1w, \
             tc.tile_pool(name="pst", bufs=1) as pst, \
             tc.tile_pool(name="pln", bufs=1) as pln:
            with tc.tile_pool(name="ps_pro", bufs=2, space="PSUM") as ps_pro, \
                 tc.tile_pool(name="pxin", bufs=3) as pxin, \
                 tc.tile_pool(name="ps_tr", bufs=2, space="PSUM") as ps_tr:

                def emit_transpose_block(tt):
                    xin = pxin.tile([128, D], F32, tag="xin", bufs=2,
                                    name="xin")
                    nc.sync.dma_start(
                        xin[:, :], ins["x"][tt * 128:(tt + 1) * 128, :])
                    for kd in range(KT):
                        pt = ps_tr.tile([128, 128], F32, tag="ptr", name="pt")
                        nc.tensor.transpose(
                            pt[:, :], xin[:, kd * 128:(kd + 1) * 128],
                            ident[:, :],
                        )
                        tsl = slice(tt * 128, (tt + 1) * 128)
                        if kd % 2 == 0:
                            nc.vector.tensor_copy(xT[:, kd, tsl], pt[:, :])
                        else:
                            nc.scalar.copy(xT[:, kd, tsl], pt[:, :])

                def emit_ada_split():
                    # partial rows for ALL batches over our 1/8 of w_ada,
                    # then AllToAll redistributes so each core gets its row.
                    ada_sb = pst.tile([NCORES, ADA_COLS], F32, tag="adasb",
                                      bufs=1, name="ada_sb")
                    for c in range(2):
                        csl = slice(c * 432, (c + 1) * 432)
                        wada_t = p1w.tile([128, KT, 432], F32, tag="wada",
                                          bufs=1, name="wada_t")
                        nc.sync.dma_start(
                            wada_t[:, :, :],
                            ins["w_ada"][:, csl]
                            .rearrange("(k p) m -> p k m", p=128),
                        )
                        wada_b = p1w.tile([128, KT, 432], BF16, tag="wadab",
                                          bufs=1, name="wada_b")
                        nc.gpsimd.tensor_copy(wada_b[:, :, :], wada_t[:, :, :])
                        pa = ps_pro.tile([NCORES, 432], F32, tag="psada",
                                         bufs=2, name="pa")
                        for k in range(KT):
                            nc.tensor.matmul(
                                pa[:, :], t_sb[:, k, :], wada_b[:, k, :],
                                start=(k == 0), stop=(k == KT - 1),
                            )
                        nc.vector.tensor_copy(ada_sb[:, csl], pa[:, :])
                    nc.sync.dma_start(
                        ada_part_dr.opt().rearrange("(b m) -> b m", b=NCORES),
                        ada_sb[:, :])
                    nc.gpsimd.collective_compute(
                        "AllToAll", ALU.bypass,
                        replica_groups=[list(range(NCORES))],
                        ins=[ada_part_dr.opt()],
                        outs=[ada_gath_dr.opt()],
                    )
                    nc.sync.dma_start(
                        ada_pp[:, :, :],
                        ada_gath_dr.opt().rearrange(
                            "(c k p) -> p c k", c=6, k=KT, p=128),
                    )

                def emit_ada_chunk(n):
                    # fallback: full w_ada on-core, M=1 bf16 matmuls
                    pa = ps_pro.tile([1, 384], F32, tag="psada", name="pa")
                    for k in range(KT):
                        wada_t = p1w.tile([128, 384], F32, tag="wsk", bufs=4,
                                          name="wada_t")
                        nc.sync.dma_start(
                            wada_t[:, :],
                            ins["w_ada"][k * 128:(k + 1) * 128,
                                         n * 384:(n + 1) * 384],
                        )
                        wada_b = p1w.tile([128, 384], BF16, tag="wskb",
                                          bufs=4, name="wada_b")
                        nc.gpsimd.tensor_copy(wada_b[:, :], wada_t[:, :])
                        nc.tensor.matmul(
                            pa[:, :], t_sb[:, k, :], wada_b[:, :],
                            start=(k == 0), stop=(k == KT - 1),
                        )
                    asb = pst.tile([1, 384], F32, tag="asb", bufs=2, name="asb")
                    nc.vector.tensor_copy(asb[:, :], pa[:, :])
                    nc.sync.dma_start(
                        ada_dr[n * 384:(n + 1) * 384]
                        .rearrange("(a b) -> a b", a=1),
                        asb[0:1, :],
                    )

                emit_bias_loads()
                if ADA_SPLIT:
                    emit_ada_split()
                    for i in range(8):
                        emit_transpose_block(i)
                else:
                    for i in range(8):
                        emit_transpose_block(i)
                        if i < 8:
                            emit_ada_chunk(2 * i)
                            emit_ada_chunk(2 * i + 1)
                    for n in range(16, 18):
                        emit_ada_chunk(n)
                    nc.sync.dma_start(
                        ada_pp[:, :, :],
                        ada_dr.opt().rearrange(
                            "(c k p) -> p c k", c=6, k=KT, p=128),
                    )
                nc.vector.tensor_add(ada_pp[:, :, :], ada_pp[:, :, :],
                                     bada_pp[:, :, :])
                nc.vector.tensor_scalar_add(ada_pp[:, 1, :], ada_pp[:, 1, :],
                                            1.0)
                nc.vector.tensor_scalar_add(ada_pp[:, 4, :], ada_pp[:, 4, :],
                                            1.0)

            if phase_limit > 0.6:
                with tc.tile_pool(name="ps_st", bufs=4, space="PSUM") as ps_st:
                    _ln_mod(tc, nc, xT, mod1T, ada_pp, 0, 1, ones_col,
                            pst, pln, ps_st)

        if phase_limit <= 1:
            es_mod1.close()
            return _truncate_out(tc, nc, out_dram)

        # ============ phase 2: qkv ==========================================
        es_qk = ExitStack()
        pqks = es_qk.enter_context(tc.tile_pool(name="pqks", bufs=1, side="right"))
        # chunks 0..15 = q heads, 16..31 = k heads; fp8 true values
        qk_st = pqks.tile([72, 32, NT], FP8, name="qk_st")
        pvaug = es_qk.enter_context(
            tc.tile_pool(name="pvaug", bufs=1, side="right"))
        # per head: cols 0..72 = v (true values), col 96 = ones, 72..96 zero
        v_aug = pvaug.tile([128, NT // 128, H, 97], FP8, name="v_aug")
        nc.gpsimd.memset(v_aug[:, :, :, HD:97], 0.0)
        nc.gpsimd.memset(v_aug[:, :, :, 96:97], 1.0)

        def emit_w2_convert(k):
            w2src = pw_s.tile([128, D], F32, tag="w2src", bufs=2,
                              name="w2src")
            nc.sync.dma_start(
                w2src[:, :], ins["w_fc2"][k * 128:(k + 1) * 128, :]
            )
            nc.gpsimd.tensor_scalar_mul(w2_sb[:, k, :], w2src[:, :], WS)

        def mm_group(psl, lhs8, rhs8, rhs_k_of, N):
            """accumulate 9 k-tiles: 4 DoubleRow pairs + 1 plain fp8."""
            for kk in range(4):
                nc.tensor.matmul(
                    psl, lhs8(2 * kk, 2), rhs_k_of(2 * kk, 2),
                    start=(kk == 0), stop=False, perf_mode=DR,
                )
            nc.tensor.matmul(psl, lhs8(8, 1), rhs_k_of(8, 1),
                             start=False, stop=True)

        with tc.tile_pool(name="p2w", bufs=1) as p2w, \
             tc.tile_pool(name="ps_mm", bufs=4, space="PSUM") as ps_mm:

            def emit_qk_dh(sec, j):
                # sec 0 = q (w cols j*144), sec 1 = k (w cols 1152 + j*144)
                wq_t = pqw.tile([128, KT, 144], F32, tag="wsq", bufs=2,
                                name="wq_t")
                nc.sync.dma_start(
                    wq_t[:, :, :],
                    ins["w_qkv"][:, sec * D + j * 144:sec * D + (j + 1) * 144]
                    .rearrange("(k p) m -> p k m", p=128),
                )
                wq_8 = pqw.tile([128, KT, 144], FP8, tag="wsbq", bufs=2,
                                name="wq_8")
                nc.gpsimd.tensor_scalar_mul(wq_8[:, :, :], wq_t[:, :, :], WS)
                for i in range(2):
                    ch = 16 * sec + 2 * j + i
                    hsl = slice(72 * i, 72 * i + 72)
                    ps2 = ps_mm.tile([72, 1024], F32, tag="mm", bufs=2,
                                     name="ps2")
                    for n in range(2):
                        nsl = slice(n * 512, (n + 1) * 512)

                        def lhs8(k0, kn, hsl=hsl):
                            w = wq_8[:, k0:k0 + kn, hsl]
                            return w if kn == 2 else wq_8[:, k0, hsl]

                        def rhs8(k0, kn, nsl=nsl):
                            m = mod1T[:, k0:k0 + kn, nsl]
                            return m if kn == 2 else mod1T[:, k0, nsl]

                        mm_group(ps2[:, nsl], lhs8, rhs8, rhs8, 512)
                    nc.scalar.activation(
                        qk_st[:, ch, :], ps2[:, :], AF.Identity,
                        bias=bqk72[:, ch:ch + 1], scale=WSI,
                    )

            def emit_v_slice(si):
                (c0, c1, h0, h1) = V_SLICES[si]
                wv_t = p2w.tile([128, KT, 432], F32, tag="wv", bufs=1,
                                name="wv_t")
                nc.sync.dma_start(
                    wv_t[:, :, 0:c1 - c0],
                    ins["w_qkv"][:, 2 * D + c0:2 * D + c1]
                    .rearrange("(k p) m -> p k m", p=128),
                )
                wv_8 = p2w.tile([128, KT, 432], FP8, tag="wvb", bufs=2,
                                name="wv_8")
                nc.scalar.activation(wv_8[:, :, 0:c1 - c0],
                                     wv_t[:, :, 0:c1 - c0], AF.Identity,
                                     scale=WS)
                for tt in range(NT // 128):
                    pmv = ps_mm.tile([128, 512], F32, tag="mmv", bufs=2,
                                     name="pmv")

                    def lhsv(k0, kn, tt=tt):
                        m = mod1T[:, k0:k0 + kn, tt * 128:(tt + 1) * 128]
                        return m if kn == 2 else mod1T[:, k0, tt * 128:(tt + 1) * 128]

                    def rhsv(k0, kn, c0=c0, c1=c1):
                        w = wv_8[:, k0:k0 + kn, 0:c1 - c0]
                        return w if kn == 2 else wv_8[:, k0, 0:c1 - c0]

                    mm_group(pmv[:, 0:c1 - c0], lhsv, rhsv, rhsv, c1 - c0)
                    for h in range(h0, h1):
                        nc.vector.tensor_scalar_mul(
                            v_aug[:, tt, h, 0:HD],
                            pmv[:, h * HD - c0:(h + 1) * HD - c0], WSI,
                        )

            # interleave q/k head-pairs and v slices so attention on early
            # heads can start while later chunks are still being produced
            for j in range(8):
                emit_qk_dh(0, j)
                emit_qk_dh(1, j)
                if j in (1, 3, 5):
                    emit_v_slice({1: 0, 3: 1, 5: 2}[j])
        es_mod1.close()
        es_qw.close()
        if phase_limit <= 2:
            es_qk.close()
            return _truncate_out(tc, nc, out_dram)

        # ============ phase 3: attention ====================================
        es_ao = ExitStack()
        pastk = es_ao.enter_context(tc.tile_pool(name="pastk", bufs=1))
        attn_st = pastk.tile([128, KT, NT], FP8, name="attn_st")

        with tc.tile_pool(name="pexp", bufs=2) as pexp, \
             tc.tile_pool(name="pattn", bufs=2) as pattn, \
             tc.tile_pool(name="ps_s", bufs=3, space="PSUM") as ps_s, \
             tc.tile_pool(name="ps_av", bufs=2, space="PSUM") as ps_av:
            for h in range(H):
                exp_h = pexp.tile([128, NT // 128, NT], FP8, tag="exp",
                                  name="exp_h")
                for kt_i in range(NT // 128):
                    pss2 = ps_s.tile([128, 1024], F32, tag="s", name="pss2")
                    for n in range(2):
                        nsl = slice(n * 512, (n + 1) * 512)
                        nc.tensor.matmul(
                            pss2[:, nsl],
                            qk_st[:, 16 + h, kt_i * 128:(kt_i + 1) * 128],
                            qk_st[:, h, nsl], start=True, stop=True,
                            skip_group_check=True,
                        )
                    nc.scalar.activation(
                        exp_h[:, kt_i, :], pss2[:, :], AF.Exp, scale=ISC
                    )
                attn_h = pattn.tile([72, NT], FP8, tag="attnh", name="attn_h")
                for n in range(2):
                    nsl = slice(n * 512, (n + 1) * 512)
                    pav = ps_av.tile([97, 512], F32, tag="av", name="pav")
                    for kk in range(4):
                        nc.tensor.matmul(
                            pav[:, :], v_aug[:, 2 * kk:2 * kk + 2, h, :],
                            exp_h[:, 2 * kk:2 * kk + 2, nsl],
                            start=(kk == 0), stop=(kk == 3), perf_mode=DR,
                        )
                    recip = pattn.tile([1, 512], F32, tag="recip", bufs=2,
                                       name="recip")
                    nc.vector.reciprocal(recip[:, :], pav[96:97, :])
                    bca = pattn.tile([72, 512], F32, tag="bca", name="bca")
                    nc.gpsimd.partition_broadcast(bca[:, :], recip[:, :])
                    atf = pattn.tile([72, 512], F32, tag="atf", bufs=2,
                                     name="atf")
                    nc.vector.tensor_mul(atf[:, :], pav[0:72, :], bca[:, :])
                    nc.vector.tensor_scalar_add(
                        attn_h[:, nsl], atf[:, :], bv_pp[:, h:h + 1]
                    )
                for (kt_i, p0, ln, off) in _head_segs(h * HD, HD):
                    nc.sync.dma_start(
                        attn_st[p0:p0 + ln, kt_i, :], attn_h[off:off + ln, :]
                    )
                with tc.tile_wait_until(0.150 + 0.006 * h):
                    for k2 in range((h * MH) // H, ((h + 1) * MH) // H):
                        emit_w2_convert(k2)
        es_qk.close()
        if phase_limit <= 3:
            es_ao.close()
            return _truncate_out(tc, nc, out_dram)

        # ============ phase 4: proj + residual1 + LN2 ========================
        es_f1h = ExitStack()
        pf1h = es_f1h.enter_context(
            tc.tile_pool(name="pf1h", bufs=1, side="right"))
        F1H = 18   # fc1 m-tiles pre-converted during phase 4
        fc1_8 = pf1h.tile([128, KT, F1H * 128], FP8, name="fc1_8")
        es_mod2 = ExitStack()
        pmod2 = es_mod2.enter_context(
            tc.tile_pool(name="pmod2", bufs=1, side="right"))
        mod2T = pmod2.tile([128, KT, NT], FP8, name="mod2T")

        def emit_f1h_convert(mo):
            wfh_t = pw_s.tile([128, KT, 128], F32, tag="ws", bufs=3,
                              name="wfh_t")
            nc.sync.dma_start(
                wfh_t[:, :, :],
                ins["w_fc1"][:, mo * 128:(mo + 1) * 128]
                .rearrange("(k p) m -> p k m", p=128),
            )
            if mo % 2 == 0:
                nc.gpsimd.tensor_scalar_mul(
                    fc1_8[:, :, mo * 128:(mo + 1) * 128], wfh_t[:, :, :], WS)
            else:
                nc.vector.tensor_scalar_mul(
                    fc1_8[:, :, mo * 128:(mo + 1) * 128], wfh_t[:, :, :], WS)

        with tc.tile_pool(name="p4w", bufs=1) as p4w, \
             tc.tile_pool(name="pst4", bufs=1) as pst4, \
             tc.tile_pool(name="pln4", bufs=1) as pln4:
            with tc.tile_pool(name="ps_mm2", bufs=2, space="PSUM") as ps_mm2:
                for mo in range(KT):
                    wp_t = pw_s.tile([128, KT, 128], F32, tag="ws", bufs=3,
                                     name="wp_t")
                    nc.sync.dma_start(
                        wp_t[:, :, :],
                        ins["w_proj"][:, mo * 128:(mo + 1) * 128]
                        .rearrange("(k p) m -> p k m", p=128),
                    )
                    wp_8 = pw_s.tile([128, KT, 128], FP8, tag="wsb", bufs=3,
                                     name="wp_8")
                    nc.gpsimd.tensor_scalar_mul(wp_8[:, :, :], wp_t[:, :, :],
                                                WS)
                    ps2 = ps_mm2.tile([128, 1024], F32, tag="mm2", name="ps2")
                    for n in range(2):
                        nsl = slice(n * 512, (n + 1) * 512)

                        def lhsp(k0, kn):
                            w = wp_8[:, k0:k0 + kn, :]
                            return w if kn == 2 else wp_8[:, k0, :]

                        def rhsp(k0, kn, nsl=nsl):
                            a = attn_st[:, k0:k0 + kn, nsl]
                            return a if kn == 2 else attn_st[:, k0, nsl]

                        mm_group(ps2[:, nsl], lhsp, rhsp, rhsp, 512)
                    t_sb4 = p4w.tile([128, 1024], F32, tag="tsb", bufs=2,
                                     name="t_sb4")
                    nc.scalar.activation(
                        t_sb4[:, :], ps2[:, :], AF.Identity,
                        bias=bproj_pp[:, mo:mo + 1], scale=WSI,
                    )
                    for n in range(2):
                        nsl = slice(n * 512, (n + 1) * 512)
                        nc.vector.scalar_tensor_tensor(
                            xT[:, mo, nsl], t_sb4[:, nsl],
                            ada_pp[:, 2, mo:mo + 1],
                            xT[:, mo, nsl], ALU.mult, ALU.add,
                        )
                    emit_f1h_convert(2 * mo)
                    emit_f1h_convert(2 * mo + 1)

            with tc.tile_pool(name="ps_st2", bufs=4, space="PSUM") as ps_st2:
                _ln_mod(tc, nc, xT, mod2T, ada_pp, 3, 4, ones_col,
                        pst4, pln4, ps_st2)
        es_ao.close()
        if phase_limit <= 4:
            es_mod2.close()
            return _truncate_out(tc, nc, out_dram)

        # ============ phase 5: FFN + output =================================
        with tc.tile_pool(name="p5w", bufs=1) as p5w, \
             tc.tile_pool(name="p5h", bufs=1) as p5h:
            hT_sb = p5h.tile([128, MH, NT], FP8, name="hT_sb")
            with tc.tile_pool(name="ps_f1", bufs=3, space="PSUM") as ps_f1:
                for mo in range(MH):
                    if mo < F1H:
                        wf_8 = None
                    else:
                        wf_t = pw_s.tile([128, KT, 128], F32, tag="ws", bufs=3,
                                         name="wf_t")
                        nc.sync.dma_start(
                            wf_t[:, :, :],
                            ins["w_fc1"][:, mo * 128:(mo + 1) * 128]
                            .rearrange("(k p) m -> p k m", p=128),
                        )
                        wf_8 = pw_s.tile([128, KT, 128], FP8, tag="wsb",
                                         bufs=3, name="wf_8")
                        if mo % 3 == 0:
                            nc.gpsimd.tensor_scalar_mul(wf_8[:, :, :],
                                                        wf_t[:, :, :], WS)
                        elif mo % 3 == 1:
                            nc.vector.tensor_scalar_mul(wf_8[:, :, :],
                                                        wf_t[:, :, :], WS)
                        else:
                            nc.scalar.activation(wf_8[:, :, :], wf_t[:, :, :],
                                                 AF.Identity, scale=WS)
                    pf1 = ps_f1.tile([128, 1024], F32, tag="f1", name="pf1")
                    for n in range(2):
                        nsl = slice(n * 512, (n + 1) * 512)

                        def lhsf(k0, kn, mo=mo, wf_8=wf_8):
                            if mo < F1H:
                                msl = slice(mo * 128, (mo + 1) * 128)
                                w = fc1_8[:, k0:k0 + kn, msl]
                                return w if kn == 2 else fc1_8[:, k0, msl]
                            w = wf_8[:, k0:k0 + kn, :]
                            return w if kn == 2 else wf_8[:, k0, :]

                        def rhsf(k0, kn, nsl=nsl):
                            m = mod2T[:, k0:k0 + kn, nsl]
                            return m if kn == 2 else mod2T[:, k0, nsl]

                        mm_group(pf1[:, nsl], lhsf, rhsf, rhsf, 512)
                    nc.scalar.activation(
                        hT_sb[:, mo, :], pf1[:, :], AF.Gelu_apprx_tanh,
                        bias=bfc1_pp[:, mo:mo + 1], scale=WSI,
                    )
            with tc.tile_pool(name="ps_f2", bufs=2, space="PSUM") as ps_f2, \
                 tc.tile_pool(name="ps_tro", bufs=2, space="PSUM") as ps_tro:
                ot2 = None
                for m in range(KT):
                    pf2 = ps_f2.tile([128, 1024], F32, tag="f2", name="pf2")
                    for n in range(2):
                        nsl = slice(n * 512, (n + 1) * 512)
                        for kk in range(MH // 2):
                            nc.tensor.matmul(
                                pf2[:, nsl],
                                w2_sb[:, 2 * kk:2 * kk + 2,
                                      m * 128:(m + 1) * 128],
                                hT_sb[:, 2 * kk:2 * kk + 2, nsl],
                                start=(kk == 0), stop=(kk == MH // 2 - 1),
                                perf_mode=DR,
                            )
                    t2 = p5w.tile([128, 1024], F32, tag="tsb", bufs=2,
                                  name="t2")
                    nc.scalar.activation(
                        t2[:, :], pf2[:, :], AF.Identity,
                        bias=bfc2_pp[:, m:m + 1], scale=WSI,
                    )
                    for n in range(2):
                        nsl = slice(n * 512, (n + 1) * 512)
                        nc.vector.scalar_tensor_tensor(
                            xT[:, m, nsl], t2[:, nsl],
                            ada_pp[:, 5, m:m + 1], xT[:, m, nsl],
                            ALU.mult, ALU.add,
                        )
                    # transpose into 2-m-tile staging; store [128,256] chunks
                    # (fewer, fatter DMAs -> less HWDGE tail)
                    if m % 2 == 0:
                        ot2 = p5w.tile([128, NT // 128, 256], F32, tag="ot2",
                                       bufs=2, name="ot2")
                    sl = slice(128 * (m % 2), 128 * (m % 2) + 128)
                    for tt in range(NT // 128):
                        pt = ps_tro.tile([128, 128], F32, tag="tro",
                                         name="pt6")
                        nc.tensor.transpose(
                            pt[:, :], xT[:, m, tt * 128:(tt + 1) * 128],
                            ident[:, :],
                        )
                        if tt % 2 == 0:
                            nc.vector.tensor_copy(ot2[:, tt, sl], pt[:, :])
                        else:
                            nc.scalar.copy(ot2[:, tt, sl], pt[:, :])
                        if m % 2 == 1 or m == KT - 1:
                            w = 128 * (m % 2) + 128
                            nc.sync.dma_start(
                                out_dram[tt * 128:(tt + 1) * 128,
                                         (m - m % 2) * 128:
                                         (m - m % 2) * 128 + w],
                                ot2[:, tt, 0:w],
                            )
        es_mod2.close()
        es_f1h.close()


_LOCK = threading.Lock()
_PROG = None


def _get_program():
    global _PROG
    with _LOCK:
        if _PROG is None:
            _PROG = _build_program()
    return _PROG


def _make_in_maps(inputs):
    arrs = {k: np.ascontiguousarray(np.asarray(v, dtype=np.float32))
            for k, v in inputs.items()}
    in_maps = []
    for c in range(NCORES):
        m = {k: v for k, v in arrs.items()
             if k not in ("x", "t_emb", "w_ada")}
        m["x"] = np.ascontiguousarray(arrs["x"][c])
        if ADA_SPLIT:
            m["t_emb"] = arrs["t_emb"]
            m["w_ada"] = np.ascontiguousarray(
                arrs["w_ada"][:, c * ADA_COLS:(c + 1) * ADA_COLS])
        else:
            m["t_emb"] = np.ascontiguousarray(arrs["t_emb"][c])
            m["w_ada"] = arrs["w_ada"]
        in_maps.append(m)
    return in_maps


def kernel(**inputs):
    nc = _get_program()
    res = run_bass_kernel_spmd(nc, _make_in_maps(inputs),
                               core_ids=list(range(NCORES)))
    return np.stack([r["out"] for r in res.results], axis=0)


def kernel_traced(inputs, **kw):
    """test-harness helper: returns full BassKernelResults with trace."""
    nc = _get_program()
    return run_bass_kernel_spmd(
        nc, _make_in_maps(inputs), core_ids=list(range(NCORES)), trace=True, **kw
    )


# revision 80
# speedup vs baseline: 1.6703x; 1.0733x over previous
"""DiT block kernel for Trainium2 (Bass/Tile), 8-core data parallel.

Shapes (hardcoded from the problem spec):
  x: (8, 1024, 1152), t_emb: (8, 1152)
  w_qkv (1152, 3456), w_proj (1152, 1152), w_fc1 (1152, 4608),
  w_fc2 (4608, 1152), w_ada (1152, 6912) + biases.

Strategy: batch-parallel across 8 cores (one batch element each).
Activations live transposed [D on partitions, tokens free]; projections are
out_T = W.T @ x_T with lhsT = W as stored.  All large GEMMs run in fp8e4
DoubleRow (two 128-deep k-tiles per PE pass); weights are scaled by 32 into
fp8 to stay in the normal range, compensated in the PSUM epilogue scale.
LayerNorm stats use f32r ones-column matmuls; softmax runs transposed with
no max subtraction and a ones-column appended to V for the denominators.
adaLN is contraction-split across the 8 cores (each core loads 1/8 of
w_ada, computes partial rows for all batches, then one small AllToAll
redistributes) -- gated by BASS_ADA_SPLIT.
"""

import os
import threading
from contextlib import ExitStack

import numpy as np

import concourse.bass as bass
import concourse.mybir as mybir
import concourse.tile as tile
from concourse import bacc
from concourse.bass_utils import run_bass_kernel_spmd
from concourse.masks import make_identity

F32 = mybir.dt.float32
F32R = mybir.dt.float32r
BF16 = mybir.dt.bfloat16
FP8 = mybir.dt.float8e4
AF = mybir.ActivationFunctionType
ALU = mybir.AluOpType
DR = mybir.MatmulPerfMode.DoubleRow

NCORES = 8
D = 1152
NT = 1024          # tokens per core (batch element)
KT = D // 128      # 9 partition-tiles of D
H = 16
HD = 72
HID = 4 * D        # 4608
MH = HID // 128    # 36
EPS = 1e-6
ISC = 1.0 / float(np.sqrt(HD))
WS = 32.0          # weight upscale into fp8e4 (avoids subnormals)
WSI = 1.0 / WS

ADA_SPLIT = os.environ.get("BASS_ADA_SPLIT", "1") == "1"
ADA_COLS = 6 * D // NCORES   # 864 columns of w_ada per core when split

# v output column slices aligned to head boundaries
V_SLICES = [(0, 432, 0, 6), (432, 864, 6, 12), (864, 1152, 12, 16)]


def _r(ap):
    return ap.bitcast(F32R)


def _head_segs(d0, n):
    """Split logical rows [d0, d0+n) of a [*,128]-tiled stacked tensor into
    (ktile, part0, length, dst_offset) segments within 128-partition tiles."""
    segs = []
    off = 0
    while n > 0:
        kt_i, p0 = divmod(d0, 128)
        ln = min(n, 128 - p0)
        segs.append((kt_i, p0, ln, off))
        d0 += ln
        off += ln
        n -= ln
    return segs


def _build_program():
    nc = bacc.Bacc(
        "TRN2", target_bir_lowering=False, debug=False, enable_asserts=False,
        num_devices=NCORES,
    )
    ins = {}
    ins["x"] = nc.dram_tensor("x", [NT, D], F32, kind="ExternalInput").ap()
    if ADA_SPLIT:
        ins["t_emb"] = nc.dram_tensor(
            "t_emb", [NCORES, D], F32, kind="ExternalInput").ap()
        ins["w_ada"] = nc.dram_tensor(
            "w_ada", [D, ADA_COLS], F32, kind="ExternalInput").ap()
    else:
        ins["t_emb"] = nc.dram_tensor(
            "t_emb", [D], F32, kind="ExternalInput").ap()
        ins["w_ada"] = nc.dram_tensor(
            "w_ada", [D, 6 * D], F32, kind="ExternalInput").ap()
    for name, shape in [
        ("w_qkv", [D, 3 * D]), ("b_qkv", [3 * D]),
        ("w_proj", [D, D]), ("b_proj", [D]),
        ("w_fc1", [D, HID]), ("b_fc1", [HID]),
        ("w_fc2", [HID, D]), ("b_fc2", [D]),
        ("b_ada", [6 * D]),
    ]:
        ins[name] = nc.dram_tensor(name, shape, F32, kind="ExternalInput").ap()
    out_dram = nc.dram_tensor("out", [NT, D], F32, kind="ExternalOutput").ap()

    with tile.TileContext(nc) as tc:
        _body(tc, ins, out_dram)
    nc.compile()
    return nc


def _ln_mod(tc, nc, src, dst, ada_pp, shift_c, scale_c, ones_col,
            pst, pln, ps_st, mul_pool=False):
    """dst[:,k,:] = fp8((src-mean)*rstd * ada[scale_c] + ada[shift_c]).
    Stats over the partition (D) axis per token via f32r ones matmuls."""
    ps_x, ps_q, st = {}, {}, {}
    for n in range(2):
        nsl = slice(n * 512, (n + 1) * 512)
        ps_x[n] = ps_st.tile([1, 512], F32, tag="st", name=f"psx{n}")
        ps_q[n] = ps_st.tile([1, 512], F32, tag="st", name=f"psq{n}")
        for k in range(KT):
            xb = pln.tile([128, 512], BF16, tag="xb", bufs=2, name="xb")
            nc.scalar.copy(xb[:, :], src[:, k, nsl])
            sq = pln.tile([128, 512], BF16, tag="sqb", bufs=2, name="sq")
            nc.vector.tensor_mul(sq[:, :], xb[:, :], xb[:, :])
            nc.tensor.matmul(
                ps_x[n][:, :], ones_col[:, :], xb[:, :],
                start=(k == 0), stop=(k == KT - 1), skip_group_check=True,
            )
            nc.tensor.matmul(
                ps_q[n][:, :], ones_col[:, :], sq[:, :],
                start=(k == 0), stop=(k == KT - 1), skip_group_check=True,
            )
    eps_sb = pst.tile([1, 1], F32, tag="eps", bufs=1, name="eps_sb")
    nc.vector.memset(eps_sb[:, :], EPS)
    for n in range(2):
        # rows: 0 = mean, 1 = E[x^2] -> rstd
        st[n] = pst.tile([1, 2, 512], F32, tag="lnst", bufs=2, name=f"st{n}")
        nc.vector.tensor_scalar_mul(st[n][:, 0, :], ps_x[n][:, :], 1.0 / D)
        work = pst.tile([1, 512], F32, tag="lnwork", bufs=2, name="work")
        nc.vector.tensor_mul(work[:, :], st[n][:, 0, :], st[n][:, 0, :])
        nc.vector.scalar_tensor_tensor(
            st[n][:, 1, :], ps_q[n][:, :], 1.0 / D, work[:, :],
            ALU.mult, ALU.subtract,
        )
        nc.scalar.activation(st[n][:, 1, :], st[n][:, 1, :], AF.Sqrt,
                             bias=eps_sb[:, :], scale=1.0)
        nc.vector.reciprocal(st[n][:, 1, :], st[n][:, 1, :])
    for n in range(2):
        nsl = slice(n * 512, (n + 1) * 512)
        meanB = pln.tile([128, 512], F32, tag="meanB", bufs=2, name="meanB")
        rstdB = pln.tile([128, 512], F32, tag="rstdB", bufs=2, name="rstdB")
        nc.gpsimd.partition_broadcast(meanB[:, :], st[n][:, 0, :])
        nc.gpsimd.partition_broadcast(rstdB[:, :], st[n][:, 1, :])
        for k in range(KT):
            tmp = pln.tile([128, 512], F32, tag="lnt", bufs=2, name="tmp")
            nc.vector.tensor_sub(tmp[:, :], src[:, k, nsl], meanB[:, :])
            meng = nc.gpsimd if (mul_pool and k % 2 == 0) else nc.vector
            meng.tensor_mul(tmp[:, :], tmp[:, :], rstdB[:, :])
            nc.gpsimd.tensor_scalar(
                dst[:, k, nsl], tmp[:, :],
                ada_pp[:, scale_c, k:k + 1], ada_pp[:, shift_c, k:k + 1],
                ALU.mult, ALU.add,
            )


def _truncate_out(tc, nc, out_dram):
    with tc.tile_pool(name="ptrunc", bufs=1) as p:
        z = p.tile([128, D], F32, name="z")
        nc.vector.memset(z[:, :], 0.0)
        for tt in range(NT // 128):
            nc.sync.dma_start(out_dram[tt * 128:(tt + 1) * 128, :], z[:, :])


def _body(tc, ins, out_dram):
    nc = tc.nc
    phase_limit = float(os.environ.get("BASS_PHASES", "6"))
    ctx = ExitStack()
    with ctx:
        dram = ctx.enter_context(tc.tile_pool(name="dram", bufs=1, space="DRAM"))
        if ADA_SPLIT:
            ada_part_dr = dram.tile([NCORES * ADA_COLS], F32)
            ada_gath_dr = dram.tile([NCORES * ADA_COLS], F32)
        else:
            ada_dr = dram.tile([6 * D], F32)

        pers = ctx.enter_context(tc.tile_pool(name="pers", bufs=1))
        ident = pers.tile([128, 128], F32)
        make_identity(nc, ident[:, :])
        ones_col = pers.tile([128, 1], BF16)
        nc.vector.memset(ones_col[:, :], 1.0)

        # silu(t_emb): split case holds all 8 batches, else just our own
        NB = NCORES if ADA_SPLIT else 1
        t_pp = pers.tile([128, NB, KT], F32)
        if ADA_SPLIT:
            nc.sync.dma_start(
                t_pp[:, :, :],
                ins["t_emb"].rearrange("b (k p) -> p b k", p=128))
        else:
            nc.sync.dma_start(
                t_pp[:, 0, :], ins["t_emb"].rearrange("(k p) -> p k", p=128))
        t_sb = pers.tile([128, NB, KT], BF16)
        nc.scalar.activation(t_sb[:, :, :], t_pp[:, :, :], AF.Silu)

        bqk72 = pers.tile([72, 32], F32)      # q,k bias per 72-head chunk
        bv_pp = pers.tile([72, H], F32)
        bproj_pp = pers.tile([128, KT], F32)
        bfc1_pp = pers.tile([128, MH], F32)
        bfc2_pp = pers.tile([128, KT], F32)
        bada_pp = pers.tile([128, 6, KT], F32)
        ada_pp = pers.tile([128, 6, KT], F32)

        def emit_bias_loads():
            nc.sync.dma_start(
                bqk72[:, :],
                ins["b_qkv"][0:2 * D].rearrange("(c p) -> p c", p=72))
            nc.sync.dma_start(
                bv_pp[:, :],
                ins["b_qkv"][2 * D:3 * D].rearrange("(h p) -> p h", p=72))
            nc.sync.dma_start(
                bproj_pp[:, :], ins["b_proj"].rearrange("(m p) -> p m", p=128))
            nc.sync.dma_start(
                bfc1_pp[:, :], ins["b_fc1"].rearrange("(m p) -> p m", p=128))
            nc.sync.dma_start(
                bfc2_pp[:, :], ins["b_fc2"].rearrange("(m p) -> p m", p=128))
            nc.sync.dma_start(
                bada_pp[:, :, :],
                ins["b_ada"].rearrange("(c k p) -> p c k", k=KT, p=128))

        xT = pers.tile([128, KT, NT], F32)   # becomes x2T, then out_T
        w2_sb = pers.tile([128, MH, D], FP8)  # fc2 weights, fp8*WS
        # weight-stream pool spanning phases 4-5 (proj/fc1 prefetch)
        pw_s = ctx.enter_context(tc.tile_pool(name="pw_s", bufs=1))
        # attn output store: allocated early so attention-phase pools do not
        # sit in freed qkv space (space-reuse would serialize the phases)
        es_ao = ExitStack()
        pastk = es_ao.enter_context(tc.tile_pool(name="pastk", bufs=1))
        attn_st = pastk.tile([128, KT, NT], FP8, name="attn_st")
        # qkv weight pool: lives through attention (closed at phase 4)
        es_qw = ExitStack()
        pqw = es_qw.enter_context(tc.tile_pool(name="pqw", bufs=1))

        # ============ phase 1: ada, x load+transpose, LN1 ====================
        es_mod1 = ExitStack()
        pmod1 = es_mod1.enter_context(tc.tile_pool(name="pmod1", bufs=1))
        mod1T = pmod1.tile([128, KT, NT], FP8, name="mod1T")

        with tc.tile_pool(name="p1w", bufs=1) as p1w, \
             tc.tile_pool(name="pst", bufs=1) as pst, \
             tc.tile_pool(name="pln", bufs=1) as pln:
            with tc.tile_pool(name="ps_pro", bufs=2, space="PSUM") as ps_pro, \
                 tc.tile_pool(name="pxin", bufs=3) as pxin, \
                 tc.tile_pool(name="ps_tr", bufs=2, space="PSUM") as ps_tr:

                def emit_transpose_block(tt):
                    xin = pxin.tile([128, D], F32, tag="xin", bufs=2,
                                    name="xin")
                    nc.sync.dma_start(
                        xin[:, :], ins["x"][tt * 128:(tt + 1) * 128, :])
                    for kd in range(KT):
                        pt = ps_tr.tile([128, 128], F32, tag="ptr", name="pt")
                        nc.tensor.transpose(
                            pt[:, :], xin[:, kd * 128:(kd + 1) * 128],
                            ident[:, :],
                        )
                        tsl = slice(tt * 128, (tt + 1) * 128)
                        if kd % 2 == 0:
                            nc.vector.tensor_copy(xT[:, kd, tsl], pt[:, :])
                        else:
                            nc.scalar.copy(xT[:, kd, tsl], pt[:, :])

                def emit_ada_split():
                    # partial rows for ALL batches over our 1/8 of w_ada,
                    # then AllToAll redistributes so each core gets its row.
                    ada_sb = pst.tile([NCORES, ADA_COLS], F32, tag="adasb",
                                      bufs=1, name="ada_sb")
                    for c in range(2):
                        csl = slice(c * 432, (c + 1) * 432)
                        wada_t = p1w.tile([128, KT, 432], F32, tag="wada",
                                          bufs=1, name="wada_t")
                        nc.sync.dma_start(
                            wada_t[:, :, :],
                            ins["w_ada"][:, csl]
                            .rearrange("(k p) m -> p k m", p=128),
                        )
                        wada_b = p1w.tile([128, KT, 432], BF16, tag="wadab",
                                          bufs=1, name="wada_b")
                        nc.gpsimd.tensor_copy(wada_b[:, :, :], wada_t[:, :, :])
                        pa = ps_pro.tile([NCORES, 432], F32, tag="psada",
                                         bufs=2, name="pa")
                        for k in range(KT):
                            nc.tensor.matmul(
                                pa[:, :], t_sb[:, :, k], wada_b[:, k, :],
                                start=(k == 0), stop=(k == KT - 1),
                            )
                        nc.vector.tensor_copy(ada_sb[:, csl], pa[:, :])
                    nc.sync.dma_start(
                        ada_part_dr.opt().rearrange("(b m) -> b m", b=NCORES),
                        ada_sb[:, :])
                    nc.gpsimd.collective_compute(
                        "AllToAll", ALU.bypass,
                        replica_groups=[list(range(NCORES))],
                        ins=[ada_part_dr.opt()],
                        outs=[ada_gath_dr.opt()],
                    )
                    nc.sync.dma_start(
                        ada_pp[:, :, :],
                        ada_gath_dr.opt().rearrange(
                            "(c k p) -> p c k", c=6, k=KT, p=128),
                    )

                def emit_ada_chunk(n):
                    # fallback: full w_ada on-core, M=1 bf16 matmuls
                    pa = ps_pro.tile([1, 384], F32, tag="psada", name="pa")
                    for k in range(KT):
                        wada_t = p1w.tile([128, 384], F32, tag="wsk", bufs=4,
                                          name="wada_t")
                        nc.sync.dma_start(
                            wada_t[:, :],
                            ins["w_ada"][k * 128:(k + 1) * 128,
                                         n * 384:(n + 1) * 384],
                        )
                        wada_b = p1w.tile([128, 384], BF16, tag="wskb",
                                          bufs=4, name="wada_b")
                        nc.gpsimd.tensor_copy(wada_b[:, :], wada_t[:, :])
                        nc.tensor.matmul(
                            pa[:, :], t_sb[:, 0, k:k + 1], wada_b[:, :],
                            start=(k == 0), stop=(k == KT - 1),
                        )
                    asb = pst.tile([1, 384], F32, tag="asb", bufs=2, name="asb")
                    nc.vector.tensor_copy(asb[:, :], pa[:, :])
                    nc.sync.dma_start(
                        ada_dr[n * 384:(n + 1) * 384]
                        .rearrange("(a b) -> a b", a=1),
                        asb[0:1, :],
                    )

                emit_bias_loads()
                if ADA_SPLIT:
                    emit_ada_split()
                    for i in range(8):
                        emit_transpose_block(i)
                else:
                    for i in range(8):
                        emit_transpose_block(i)
                        if i < 8:
                            emit_ada_chunk(2 * i)
                            emit_ada_chunk(2 * i + 1)
                    for n in range(16, 18):
                        emit_ada_chunk(n)
                    nc.sync.dma_start(
                        ada_pp[:, :, :],
                        ada_dr.opt().rearrange(
                            "(c k p) -> p c k", c=6, k=KT, p=128),
                    )
                nc.vector.tensor_add(ada_pp[:, :, :], ada_pp[:, :, :],
                                     bada_pp[:, :, :])
                nc.vector.tensor_scalar_add(ada_pp[:, 1, :], ada_pp[:, 1, :],
                                            1.0)
                nc.vector.tensor_scalar_add(ada_pp[:, 4, :], ada_pp[:, 4, :],
                                            1.0)

            if phase_limit > 0.6:
                with tc.tile_pool(name="ps_st", bufs=4, space="PSUM") as ps_st:
                    _ln_mod(tc, nc, xT, mod1T, ada_pp, 0, 1, ones_col,
                            pst, pln, ps_st)

        if phase_limit <= 1:
            es_mod1.close()
            return _truncate_out(tc, nc, out_dram)

        # ============ phase 2: qkv ==========================================
        es_qk = ExitStack()
        pqks = es_qk.enter_context(tc.tile_pool(name="pqks", bufs=1, side="right"))
        # chunks 0..15 = q heads, 16..31 = k heads; fp8 true values
        qk_st = pqks.tile([72, 32, NT], FP8, name="qk_st")
        pvaug = es_qk.enter_context(
            tc.tile_pool(name="pvaug", bufs=1, side="right"))
        # per head: cols 0..72 = v (true values), col 96 = ones, 72..96 zero
        v_aug = pvaug.tile([128, NT // 128, H, 97], FP8, name="v_aug")
        nc.gpsimd.memset(v_aug[:, :, :, HD:97], 0.0)
        nc.gpsimd.memset(v_aug[:, :, :, 96:97], 1.0)

        def emit_w2_convert(k):
            w2src = pw_s.tile([128, D], F32, tag="w2src", bufs=2,
                              name="w2src")
            nc.sync.dma_start(
                w2src[:, :], ins["w_fc2"][k * 128:(k + 1) * 128, :]
            )
            nc.gpsimd.tensor_scalar_mul(w2_sb[:, k, :], w2src[:, :], WS)

        def mm_group(psl, lhs8, rhs8, rhs_k_of, N):
            """accumulate 9 k-tiles: 4 DoubleRow pairs + 1 plain fp8."""
            for kk in range(4):
                nc.tensor.matmul(
                    psl, lhs8(2 * kk, 2), rhs_k_of(2 * kk, 2),
                    start=(kk == 0), stop=False, perf_mode=DR,
                )
            nc.tensor.matmul(psl, lhs8(8, 1), rhs_k_of(8, 1),
                             start=False, stop=True)

        with tc.tile_pool(name="p2w", bufs=1) as p2w, \
             tc.tile_pool(name="ps_mm", bufs=4, space="PSUM") as ps_mm:

            def emit_qk_dh(sec, j):
                # sec 0 = q (w cols j*144), sec 1 = k (w cols 1152 + j*144)
                wq_t = pqw.tile([128, KT, 144], F32, tag="wsq", bufs=2,
                                name="wq_t")
                nc.sync.dma_start(
                    wq_t[:, :, :],
                    ins["w_qkv"][:, sec * D + j * 144:sec * D + (j + 1) * 144]
                    .rearrange("(k p) m -> p k m", p=128),
                )
                wq_8 = pqw.tile([128, KT, 144], FP8, tag="wsbq", bufs=2,
                                name="wq_8")
                nc.gpsimd.tensor_scalar_mul(wq_8[:, :, :], wq_t[:, :, :], WS)
                for i in range(2):
                    ch = 16 * sec + 2 * j + i
                    hsl = slice(72 * i, 72 * i + 72)
                    ps2 = ps_mm.tile([72, 1024], F32, tag="mm", bufs=2,
                                     name="ps2")
                    for n in range(2):
                        nsl = slice(n * 512, (n + 1) * 512)

                        def lhs8(k0, kn, hsl=hsl):
                            w = wq_8[:, k0:k0 + kn, hsl]
                            return w if kn == 2 else wq_8[:, k0, hsl]

                        def rhs8(k0, kn, nsl=nsl):
                            m = mod1T[:, k0:k0 + kn, nsl]
                            return m if kn == 2 else mod1T[:, k0, nsl]

                        mm_group(ps2[:, nsl], lhs8, rhs8, rhs8, 512)
                    nc.scalar.activation(
                        qk_st[:, ch, :], ps2[:, :], AF.Identity,
                        bias=bqk72[:, ch:ch + 1], scale=WSI,
                    )

            def emit_v_slice(si):
                (c0, c1, h0, h1) = V_SLICES[si]
                wv_t = p2w.tile([128, KT, 432], F32, tag="wv", bufs=1,
                                name="wv_t")
                nc.sync.dma_start(
                    wv_t[:, :, 0:c1 - c0],
                    ins["w_qkv"][:, 2 * D + c0:2 * D + c1]
                    .rearrange("(k p) m -> p k m", p=128),
                )
                wv_8 = p2w.tile([128, KT, 432], FP8, tag="wvb", bufs=2,
                                name="wv_8")
                nc.scalar.activation(wv_8[:, :, 0:c1 - c0],
                                     wv_t[:, :, 0:c1 - c0], AF.Identity,
                                     scale=WS)
                for tt in range(NT // 128):
                    pmv = ps_mm.tile([128, 512], F32, tag="mmv", bufs=2,
                                     name="pmv")

                    def lhsv(k0, kn, tt=tt):
                        m = mod1T[:, k0:k0 + kn, tt * 128:(tt + 1) * 128]
                        return m if kn == 2 else mod1T[:, k0, tt * 128:(tt + 1) * 128]

                    def rhsv(k0, kn, c0=c0, c1=c1):
                        w = wv_8[:, k0:k0 + kn, 0:c1 - c0]
                        return w if kn == 2 else wv_8[:, k0, 0:c1 - c0]

                    mm_group(pmv[:, 0:c1 - c0], lhsv, rhsv, rhsv, c1 - c0)
                    for h in range(h0, h1):
                        nc.vector.tensor_scalar_mul(
                            v_aug[:, tt, h, 0:HD],
                            pmv[:, h * HD - c0:(h + 1) * HD - c0], WSI,
                        )

            # interleave q/k head-pairs and v slices so attention on early
            # heads can start while later chunks are still being produced
            for j in range(8):
                emit_qk_dh(0, j)
                emit_qk_dh(1, j)
                if j in (1, 3, 5):
                    emit_v_slice({1: 0, 3: 1, 5: 2}[j])
        es_mod1.close()
        es_qw.close()
        if phase_limit <= 2:
            es_qk.close()
            return _truncate_out(tc, nc, out_dram)

        # ============ phase 3: attention ====================================
        es_ao = ExitStack()
        pastk = es_ao.enter_context(tc.tile_pool(name="pastk", bufs=1))
        attn_st = pastk.tile([128, KT, NT], FP8, name="attn_st")

        with tc.tile_pool(name="pexp", bufs=2) as pexp, \
             tc.tile_pool(name="pattn", bufs=2) as pattn, \
             tc.tile_pool(name="ps_s", bufs=2, space="PSUM") as ps_s, \
             tc.tile_pool(name="ps_av", bufs=2, space="PSUM") as ps_av:
            for h in range(H):
                exp_h = pexp.tile([128, NT // 128, NT], FP8, tag="exp",
                                  name="exp_h")
                for kt_i in range(NT // 128):
                    pss2 = ps_s.tile([128, 1024], F32, tag="s", name="pss2")
                    for n in range(2):
                        nsl = slice(n * 512, (n + 1) * 512)
                        nc.tensor.matmul(
                            pss2[:, nsl],
                            qk_st[:, 16 + h, kt_i * 128:(kt_i + 1) * 128],
                            qk_st[:, h, nsl], start=True, stop=True,
                            skip_group_check=True,
                        )
                    nc.scalar.activation(
                        exp_h[:, kt_i, :], pss2[:, :], AF.Exp, scale=ISC
                    )
                attn_h = pattn.tile([72, NT], FP8, tag="attnh", name="attn_h")
                for n in range(2):
                    nsl = slice(n * 512, (n + 1) * 512)
                    pav = ps_av.tile([97, 512], F32, tag="av", name="pav")
                    for kk in range(4):
                        nc.tensor.matmul(
                            pav[:, :], v_aug[:, 2 * kk:2 * kk + 2, h, :],
                            exp_h[:, 2 * kk:2 * kk + 2, nsl],
                            start=(kk == 0), stop=(kk == 3), perf_mode=DR,
                        )
                    recip = pattn.tile([1, 512], F32, tag="recip", bufs=2,
                                       name="recip")
                    nc.vector.reciprocal(recip[:, :], pav[96:97, :])
                    bca = pattn.tile([72, 512], F32, tag="bca", name="bca")
                    nc.gpsimd.partition_broadcast(bca[:, :], recip[:, :])
                    atf = pattn.tile([72, 512], F32, tag="atf", bufs=2,
                                     name="atf")
                    nc.vector.tensor_mul(atf[:, :], pav[0:72, :], bca[:, :])
                    nc.vector.tensor_scalar_add(
                        attn_h[:, nsl], atf[:, :], bv_pp[:, h:h + 1]
                    )
                for (kt_i, p0, ln, off) in _head_segs(h * HD, HD):
                    nc.sync.dma_start(
                        attn_st[p0:p0 + ln, kt_i, :], attn_h[off:off + ln, :]
                    )
                with tc.tile_wait_until(0.150 + 0.006 * h):
                    for k2 in range((h * MH) // H, ((h + 1) * MH) // H):
                        emit_w2_convert(k2)

            # proj chases head completion: its k-accumulation consumes
            # attn_st k-tiles as the covering heads finish
            for mo in range(KT):
                wp_t = pw_s.tile([128, KT, 128], F32, tag="ws", bufs=3,
                                 name="wp_t")
                nc.sync.dma_start(
                    wp_t[:, :, :],
                    ins["w_proj"][:, mo * 128:(mo + 1) * 128]
                    .rearrange("(k p) m -> p k m", p=128),
                )
                wp_8 = pw_s.tile([128, KT, 128], FP8, tag="wsb", bufs=3,
                                 name="wp_8")
                nc.gpsimd.tensor_scalar_mul(wp_8[:, :, :], wp_t[:, :, :],
                                            WS)
                for n in range(2):
                    nsl = slice(n * 512, (n + 1) * 512)
                    ps2p = ps_s.tile([128, 512], F32, tag="mm2", bufs=2,
                                     name="ps2p")

                    def lhsp(k0, kn):
                        w = wp_8[:, k0:k0 + kn, :]
                        return w if kn == 2 else wp_8[:, k0, :]

                    def rhsp(k0, kn, nsl=nsl):
                        a = attn_st[:, k0:k0 + kn, nsl]
                        return a if kn == 2 else attn_st[:, k0, nsl]

                    mm_group(ps2p[:, :], lhsp, rhsp, rhsp, 512)
                    t_sb4 = pattn.tile([128, 512], F32, tag="tsb", bufs=2,
                                       name="t_sb4")
                    nc.scalar.activation(
                        t_sb4[:, :], ps2p[:, :], AF.Identity,
                        bias=bproj_pp[:, mo:mo + 1], scale=WSI,
                    )
                    nc.vector.scalar_tensor_tensor(
                        xT[:, mo, nsl], t_sb4[:, :],
                        ada_pp[:, 2, mo:mo + 1],
                        xT[:, mo, nsl], ALU.mult, ALU.add,
                    )
        es_qk.close()
        es_ao.close()
        if phase_limit <= 3:
            es_ao.close()
            return _truncate_out(tc, nc, out_dram)

        # ============ phase 4: proj + residual1 + LN2 ========================
        es_f1h = ExitStack()
        pf1h = es_f1h.enter_context(
            tc.tile_pool(name="pf1h", bufs=1, side="right"))
        F1H = 18   # fc1 m-tiles pre-converted during phase 4
        fc1_8 = pf1h.tile([128, KT, F1H * 128], FP8, name="fc1_8")
        es_mod2 = ExitStack()
        pmod2 = es_mod2.enter_context(
            tc.tile_pool(name="pmod2", bufs=1, side="right"))
        mod2T = pmod2.tile([128, KT, NT], FP8, name="mod2T")

        def emit_f1h_convert(p4f, mo):
            wfh_t = p4f.tile([128, KT, 128], F32, tag="wsf", bufs=6,
                             name="wfh_t")
            nc.sync.dma_start(
                wfh_t[:, :, :],
                ins["w_fc1"][:, mo * 128:(mo + 1) * 128]
                .rearrange("(k p) m -> p k m", p=128),
            )
            if mo % 3 == 2:
                nc.gpsimd.tensor_scalar_mul(
                    fc1_8[:, :, mo * 128:(mo + 1) * 128], wfh_t[:, :, :], WS)
            else:
                nc.scalar.activation(
                    fc1_8[:, :, mo * 128:(mo + 1) * 128], wfh_t[:, :, :],
                    AF.Identity, scale=WS)

        with tc.tile_pool(name="pst4", bufs=1) as pst4, \
             tc.tile_pool(name="pln4", bufs=1) as pln4, \
             tc.tile_pool(name="p4f", bufs=1) as p4f:
            for mo in range(F1H):
                emit_f1h_convert(p4f, mo)
            with tc.tile_pool(name="ps_st2", bufs=4, space="PSUM") as ps_st2:
                _ln_mod(tc, nc, xT, mod2T, ada_pp, 3, 4, ones_col,
                        pst4, pln4, ps_st2)
        if phase_limit <= 4:
            es_mod2.close()
            return _truncate_out(tc, nc, out_dram)

        # ============ phase 5: FFN + output =================================
        # half-granular pipeline: fc1 half-0 (prefetched weights) starts as
        # soon as mod2T half-0 exists; fc2 half-0 chases; streamed fc1
        # weights cover both halves between the two prefetched passes.
        with tc.tile_pool(name="p5w", bufs=1) as p5w, \
             tc.tile_pool(name="p5h", bufs=1) as p5h, \
             tc.tile_pool(name="ps_5", bufs=1, space="PSUM") as ps_5:
            hT_sb = p5h.tile([128, MH, NT], FP8, name="hT_sb")

            def fc1_group(mo, n, wf_8):
                nsl = slice(n * 512, (n + 1) * 512)
                pf1 = ps_5.tile([128, 512], F32, tag="f1", bufs=3,
                                name="pf1")

                def lhsf(k0, kn, mo=mo, wf_8=wf_8):
                    if wf_8 is None:
                        msl = slice(mo * 128, (mo + 1) * 128)
                        w = fc1_8[:, k0:k0 + kn, msl]
                        return w if kn == 2 else fc1_8[:, k0, msl]
                    w = wf_8[:, k0:k0 + kn, :]
                    return w if kn == 2 else wf_8[:, k0, :]

                def rhsf(k0, kn, nsl=nsl):
                    m = mod2T[:, k0:k0 + kn, nsl]
                    return m if kn == 2 else mod2T[:, k0, nsl]

                mm_group(pf1[:, :], lhsf, rhsf, rhsf, 512)
                nc.scalar.activation(
                    hT_sb[:, mo, nsl], pf1[:, :], AF.Gelu_apprx_tanh,
                    bias=bfc1_pp[:, mo:mo + 1], scale=WSI,
                )

            def fc2_m(m, n):
                nsl = slice(n * 512, (n + 1) * 512)
                pf2 = ps_5.tile([128, 512], F32, tag="f2", bufs=2,
                                name="pf2")
                for kk in range(MH // 2):
                    nc.tensor.matmul(
                        pf2[:, :],
                        w2_sb[:, 2 * kk:2 * kk + 2, m * 128:(m + 1) * 128],
                        hT_sb[:, 2 * kk:2 * kk + 2, nsl],
                        start=(kk == 0), stop=(kk == MH // 2 - 1),
                        perf_mode=DR,
                    )
                t2 = p5w.tile([128, 512], F32, tag="tsb", bufs=2, name="t2")
                nc.scalar.activation(
                    t2[:, :], pf2[:, :], AF.Identity,
                    bias=bfc2_pp[:, m:m + 1], scale=WSI,
                )
                nc.vector.scalar_tensor_tensor(
                    xT[:, m, nsl], t2[:, :],
                    ada_pp[:, 5, m:m + 1], xT[:, m, nsl],
                    ALU.mult, ALU.add,
                )

            # fc1 prefetched half-0, then streamed mo both halves, then
            # prefetched half-1 (fc2 half-0 can start during the latter)
            for mo in range(F1H):
                fc1_group(mo, 0, None)
            for mo in range(F1H, MH):
                wf_t = pw_s.tile([128, KT, 128], F32, tag="ws", bufs=3,
                                 name="wf_t")
                nc.sync.dma_start(
                    wf_t[:, :, :],
                    ins["w_fc1"][:, mo * 128:(mo + 1) * 128]
                    .rearrange("(k p) m -> p k m", p=128),
                )
                wf_8 = pw_s.tile([128, KT, 128], FP8, tag="wsb",
                                 bufs=3, name="wf_8")
                if mo % 2 == 0:
                    nc.gpsimd.tensor_scalar_mul(wf_8[:, :, :],
                                                wf_t[:, :, :], WS)
                else:
                    nc.vector.tensor_scalar_mul(wf_8[:, :, :],
                                                wf_t[:, :, :], WS)
                fc1_group(mo, 0, wf_8)
                fc1_group(mo, 1, wf_8)
            for mo in range(F1H):
                fc1_group(mo, 1, None)

            ot2 = {}
            for n in range(2):
                for m in range(KT):
                    fc2_m(m, n)
                    # transpose this half's token tiles; store [128,256]
                    # chunks once both m's of a pair are done
                    if m % 2 == 0:
                        ot2[n] = p5w.tile([128, 4, 256], F32, tag="ot2",
                                          bufs=2, name="ot2")
                    sl = slice(128 * (m % 2), 128 * (m % 2) + 128)
                    for tt in range(4 * n, 4 * n + 4):
                        pt = ps_5.tile([128, 128], F32, tag="tro", bufs=2,
                                       name="pt6")
                        nc.tensor.transpose(
                            pt[:, :], xT[:, m, tt * 128:(tt + 1) * 128],
                            ident[:, :],
                        )
                        nc.vector.tensor_copy(ot2[n][:, tt - 4 * n, sl],
                                              pt[:, :])
                        if m % 2 == 1 or m == KT - 1:
                            w = 128 * (m % 2) + 128
                            nc.sync.dma_start(
                                out_dram[tt * 128:(tt + 1) * 128,
                                         (m - m % 2) * 128:
                                         (m - m % 2) * 128 + w],
                                ot2[n][:, tt - 4 * n, 0:w],
                            )
        es_mod2.close()
        es_f1h.close()


_LOCK = threading.Lock()
_PROG = None


def _get_program():
    global _PROG
    with _LOCK:
        if _PROG is None:
            _PROG = _build_program()
    return _PROG


def _make_in_maps(inputs):
    arrs = {k: np.ascontiguousarray(np.asarray(v, dtype=np.float32))
            for k, v in inputs.items()}
    in_maps = []
    for c in range(NCORES):
        m = {k: v for k, v in arrs.items()
             if k not in ("x", "t_emb", "w_ada")}
        m["x"] = np.ascontiguousarray(arrs["x"][c])
        if ADA_SPLIT:
            m["t_emb"] = arrs["t_emb"]
            m["w_ada"] = np.ascontiguousarray(
                arrs["w_ada"][:, c * ADA_COLS:(c + 1) * ADA_COLS])
        else:
            m["t_emb"] = np.ascontiguousarray(arrs["t_emb"][c])
            m["w_ada"] = arrs["w_ada"]
        in_maps.append(m)
    return in_maps


def kernel(**inputs):
    nc = _get_program()
    res = run_bass_kernel_spmd(nc, _make_in_maps(inputs),
                               core_ids=list(range(NCORES)))
    return np.stack([r["out"] for r in res.results], axis=0)


def kernel_traced(inputs, **kw):
    """test-harness helper: returns full BassKernelResults with trace."""
    nc = _get_program()
    return run_bass_kernel_spmd(
        nc, _make_in_maps(inputs), core_ids=list(range(NCORES)), trace=True, **kw
    )


# revision 81
# speedup vs baseline: 1.6818x; 1.0069x over previous
"""DiT block kernel for Trainium2 (Bass/Tile), 8-core data parallel.

Shapes (hardcoded from the problem spec):
  x: (8, 1024, 1152), t_emb: (8, 1152)
  w_qkv (1152, 3456), w_proj (1152, 1152), w_fc1 (1152, 4608),
  w_fc2 (4608, 1152), w_ada (1152, 6912) + biases.

Strategy: batch-parallel across 8 cores (one batch element each).
Activations live transposed [D on partitions, tokens free]; projections are
out_T = W.T @ x_T with lhsT = W as stored.  All large GEMMs run in fp8e4
DoubleRow (two 128-deep k-tiles per PE pass); weights are scaled by 32 into
fp8 to stay in the normal range, compensated in the PSUM epilogue scale.
LayerNorm stats use f32r ones-column matmuls; softmax runs transposed with
no max subtraction and a ones-column appended to V for the denominators.
adaLN is contraction-split across the 8 cores (each core loads 1/8 of
w_ada, computes partial rows for all batches, then one small AllToAll
redistributes) -- gated by BASS_ADA_SPLIT.
"""

import os
import threading
from contextlib import ExitStack

import numpy as np

import concourse.bass as bass
import concourse.mybir as mybir
import concourse.tile as tile
from concourse import bacc
from concourse.bass_utils import run_bass_kernel_spmd
from concourse.masks import make_identity

F32 = mybir.dt.float32
F32R = mybir.dt.float32r
BF16 = mybir.dt.bfloat16
FP8 = mybir.dt.float8e4
AF = mybir.ActivationFunctionType
ALU = mybir.AluOpType
DR = mybir.MatmulPerfMode.DoubleRow

NCORES = 8
D = 1152
NT = 1024          # tokens per core (batch element)
KT = D // 128      # 9 partition-tiles of D
H = 16
HD = 72
HID = 4 * D        # 4608
MH = HID // 128    # 36
EPS = 1e-6
ISC = 1.0 / float(np.sqrt(HD))
WS = 32.0          # weight upscale into fp8e4 (avoids subnormals)
WSI = 1.0 / WS

ADA_SPLIT = os.environ.get("BASS_ADA_SPLIT", "1") == "1"
ADA_COLS = 6 * D // NCORES   # 864 columns of w_ada per core when split

# v output column slices aligned to head boundaries
V_SLICES = [(0, 432, 0, 6), (432, 864, 6, 12), (864, 1152, 12, 16)]


def _r(ap):
    return ap.bitcast(F32R)


def _head_segs(d0, n):
    """Split logical rows [d0, d0+n) of a [*,128]-tiled stacked tensor into
    (ktile, part0, length, dst_offset) segments within 128-partition tiles."""
    segs = []
    off = 0
    while n > 0:
        kt_i, p0 = divmod(d0, 128)
        ln = min(n, 128 - p0)
        segs.append((kt_i, p0, ln, off))
        d0 += ln
        off += ln
        n -= ln
    return segs


def _build_program():
    nc = bacc.Bacc(
        "TRN2", target_bir_lowering=False, debug=False, enable_asserts=False,
        num_devices=NCORES,
    )
    ins = {}
    ins["x"] = nc.dram_tensor("x", [NT, D], F32, kind="ExternalInput").ap()
    if ADA_SPLIT:
        ins["t_emb"] = nc.dram_tensor(
            "t_emb", [NCORES, D], F32, kind="ExternalInput").ap()
        ins["w_ada"] = nc.dram_tensor(
            "w_ada", [D, ADA_COLS], F32, kind="ExternalInput").ap()
    else:
        ins["t_emb"] = nc.dram_tensor(
            "t_emb", [D], F32, kind="ExternalInput").ap()
        ins["w_ada"] = nc.dram_tensor(
            "w_ada", [D, 6 * D], F32, kind="ExternalInput").ap()
    for name, shape in [
        ("w_qkv", [D, 3 * D]), ("b_qkv", [3 * D]),
        ("w_proj", [D, D]), ("b_proj", [D]),
        ("w_fc1", [D, HID]), ("b_fc1", [HID]),
        ("w_fc2", [HID, D]), ("b_fc2", [D]),
        ("b_ada", [6 * D]),
    ]:
        ins[name] = nc.dram_tensor(name, shape, F32, kind="ExternalInput").ap()
    out_dram = nc.dram_tensor("out", [NT, D], F32, kind="ExternalOutput").ap()

    with tile.TileContext(nc) as tc:
        _body(tc, ins, out_dram)
    nc.compile()
    return nc


def _ln_mod(tc, nc, src, dst, ada_pp, shift_c, scale_c, ones_col,
            pst, pln, ps_st, mul_pool=False):
    """dst[:,k,:] = fp8((src-mean)*rstd * ada[scale_c] + ada[shift_c]).
    Stats over the partition (D) axis per token via f32r ones matmuls."""
    ps_x, ps_q, st = {}, {}, {}
    for n in range(2):
        nsl = slice(n * 512, (n + 1) * 512)
        ps_x[n] = ps_st.tile([1, 512], F32, tag="st", name=f"psx{n}")
        ps_q[n] = ps_st.tile([1, 512], F32, tag="st", name=f"psq{n}")
        for k in range(KT):
            xb = pln.tile([128, 512], BF16, tag="xb", bufs=2, name="xb")
            if mul_pool or k % 2 == 0:
                nc.scalar.copy(xb[:, :], src[:, k, nsl])
            else:
                nc.vector.tensor_copy(xb[:, :], src[:, k, nsl])
            sq = pln.tile([128, 512], BF16, tag="sqb", bufs=2, name="sq")
            nc.vector.tensor_mul(sq[:, :], xb[:, :], xb[:, :])
            nc.tensor.matmul(
                ps_x[n][:, :], ones_col[:, :], xb[:, :],
                start=(k == 0), stop=(k == KT - 1), skip_group_check=True,
            )
            nc.tensor.matmul(
                ps_q[n][:, :], ones_col[:, :], sq[:, :],
                start=(k == 0), stop=(k == KT - 1), skip_group_check=True,
            )
    eps_sb = pst.tile([1, 1], F32, tag="eps", bufs=1, name="eps_sb")
    nc.vector.memset(eps_sb[:, :], EPS)
    for n in range(2):
        # rows: 0 = mean, 1 = E[x^2] -> rstd
        st[n] = pst.tile([1, 2, 512], F32, tag="lnst", bufs=2, name=f"st{n}")
        nc.vector.tensor_scalar_mul(st[n][:, 0, :], ps_x[n][:, :], 1.0 / D)
        work = pst.tile([1, 512], F32, tag="lnwork", bufs=2, name="work")
        nc.vector.tensor_mul(work[:, :], st[n][:, 0, :], st[n][:, 0, :])
        nc.vector.scalar_tensor_tensor(
            st[n][:, 1, :], ps_q[n][:, :], 1.0 / D, work[:, :],
            ALU.mult, ALU.subtract,
        )
        nc.scalar.activation(st[n][:, 1, :], st[n][:, 1, :], AF.Sqrt,
                             bias=eps_sb[:, :], scale=1.0)
        nc.vector.reciprocal(st[n][:, 1, :], st[n][:, 1, :])
    for n in range(2):
        nsl = slice(n * 512, (n + 1) * 512)
        meanB = pln.tile([128, 512], F32, tag="meanB", bufs=2, name="meanB")
        rstdB = pln.tile([128, 512], F32, tag="rstdB", bufs=2, name="rstdB")
        nc.gpsimd.partition_broadcast(meanB[:, :], st[n][:, 0, :])
        nc.gpsimd.partition_broadcast(rstdB[:, :], st[n][:, 1, :])
        for k in range(KT):
            tmp = pln.tile([128, 512], F32, tag="lnt", bufs=2, name="tmp")
            nc.vector.tensor_sub(tmp[:, :], src[:, k, nsl], meanB[:, :])
            meng = nc.gpsimd if (mul_pool and k % 2 == 0) else nc.vector
            meng.tensor_mul(tmp[:, :], tmp[:, :], rstdB[:, :])
            nc.gpsimd.tensor_scalar(
                dst[:, k, nsl], tmp[:, :],
                ada_pp[:, scale_c, k:k + 1], ada_pp[:, shift_c, k:k + 1],
                ALU.mult, ALU.add,
            )


def _truncate_out(tc, nc, out_dram):
    with tc.tile_pool(name="ptrunc", bufs=1) as p:
        z = p.tile([128, D], F32, name="z")
        nc.vector.memset(z[:, :], 0.0)
        for tt in range(NT // 128):
            nc.sync.dma_start(out_dram[tt * 128:(tt + 1) * 128, :], z[:, :])


def _body(tc, ins, out_dram):
    nc = tc.nc
    phase_limit = float(os.environ.get("BASS_PHASES", "6"))
    ctx = ExitStack()
    with ctx:
        dram = ctx.enter_context(tc.tile_pool(name="dram", bufs=1, space="DRAM"))
        if ADA_SPLIT:
            ada_part_dr = dram.tile([NCORES * ADA_COLS], F32)
            ada_gath_dr = dram.tile([NCORES * ADA_COLS], F32)
        else:
            ada_dr = dram.tile([6 * D], F32)

        pers = ctx.enter_context(tc.tile_pool(name="pers", bufs=1))
        ident = pers.tile([128, 128], F32)
        make_identity(nc, ident[:, :])
        ones_col = pers.tile([128, 1], BF16)
        nc.vector.memset(ones_col[:, :], 1.0)

        # silu(t_emb): split case holds all 8 batches, else just our own
        NB = NCORES if ADA_SPLIT else 1
        t_pp = pers.tile([128, NB, KT], F32)
        if ADA_SPLIT:
            nc.sync.dma_start(
                t_pp[:, :, :],
                ins["t_emb"].rearrange("b (k p) -> p b k", p=128))
        else:
            nc.sync.dma_start(
                t_pp[:, 0, :], ins["t_emb"].rearrange("(k p) -> p k", p=128))
        t_sb = pers.tile([128, NB, KT], BF16)
        nc.scalar.activation(t_sb[:, :, :], t_pp[:, :, :], AF.Silu)

        bqk72 = pers.tile([72, 32], F32)      # q,k bias per 72-head chunk
        bv_pp = pers.tile([72, H], F32)
        bproj_pp = pers.tile([128, KT], F32)
        bfc1_pp = pers.tile([128, MH], F32)
        bfc2_pp = pers.tile([128, KT], F32)
        bada_pp = pers.tile([128, 6, KT], F32)
        ada_pp = pers.tile([128, 6, KT], F32)

        def emit_bias_loads():
            nc.sync.dma_start(
                bqk72[:, :],
                ins["b_qkv"][0:2 * D].rearrange("(c p) -> p c", p=72))
            nc.sync.dma_start(
                bv_pp[:, :],
                ins["b_qkv"][2 * D:3 * D].rearrange("(h p) -> p h", p=72))
            nc.sync.dma_start(
                bproj_pp[:, :], ins["b_proj"].rearrange("(m p) -> p m", p=128))
            nc.sync.dma_start(
                bfc1_pp[:, :], ins["b_fc1"].rearrange("(m p) -> p m", p=128))
            nc.sync.dma_start(
                bfc2_pp[:, :], ins["b_fc2"].rearrange("(m p) -> p m", p=128))
            nc.sync.dma_start(
                bada_pp[:, :, :],
                ins["b_ada"].rearrange("(c k p) -> p c k", k=KT, p=128))

        xT = pers.tile([128, KT, NT], F32)   # becomes x2T, then out_T
        w2_sb = pers.tile([128, MH, D], FP8)  # fc2 weights, fp8*WS
        # weight-stream pool spanning phases 4-5 (proj/fc1 prefetch)
        pw_s = ctx.enter_context(tc.tile_pool(name="pw_s", bufs=1))
        # attn output store: allocated early so attention-phase pools do not
        # sit in freed qkv space (space-reuse would serialize the phases)
        es_ao = ExitStack()
        pastk = es_ao.enter_context(tc.tile_pool(name="pastk", bufs=1))
        attn_st = pastk.tile([128, KT, NT], FP8, name="attn_st")
        # qkv weight pool: lives through attention (closed at phase 4)
        es_qw = ExitStack()
        pqw = es_qw.enter_context(tc.tile_pool(name="pqw", bufs=1))

        # ============ phase 1: ada, x load+transpose, LN1 ====================
        es_mod1 = ExitStack()
        pmod1 = es_mod1.enter_context(tc.tile_pool(name="pmod1", bufs=1))
        mod1T = pmod1.tile([128, KT, NT], FP8, name="mod1T")

        with tc.tile_pool(name="p1w", bufs=1) as p1w, \
             tc.tile_pool(name="pst", bufs=1) as pst, \
             tc.tile_pool(name="pln", bufs=1) as pln:
            with tc.tile_pool(name="ps_pro", bufs=2, space="PSUM") as ps_pro, \
                 tc.tile_pool(name="pxin", bufs=3) as pxin, \
                 tc.tile_pool(name="ps_tr", bufs=2, space="PSUM") as ps_tr:

                def emit_transpose_block(tt):
                    xin = pxin.tile([128, D], F32, tag="xin", bufs=2,
                                    name="xin")
                    nc.sync.dma_start(
                        xin[:, :], ins["x"][tt * 128:(tt + 1) * 128, :])
                    for kd in range(KT):
                        pt = ps_tr.tile([128, 128], F32, tag="ptr", name="pt")
                        nc.tensor.transpose(
                            pt[:, :], xin[:, kd * 128:(kd + 1) * 128],
                            ident[:, :],
                        )
                        tsl = slice(tt * 128, (tt + 1) * 128)
                        if kd % 2 == 0:
                            nc.vector.tensor_copy(xT[:, kd, tsl], pt[:, :])
                        else:
                            nc.scalar.copy(xT[:, kd, tsl], pt[:, :])

                def emit_ada_split():
                    # partial rows for ALL batches over our 1/8 of w_ada,
                    # then AllToAll redistributes so each core gets its row.
                    ada_sb = pst.tile([NCORES, ADA_COLS], F32, tag="adasb",
                                      bufs=1, name="ada_sb")
                    for c in range(2):
                        csl = slice(c * 432, (c + 1) * 432)
                        wada_t = p1w.tile([128, KT, 432], F32, tag="wada",
                                          bufs=1, name="wada_t")
                        nc.sync.dma_start(
                            wada_t[:, :, :],
                            ins["w_ada"][:, csl]
                            .rearrange("(k p) m -> p k m", p=128),
                        )
                        wada_b = p1w.tile([128, KT, 432], BF16, tag="wadab",
                                          bufs=1, name="wada_b")
                        nc.gpsimd.tensor_copy(wada_b[:, :, :], wada_t[:, :, :])
                        pa = ps_pro.tile([NCORES, 432], F32, tag="psada",
                                         bufs=2, name="pa")
                        for k in range(KT):
                            nc.tensor.matmul(
                                pa[:, :], t_sb[:, :, k], wada_b[:, k, :],
                                start=(k == 0), stop=(k == KT - 1),
                            )
                        nc.vector.tensor_copy(ada_sb[:, csl], pa[:, :])
                    nc.sync.dma_start(
                        ada_part_dr.opt().rearrange("(b m) -> b m", b=NCORES),
                        ada_sb[:, :])
                    nc.gpsimd.collective_compute(
                        "AllToAll", ALU.bypass,
                        replica_groups=[list(range(NCORES))],
                        ins=[ada_part_dr.opt()],
                        outs=[ada_gath_dr.opt()],
                    )
                    nc.sync.dma_start(
                        ada_pp[:, :, :],
                        ada_gath_dr.opt().rearrange(
                            "(c k p) -> p c k", c=6, k=KT, p=128),
                    )

                def emit_ada_chunk(n):
                    # fallback: full w_ada on-core, M=1 bf16 matmuls
                    pa = ps_pro.tile([1, 384], F32, tag="psada", name="pa")
                    for k in range(KT):
                        wada_t = p1w.tile([128, 384], F32, tag="wsk", bufs=4,
                                          name="wada_t")
                        nc.sync.dma_start(
                            wada_t[:, :],
                            ins["w_ada"][k * 128:(k + 1) * 128,
                                         n * 384:(n + 1) * 384],
                        )
                        wada_b = p1w.tile([128, 384], BF16, tag="wskb",
                                          bufs=4, name="wada_b")
                        nc.gpsimd.tensor_copy(wada_b[:, :], wada_t[:, :])
                        nc.tensor.matmul(
                            pa[:, :], t_sb[:, 0, k:k + 1], wada_b[:, :],
                            start=(k == 0), stop=(k == KT - 1),
                        )
                    asb = pst.tile([1, 384], F32, tag="asb", bufs=2, name="asb")
                    nc.vector.tensor_copy(asb[:, :], pa[:, :])
                    nc.sync.dma_start(
                        ada_dr[n * 384:(n + 1) * 384]
                        .rearrange("(a b) -> a b", a=1),
                        asb[0:1, :],
                    )

                emit_bias_loads()
                if ADA_SPLIT:
                    emit_ada_split()
                    for i in range(8):
                        emit_transpose_block(i)
                else:
                    for i in range(8):
                        emit_transpose_block(i)
                        if i < 8:
                            emit_ada_chunk(2 * i)
                            emit_ada_chunk(2 * i + 1)
                    for n in range(16, 18):
                        emit_ada_chunk(n)
                    nc.sync.dma_start(
                        ada_pp[:, :, :],
                        ada_dr.opt().rearrange(
                            "(c k p) -> p c k", c=6, k=KT, p=128),
                    )
                nc.vector.tensor_add(ada_pp[:, :, :], ada_pp[:, :, :],
                                     bada_pp[:, :, :])
                nc.vector.tensor_scalar_add(ada_pp[:, 1, :], ada_pp[:, 1, :],
                                            1.0)
                nc.vector.tensor_scalar_add(ada_pp[:, 4, :], ada_pp[:, 4, :],
                                            1.0)

            if phase_limit > 0.6:
                with tc.tile_pool(name="ps_st", bufs=4, space="PSUM") as ps_st:
                    _ln_mod(tc, nc, xT, mod1T, ada_pp, 0, 1, ones_col,
                            pst, pln, ps_st)

        if phase_limit <= 1:
            es_mod1.close()
            return _truncate_out(tc, nc, out_dram)

        # ============ phase 2: qkv ==========================================
        es_qk = ExitStack()
        pqks = es_qk.enter_context(tc.tile_pool(name="pqks", bufs=1, side="right"))
        # chunks 0..15 = q heads, 16..31 = k heads; fp8 true values
        qk_st = pqks.tile([72, 32, NT], FP8, name="qk_st")
        pvaug = es_qk.enter_context(
            tc.tile_pool(name="pvaug", bufs=1, side="right"))
        # per head: cols 0..72 = v (true values), col 96 = ones, 72..96 zero
        v_aug = pvaug.tile([128, NT // 128, H, 97], FP8, name="v_aug")
        nc.gpsimd.memset(v_aug[:, :, :, HD:97], 0.0)
        nc.gpsimd.memset(v_aug[:, :, :, 96:97], 1.0)

        def emit_w2_convert(k):
            w2src = pw_s.tile([128, D], F32, tag="w2src", bufs=2,
                              name="w2src")
            nc.sync.dma_start(
                w2src[:, :], ins["w_fc2"][k * 128:(k + 1) * 128, :]
            )
            nc.gpsimd.tensor_scalar_mul(w2_sb[:, k, :], w2src[:, :], WS)

        def mm_group(psl, lhs8, rhs8, rhs_k_of, N):
            """accumulate 9 k-tiles: 4 DoubleRow pairs + 1 plain fp8."""
            for kk in range(4):
                nc.tensor.matmul(
                    psl, lhs8(2 * kk, 2), rhs_k_of(2 * kk, 2),
                    start=(kk == 0), stop=False, perf_mode=DR,
                )
            nc.tensor.matmul(psl, lhs8(8, 1), rhs_k_of(8, 1),
                             start=False, stop=True)

        with tc.tile_pool(name="p2w", bufs=1) as p2w, \
             tc.tile_pool(name="ps_mm", bufs=4, space="PSUM") as ps_mm:

            def emit_qk_dh(sec, j):
                # sec 0 = q (w cols j*144), sec 1 = k (w cols 1152 + j*144)
                wq_t = pqw.tile([128, KT, 144], F32, tag="wsq", bufs=2,
                                name="wq_t")
                nc.sync.dma_start(
                    wq_t[:, :, :],
                    ins["w_qkv"][:, sec * D + j * 144:sec * D + (j + 1) * 144]
                    .rearrange("(k p) m -> p k m", p=128),
                )
                wq_8 = pqw.tile([128, KT, 144], FP8, tag="wsbq", bufs=2,
                                name="wq_8")
                nc.gpsimd.tensor_scalar_mul(wq_8[:, :, :], wq_t[:, :, :], WS)
                for i in range(2):
                    ch = 16 * sec + 2 * j + i
                    hsl = slice(72 * i, 72 * i + 72)
                    ps2 = ps_mm.tile([72, 1024], F32, tag="mm", bufs=2,
                                     name="ps2")
                    for n in range(2):
                        nsl = slice(n * 512, (n + 1) * 512)

                        def lhs8(k0, kn, hsl=hsl):
                            w = wq_8[:, k0:k0 + kn, hsl]
                            return w if kn == 2 else wq_8[:, k0, hsl]

                        def rhs8(k0, kn, nsl=nsl):
                            m = mod1T[:, k0:k0 + kn, nsl]
                            return m if kn == 2 else mod1T[:, k0, nsl]

                        mm_group(ps2[:, nsl], lhs8, rhs8, rhs8, 512)
                    nc.scalar.activation(
                        qk_st[:, ch, :], ps2[:, :], AF.Identity,
                        bias=bqk72[:, ch:ch + 1], scale=WSI,
                    )

            def emit_v_slice(si):
                (c0, c1, h0, h1) = V_SLICES[si]
                wv_t = p2w.tile([128, KT, 432], F32, tag="wv", bufs=1,
                                name="wv_t")
                nc.sync.dma_start(
                    wv_t[:, :, 0:c1 - c0],
                    ins["w_qkv"][:, 2 * D + c0:2 * D + c1]
                    .rearrange("(k p) m -> p k m", p=128),
                )
                wv_8 = p2w.tile([128, KT, 432], FP8, tag="wvb", bufs=2,
                                name="wv_8")
                nc.scalar.activation(wv_8[:, :, 0:c1 - c0],
                                     wv_t[:, :, 0:c1 - c0], AF.Identity,
                                     scale=WS)
                for tt in range(NT // 128):
                    pmv = ps_mm.tile([128, 512], F32, tag="mmv", bufs=2,
                                     name="pmv")

                    def lhsv(k0, kn, tt=tt):
                        m = mod1T[:, k0:k0 + kn, tt * 128:(tt + 1) * 128]
                        return m if kn == 2 else mod1T[:, k0, tt * 128:(tt + 1) * 128]

                    def rhsv(k0, kn, c0=c0, c1=c1):
                        w = wv_8[:, k0:k0 + kn, 0:c1 - c0]
                        return w if kn == 2 else wv_8[:, k0, 0:c1 - c0]

                    mm_group(pmv[:, 0:c1 - c0], lhsv, rhsv, rhsv, c1 - c0)
                    for h in range(h0, h1):
                        nc.vector.tensor_scalar_mul(
                            v_aug[:, tt, h, 0:HD],
                            pmv[:, h * HD - c0:(h + 1) * HD - c0], WSI,
                        )

            # interleave q/k head-pairs and v slices so attention on early
            # heads can start while later chunks are still being produced
            for j in range(8):
                emit_qk_dh(0, j)
                emit_qk_dh(1, j)
                if j in (1, 3, 5):
                    emit_v_slice({1: 0, 3: 1, 5: 2}[j])
        es_mod1.close()
        es_qw.close()
        if phase_limit <= 2:
            es_qk.close()
            return _truncate_out(tc, nc, out_dram)

        # ============ phase 3: attention ====================================
        es_ao = ExitStack()
        pastk = es_ao.enter_context(tc.tile_pool(name="pastk", bufs=1))
        attn_st = pastk.tile([128, KT, NT], FP8, name="attn_st")

        with tc.tile_pool(name="pexp", bufs=2) as pexp, \
             tc.tile_pool(name="pattn", bufs=2) as pattn, \
             tc.tile_pool(name="ps_s", bufs=2, space="PSUM") as ps_s, \
             tc.tile_pool(name="ps_av", bufs=2, space="PSUM") as ps_av:
            for h in range(H):
                exp_h = pexp.tile([128, NT // 128, NT], FP8, tag="exp",
                                  name="exp_h")
                for kt_i in range(NT // 128):
                    pss2 = ps_s.tile([128, 1024], F32, tag="s", name="pss2")
                    for n in range(2):
                        nsl = slice(n * 512, (n + 1) * 512)
                        nc.tensor.matmul(
                            pss2[:, nsl],
                            qk_st[:, 16 + h, kt_i * 128:(kt_i + 1) * 128],
                            qk_st[:, h, nsl], start=True, stop=True,
                            skip_group_check=True,
                        )
                    nc.scalar.activation(
                        exp_h[:, kt_i, :], pss2[:, :], AF.Exp, scale=ISC
                    )
                attn_h = pattn.tile([72, NT], FP8, tag="attnh", name="attn_h")
                for n in range(2):
                    nsl = slice(n * 512, (n + 1) * 512)
                    pav = ps_av.tile([97, 512], F32, tag="av", name="pav")
                    for kk in range(4):
                        nc.tensor.matmul(
                            pav[:, :], v_aug[:, 2 * kk:2 * kk + 2, h, :],
                            exp_h[:, 2 * kk:2 * kk + 2, nsl],
                            start=(kk == 0), stop=(kk == 3), perf_mode=DR,
                        )
                    recip = pattn.tile([1, 512], F32, tag="recip", bufs=2,
                                       name="recip")
                    nc.vector.reciprocal(recip[:, :], pav[96:97, :])
                    bca = pattn.tile([72, 512], F32, tag="bca", name="bca")
                    nc.gpsimd.partition_broadcast(bca[:, :], recip[:, :])
                    atf = pattn.tile([72, 512], F32, tag="atf", bufs=2,
                                     name="atf")
                    nc.vector.tensor_mul(atf[:, :], pav[0:72, :], bca[:, :])
                    nc.vector.tensor_scalar_add(
                        attn_h[:, nsl], atf[:, :], bv_pp[:, h:h + 1]
                    )
                for (kt_i, p0, ln, off) in _head_segs(h * HD, HD):
                    nc.sync.dma_start(
                        attn_st[p0:p0 + ln, kt_i, :], attn_h[off:off + ln, :]
                    )
                with tc.tile_wait_until(0.150 + 0.006 * h):
                    for k2 in range((h * MH) // H, ((h + 1) * MH) // H):
                        emit_w2_convert(k2)

            # proj chases head completion: its k-accumulation consumes
            # attn_st k-tiles as the covering heads finish
            for mo in range(KT):
                wp_t = pw_s.tile([128, KT, 128], F32, tag="ws", bufs=3,
                                 name="wp_t")
                nc.sync.dma_start(
                    wp_t[:, :, :],
                    ins["w_proj"][:, mo * 128:(mo + 1) * 128]
                    .rearrange("(k p) m -> p k m", p=128),
                )
                wp_8 = pw_s.tile([128, KT, 128], FP8, tag="wsb", bufs=3,
                                 name="wp_8")
                nc.gpsimd.tensor_scalar_mul(wp_8[:, :, :], wp_t[:, :, :],
                                            WS)
                for n in range(2):
                    nsl = slice(n * 512, (n + 1) * 512)
                    ps2p = ps_s.tile([128, 512], F32, tag="mm2", bufs=2,
                                     name="ps2p")

                    def lhsp(k0, kn):
                        w = wp_8[:, k0:k0 + kn, :]
                        return w if kn == 2 else wp_8[:, k0, :]

                    def rhsp(k0, kn, nsl=nsl):
                        a = attn_st[:, k0:k0 + kn, nsl]
                        return a if kn == 2 else attn_st[:, k0, nsl]

                    mm_group(ps2p[:, :], lhsp, rhsp, rhsp, 512)
                    t_sb4 = pattn.tile([128, 512], F32, tag="tsb", bufs=2,
                                       name="t_sb4")
                    nc.scalar.activation(
                        t_sb4[:, :], ps2p[:, :], AF.Identity,
                        bias=bproj_pp[:, mo:mo + 1], scale=WSI,
                    )
                    nc.vector.scalar_tensor_tensor(
                        xT[:, mo, nsl], t_sb4[:, :],
                        ada_pp[:, 2, mo:mo + 1],
                        xT[:, mo, nsl], ALU.mult, ALU.add,
                    )
        es_qk.close()
        es_ao.close()
        if phase_limit <= 3:
            es_ao.close()
            return _truncate_out(tc, nc, out_dram)

        # ============ phase 4: proj + residual1 + LN2 ========================
        es_f1h = ExitStack()
        pf1h = es_f1h.enter_context(
            tc.tile_pool(name="pf1h", bufs=1, side="right"))
        F1H = 18   # fc1 m-tiles pre-converted during phase 4
        fc1_8 = pf1h.tile([128, KT, F1H * 128], FP8, name="fc1_8")
        es_mod2 = ExitStack()
        pmod2 = es_mod2.enter_context(
            tc.tile_pool(name="pmod2", bufs=1, side="right"))
        mod2T = pmod2.tile([128, KT, NT], FP8, name="mod2T")

        def emit_f1h_convert(p4f, mo):
            wfh_t = p4f.tile([128, KT, 128], F32, tag="wsf", bufs=6,
                             name="wfh_t")
            nc.sync.dma_start(
                wfh_t[:, :, :],
                ins["w_fc1"][:, mo * 128:(mo + 1) * 128]
                .rearrange("(k p) m -> p k m", p=128),
            )
            if mo % 3 == 2:
                nc.gpsimd.tensor_scalar_mul(
                    fc1_8[:, :, mo * 128:(mo + 1) * 128], wfh_t[:, :, :], WS)
            else:
                nc.scalar.activation(
                    fc1_8[:, :, mo * 128:(mo + 1) * 128], wfh_t[:, :, :],
                    AF.Identity, scale=WS)

        with tc.tile_pool(name="pst4", bufs=1) as pst4, \
             tc.tile_pool(name="pln4", bufs=1) as pln4, \
             tc.tile_pool(name="p4f", bufs=1) as p4f:
            for mo in range(F1H):
                emit_f1h_convert(p4f, mo)
            with tc.tile_pool(name="ps_st2", bufs=4, space="PSUM") as ps_st2:
                _ln_mod(tc, nc, xT, mod2T, ada_pp, 3, 4, ones_col,
                        pst4, pln4, ps_st2)
        if phase_limit <= 4:
            es_mod2.close()
            return _truncate_out(tc, nc, out_dram)

        # ============ phase 5: FFN + output =================================
        # half-granular pipeline: fc1 half-0 (prefetched weights) starts as
        # soon as mod2T half-0 exists; fc2 half-0 chases; streamed fc1
        # weights cover both halves between the two prefetched passes.
        with tc.tile_pool(name="p5w", bufs=1) as p5w, \
             tc.tile_pool(name="p5h", bufs=1) as p5h, \
             tc.tile_pool(name="ps_5", bufs=1, space="PSUM") as ps_5:
            hT_sb = p5h.tile([128, MH, NT], FP8, name="hT_sb")

            def fc1_group(mo, n, wf_8):
                nsl = slice(n * 512, (n + 1) * 512)
                pf1 = ps_5.tile([128, 512], F32, tag="f1", bufs=3,
                                name="pf1")

                def lhsf(k0, kn, mo=mo, wf_8=wf_8):
                    if wf_8 is None:
                        msl = slice(mo * 128, (mo + 1) * 128)
                        w = fc1_8[:, k0:k0 + kn, msl]
                        return w if kn == 2 else fc1_8[:, k0, msl]
                    w = wf_8[:, k0:k0 + kn, :]
                    return w if kn == 2 else wf_8[:, k0, :]

                def rhsf(k0, kn, nsl=nsl):
                    m = mod2T[:, k0:k0 + kn, nsl]
                    return m if kn == 2 else mod2T[:, k0, nsl]

                mm_group(pf1[:, :], lhsf, rhsf, rhsf, 512)
                nc.scalar.activation(
                    hT_sb[:, mo, nsl], pf1[:, :], AF.Gelu_apprx_tanh,
                    bias=bfc1_pp[:, mo:mo + 1], scale=WSI,
                )

            def fc2_m(m, n):
                nsl = slice(n * 512, (n + 1) * 512)
                pf2 = ps_5.tile([128, 512], F32, tag="f2", bufs=2,
                                name="pf2")
                for kk in range(MH // 2):
                    nc.tensor.matmul(
                        pf2[:, :],
                        w2_sb[:, 2 * kk:2 * kk + 2, m * 128:(m + 1) * 128],
                        hT_sb[:, 2 * kk:2 * kk + 2, nsl],
                        start=(kk == 0), stop=(kk == MH // 2 - 1),
                        perf_mode=DR,
                    )
                t2 = p5w.tile([128, 512], F32, tag="tsb", bufs=2, name="t2")
                nc.scalar.activation(
                    t2[:, :], pf2[:, :], AF.Identity,
                    bias=bfc2_pp[:, m:m + 1], scale=WSI,
                )
                nc.vector.scalar_tensor_tensor(
                    xT[:, m, nsl], t2[:, :],
                    ada_pp[:, 5, m:m + 1], xT[:, m, nsl],
                    ALU.mult, ALU.add,
                )

            # fc1 prefetched half-0, then streamed mo both halves, then
            # prefetched half-1 (fc2 half-0 can start during the latter)
            for mo in range(F1H):
                fc1_group(mo, 0, None)
            for mo in range(F1H, MH):
                wf_t = pw_s.tile([128, KT, 128], F32, tag="ws", bufs=3,
                                 name="wf_t")
                nc.sync.dma_start(
                    wf_t[:, :, :],
                    ins["w_fc1"][:, mo * 128:(mo + 1) * 128]
                    .rearrange("(k p) m -> p k m", p=128),
                )
                wf_8 = pw_s.tile([128, KT, 128], FP8, tag="wsb",
                                 bufs=3, name="wf_8")
                if mo % 2 == 0:
                    nc.gpsimd.tensor_scalar_mul(wf_8[:, :, :],
                                                wf_t[:, :, :], WS)
                else:
                    nc.vector.tensor_scalar_mul(wf_8[:, :, :],
                                                wf_t[:, :, :], WS)
                fc1_group(mo, 0, wf_8)
                fc1_group(mo, 1, wf_8)
            for mo in range(F1H):
                fc1_group(mo, 1, None)

            ot2 = {}
            for n in range(2):
                for m in range(KT):
                    fc2_m(m, n)
                    # transpose this half's token tiles; store [128,256]
                    # chunks once both m's of a pair are done
                    if m % 2 == 0:
                        ot2[n] = p5w.tile([128, 4, 256], F32, tag="ot2",
                                          bufs=2, name="ot2")
                    sl = slice(128 * (m % 2), 128 * (m % 2) + 128)
                    for tt in range(4 * n, 4 * n + 4):
                        pt = ps_5.tile([128, 128], F32, tag="tro", bufs=2,
                                       name="pt6")
                        nc.tensor.transpose(
                            pt[:, :], xT[:, m, tt * 128:(tt + 1) * 128],
                            ident[:, :],
                        )
                        nc.vector.tensor_copy(ot2[n][:, tt - 4 * n, sl],
                                              pt[:, :])
                        if m % 2 == 1 or m == KT - 1:
                            w = 128 * (m % 2) + 128
                            nc.sync.dma_start(
                                out_dram[tt * 128:(tt + 1) * 128,
                                         (m - m % 2) * 128:
                                         (m - m % 2) * 128 + w],
                                ot2[n][:, tt - 4 * n, 0:w],
                            )
        es_mod2.close()
        es_f1h.close()


_LOCK = threading.Lock()
_PROG = None


def _get_program():
    global _PROG
    with _LOCK:
        if _PROG is None:
            _PROG = _build_program()
    return _PROG


def _make_in_maps(inputs):
    arrs = {k: np.ascontiguousarray(np.asarray(v, dtype=np.float32))
            for k, v in inputs.items()}
    in_maps = []
    for c in range(NCORES):
        m = {k: v for k, v in arrs.items()
             if k not in ("x", "t_emb", "w_ada")}
        m["x"] = np.ascontiguousarray(arrs["x"][c])
        if ADA_SPLIT:
            m["t_emb"] = arrs["t_emb"]
            m["w_ada"] = np.ascontiguousarray(
                arrs["w_ada"][:, c * ADA_COLS:(c + 1) * ADA_COLS])
        else:
            m["t_emb"] = np.ascontiguousarray(arrs["t_emb"][c])
            m["w_ada"] = arrs["w_ada"]
        in_maps.append(m)
    return in_maps


def kernel(**inputs):
    nc = _get_program()
    res = run_bass_kernel_spmd(nc, _make_in_maps(inputs),
                               core_ids=list(range(NCORES)))
    return np.stack([r["out"] for r in res.results], axis=0)


def kernel_traced(inputs, **kw):
    """test-harness helper: returns full BassKernelResults with trace."""
    nc = _get_program()
    return run_bass_kernel_spmd(
        nc, _make_in_maps(inputs), core_ids=list(range(NCORES)), trace=True, **kw
    )


# revision 83
# speedup vs baseline: 1.7071x; 1.0150x over previous
"""DiT block kernel for Trainium2 (Bass/Tile), 8-core data parallel.

Shapes (hardcoded from the problem spec):
  x: (8, 1024, 1152), t_emb: (8, 1152)
  w_qkv (1152, 3456), w_proj (1152, 1152), w_fc1 (1152, 4608),
  w_fc2 (4608, 1152), w_ada (1152, 6912) + biases.

Strategy: batch-parallel across 8 cores (one batch element each).
Activations live transposed [D on partitions, tokens free]; projections are
out_T = W.T @ x_T with lhsT = W as stored.  All large GEMMs run in fp8e4
DoubleRow (two 128-deep k-tiles per PE pass); weights are scaled by 32 into
fp8 to stay in the normal range, compensated in the PSUM epilogue scale.
LayerNorm stats use f32r ones-column matmuls; softmax runs transposed with
no max subtraction and a ones-column appended to V for the denominators.
adaLN is contraction-split across the 8 cores (each core loads 1/8 of
w_ada, computes partial rows for all batches, then one small AllToAll
redistributes) -- gated by BASS_ADA_SPLIT.
"""

import os
import threading
from contextlib import ExitStack

import numpy as np

import concourse.bass as bass
import concourse.mybir as mybir
import concourse.tile as tile
from concourse import bacc
from concourse.bass_utils import run_bass_kernel_spmd
from concourse.masks import make_identity

F32 = mybir.dt.float32
F32R = mybir.dt.float32r
BF16 = mybir.dt.bfloat16
FP8 = mybir.dt.float8e4
AF = mybir.ActivationFunctionType
ALU = mybir.AluOpType
DR = mybir.MatmulPerfMode.DoubleRow

NCORES = 8
D = 1152
NT = 1024          # tokens per core (batch element)
KT = D // 128      # 9 partition-tiles of D
H = 16
HD = 72
HID = 4 * D        # 4608
MH = HID // 128    # 36
EPS = 1e-6
ISC = 1.0 / float(np.sqrt(HD))
WS = 32.0          # weight upscale into fp8e4 (avoids subnormals)
WSI = 1.0 / WS

ADA_SPLIT = os.environ.get("BASS_ADA_SPLIT", "1") == "1"
ADA_COLS = 6 * D // NCORES   # 864 columns of w_ada per core when split

# v output column slices aligned to head boundaries
V_SLICES = [(0, 432, 0, 6), (432, 864, 6, 12), (864, 1152, 12, 16)]


def _r(ap):
    return ap.bitcast(F32R)


def _head_segs(d0, n):
    """Split logical rows [d0, d0+n) of a [*,128]-tiled stacked tensor into
    (ktile, part0, length, dst_offset) segments within 128-partition tiles."""
    segs = []
    off = 0
    while n > 0:
        kt_i, p0 = divmod(d0, 128)
        ln = min(n, 128 - p0)
        segs.append((kt_i, p0, ln, off))
        d0 += ln
        off += ln
        n -= ln
    return segs


def _build_program():
    nc = bacc.Bacc(
        "TRN2", target_bir_lowering=False, debug=False, enable_asserts=False,
        num_devices=NCORES,
    )
    ins = {}
    ins["x"] = nc.dram_tensor("x", [NT, D], F32, kind="ExternalInput").ap()
    if ADA_SPLIT:
        ins["t_emb"] = nc.dram_tensor(
            "t_emb", [NCORES, D], F32, kind="ExternalInput").ap()
        ins["w_ada"] = nc.dram_tensor(
            "w_ada", [D, ADA_COLS], F32, kind="ExternalInput").ap()
    else:
        ins["t_emb"] = nc.dram_tensor(
            "t_emb", [D], F32, kind="ExternalInput").ap()
        ins["w_ada"] = nc.dram_tensor(
            "w_ada", [D, 6 * D], F32, kind="ExternalInput").ap()
    for name, shape in [
        ("w_qkv", [D, 3 * D]), ("b_qkv", [3 * D]),
        ("w_proj", [D, D]), ("b_proj", [D]),
        ("w_fc1", [D, HID]), ("b_fc1", [HID]),
        ("w_fc2", [HID, D]), ("b_fc2", [D]),
        ("b_ada", [6 * D]),
    ]:
        ins[name] = nc.dram_tensor(name, shape, F32, kind="ExternalInput").ap()
    out_dram = nc.dram_tensor("out", [NT, D], F32, kind="ExternalOutput").ap()

    with tile.TileContext(nc) as tc:
        _body(tc, ins, out_dram)
    nc.compile()
    return nc


def _ln_mod(tc, nc, src, dst, ada_pp, shift_c, scale_c, ones_col,
            pst, pln, ps_st, mul_pool=False):
    """dst[:,k,:] = fp8((src-mean)*rstd * ada[scale_c] + ada[shift_c]).
    Stats over the partition (D) axis per token via f32r ones matmuls."""
    ps_x, ps_q, st = {}, {}, {}
    for n in range(2):
        nsl = slice(n * 512, (n + 1) * 512)
        ps_x[n] = ps_st.tile([1, 512], F32, tag="st", name=f"psx{n}")
        ps_q[n] = ps_st.tile([1, 512], F32, tag="st", name=f"psq{n}")
        for k in range(KT):
            xb = pln.tile([128, 512], BF16, tag="xb", bufs=2, name="xb")
            nc.scalar.copy(xb[:, :], src[:, k, nsl])
            sq = pln.tile([128, 512], BF16, tag="sqb", bufs=2, name="sq")
            nc.vector.tensor_mul(sq[:, :], xb[:, :], xb[:, :])
            nc.tensor.matmul(
                ps_x[n][:, :], ones_col[:, :], xb[:, :],
                start=(k == 0), stop=(k == KT - 1), skip_group_check=True,
            )
            nc.tensor.matmul(
                ps_q[n][:, :], ones_col[:, :], sq[:, :],
                start=(k == 0), stop=(k == KT - 1), skip_group_check=True,
            )
    eps_sb = pst.tile([1, 1], F32, tag="eps", bufs=1, name="eps_sb")
    nc.vector.memset(eps_sb[:, :], EPS)
    for n in range(2):
        # rows: 0 = mean, 1 = E[x^2] -> rstd
        st[n] = pst.tile([1, 2, 512], F32, tag="lnst", bufs=2, name=f"st{n}")
        nc.vector.tensor_scalar_mul(st[n][:, 0, :], ps_x[n][:, :], 1.0 / D)
        work = pst.tile([1, 512], F32, tag="lnwork", bufs=2, name="work")
        nc.vector.tensor_mul(work[:, :], st[n][:, 0, :], st[n][:, 0, :])
        nc.vector.scalar_tensor_tensor(
            st[n][:, 1, :], ps_q[n][:, :], 1.0 / D, work[:, :],
            ALU.mult, ALU.subtract,
        )
        nc.scalar.activation(st[n][:, 1, :], st[n][:, 1, :], AF.Sqrt,
                             bias=eps_sb[:, :], scale=1.0)
        nc.vector.reciprocal(st[n][:, 1, :], st[n][:, 1, :])
    for n in range(2):
        nsl = slice(n * 512, (n + 1) * 512)
        meanB = pln.tile([128, 512], F32, tag="meanB", bufs=2, name="meanB")
        rstdB = pln.tile([128, 512], F32, tag="rstdB", bufs=2, name="rstdB")
        nc.gpsimd.partition_broadcast(meanB[:, :], st[n][:, 0, :])
        nc.gpsimd.partition_broadcast(rstdB[:, :], st[n][:, 1, :])
        for k in range(KT):
            tmp = pln.tile([128, 512], F32, tag="lnt", bufs=2, name="tmp")
            nc.vector.tensor_sub(tmp[:, :], src[:, k, nsl], meanB[:, :])
            meng = nc.gpsimd if (mul_pool and k % 2 == 0) else nc.vector
            meng.tensor_mul(tmp[:, :], tmp[:, :], rstdB[:, :])
            nc.gpsimd.tensor_scalar(
                dst[:, k, nsl], tmp[:, :],
                ada_pp[:, scale_c, k:k + 1], ada_pp[:, shift_c, k:k + 1],
                ALU.mult, ALU.add,
            )


def _truncate_out(tc, nc, out_dram):
    with tc.tile_pool(name="ptrunc", bufs=1) as p:
        z = p.tile([128, D], F32, name="z")
        nc.vector.memset(z[:, :], 0.0)
        for tt in range(NT // 128):
            nc.sync.dma_start(out_dram[tt * 128:(tt + 1) * 128, :], z[:, :])


def _body(tc, ins, out_dram):
    nc = tc.nc
    phase_limit = float(os.environ.get("BASS_PHASES", "6"))
    ctx = ExitStack()
    with ctx:
        dram = ctx.enter_context(tc.tile_pool(name="dram", bufs=1, space="DRAM"))
        if ADA_SPLIT:
            ada_part_dr = dram.tile([NCORES * ADA_COLS], F32)
            ada_gath_dr = dram.tile([NCORES * ADA_COLS], F32)
        else:
            ada_dr = dram.tile([6 * D], F32)

        pers = ctx.enter_context(tc.tile_pool(name="pers", bufs=1))
        ident = pers.tile([128, 128], F32)
        make_identity(nc, ident[:, :])
        ones_col = pers.tile([128, 1], BF16)
        nc.vector.memset(ones_col[:, :], 1.0)

        # silu(t_emb): split case holds all 8 batches, else just our own
        NB = NCORES if ADA_SPLIT else 1
        t_pp = pers.tile([128, NB, KT], F32)
        if ADA_SPLIT:
            nc.sync.dma_start(
                t_pp[:, :, :],
                ins["t_emb"].rearrange("b (k p) -> p b k", p=128))
        else:
            nc.sync.dma_start(
                t_pp[:, 0, :], ins["t_emb"].rearrange("(k p) -> p k", p=128))
        t_sb = pers.tile([128, NB, KT], BF16)
        nc.scalar.activation(t_sb[:, :, :], t_pp[:, :, :], AF.Silu)

        bqk72 = pers.tile([72, 32], F32)      # q,k bias per 72-head chunk
        bv_pp = pers.tile([72, H], F32)
        bproj_pp = pers.tile([128, KT], F32)
        bfc1_pp = pers.tile([128, MH], F32)
        bfc2_pp = pers.tile([128, KT], F32)
        bada_pp = pers.tile([128, 6, KT], F32)
        ada_pp = pers.tile([128, 6, KT], F32)

        def emit_bias_loads():
            nc.sync.dma_start(
                bqk72[:, :],
                ins["b_qkv"][0:2 * D].rearrange("(c p) -> p c", p=72))
            nc.sync.dma_start(
                bv_pp[:, :],
                ins["b_qkv"][2 * D:3 * D].rearrange("(h p) -> p h", p=72))
            nc.sync.dma_start(
                bproj_pp[:, :], ins["b_proj"].rearrange("(m p) -> p m", p=128))
            nc.sync.dma_start(
                bfc1_pp[:, :], ins["b_fc1"].rearrange("(m p) -> p m", p=128))
            nc.sync.dma_start(
                bfc2_pp[:, :], ins["b_fc2"].rearrange("(m p) -> p m", p=128))
            nc.sync.dma_start(
                bada_pp[:, :, :],
                ins["b_ada"].rearrange("(c k p) -> p c k", k=KT, p=128))

        xT = pers.tile([128, KT, NT], F32)   # becomes x2T, then out_T
        w2_sb = pers.tile([128, MH, D], FP8)  # fc2 weights, fp8*WS
        # weight-stream pool spanning phases 4-5 (proj/fc1 prefetch)
        pw_s = ctx.enter_context(tc.tile_pool(name="pw_s", bufs=1))
        # attn output store: allocated early so attention-phase pools do not
        # sit in freed qkv space (space-reuse would serialize the phases)
        es_ao = ExitStack()
        pastk = es_ao.enter_context(tc.tile_pool(name="pastk", bufs=1))
        attn_st = pastk.tile([128, KT, NT], FP8, name="attn_st")
        # qkv weight pool: lives through attention (closed at phase 4)
        es_qw = ExitStack()
        pqw = es_qw.enter_context(tc.tile_pool(name="pqw", bufs=1))

        # ============ phase 1: ada, x load+transpose, LN1 ====================
        es_mod1 = ExitStack()
        pmod1 = es_mod1.enter_context(tc.tile_pool(name="pmod1", bufs=1))
        mod1T = pmod1.tile([128, KT, NT], FP8, name="mod1T")

        with tc.tile_pool(name="p1w", bufs=1) as p1w, \
             tc.tile_pool(name="pst", bufs=1) as pst, \
             tc.tile_pool(name="pln", bufs=1) as pln:
            with tc.tile_pool(name="ps_pro", bufs=2, space="PSUM") as ps_pro, \
                 tc.tile_pool(name="pxin", bufs=3) as pxin, \
                 tc.tile_pool(name="ps_tr", bufs=3, space="PSUM") as ps_tr:

                def emit_transpose_block(tt):
                    xin = pxin.tile([128, D], F32, tag="xin", bufs=3,
                                    name="xin")
                    nc.sync.dma_start(
                        xin[:, :], ins["x"][tt * 128:(tt + 1) * 128, :])
                    for kd in range(KT):
                        pt = ps_tr.tile([128, 128], F32, tag="ptr", name="pt")
                        nc.tensor.transpose(
                            pt[:, :], xin[:, kd * 128:(kd + 1) * 128],
                            ident[:, :],
                        )
                        tsl = slice(tt * 128, (tt + 1) * 128)
                        if kd % 2 == 0:
                            nc.vector.tensor_copy(xT[:, kd, tsl], pt[:, :])
                        else:
                            nc.scalar.copy(xT[:, kd, tsl], pt[:, :])

                def emit_ada_split():
                    # partial rows for ALL batches over our 1/8 of w_ada,
                    # then AllToAll redistributes so each core gets its row.
                    ada_sb = pst.tile([NCORES, ADA_COLS], F32, tag="adasb",
                                      bufs=1, name="ada_sb")
                    for c in range(2):
                        csl = slice(c * 432, (c + 1) * 432)
                        wada_t = p1w.tile([128, KT, 432], F32, tag="wada",
                                          bufs=1, name="wada_t")
                        nc.sync.dma_start(
                            wada_t[:, :, :],
                            ins["w_ada"][:, csl]
                            .rearrange("(k p) m -> p k m", p=128),
                        )
                        wada_b = p1w.tile([128, KT, 432], BF16, tag="wadab",
                                          bufs=1, name="wada_b")
                        nc.gpsimd.tensor_copy(wada_b[:, :, :], wada_t[:, :, :])
                        pa = ps_pro.tile([NCORES, 432], F32, tag="psada",
                                         bufs=2, name="pa")
                        for k in range(KT):
                            nc.tensor.matmul(
                                pa[:, :], t_sb[:, :, k], wada_b[:, k, :],
                                start=(k == 0), stop=(k == KT - 1),
                            )
                        nc.vector.tensor_copy(ada_sb[:, csl], pa[:, :])
                    nc.sync.dma_start(
                        ada_part_dr.opt().rearrange("(b m) -> b m", b=NCORES),
                        ada_sb[:, :])
                    nc.gpsimd.collective_compute(
                        "AllToAll", ALU.bypass,
                        replica_groups=[list(range(NCORES))],
                        ins=[ada_part_dr.opt()],
                        outs=[ada_gath_dr.opt()],
                    )
                    nc.sync.dma_start(
                        ada_pp[:, :, :],
                        ada_gath_dr.opt().rearrange(
                            "(c k p) -> p c k", c=6, k=KT, p=128),
                    )

                def emit_ada_chunk(n):
                    # fallback: full w_ada on-core, M=1 bf16 matmuls
                    pa = ps_pro.tile([1, 384], F32, tag="psada", name="pa")
                    for k in range(KT):
                        wada_t = p1w.tile([128, 384], F32, tag="wsk", bufs=4,
                                          name="wada_t")
                        nc.sync.dma_start(
                            wada_t[:, :],
                            ins["w_ada"][k * 128:(k + 1) * 128,
                                         n * 384:(n + 1) * 384],
                        )
                        wada_b = p1w.tile([128, 384], BF16, tag="wskb",
                                          bufs=4, name="wada_b")
                        nc.gpsimd.tensor_copy(wada_b[:, :], wada_t[:, :])
                        nc.tensor.matmul(
                            pa[:, :], t_sb[:, 0, k:k + 1], wada_b[:, :],
                            start=(k == 0), stop=(k == KT - 1),
                        )
                    asb = pst.tile([1, 384], F32, tag="asb", bufs=2, name="asb")
                    nc.vector.tensor_copy(asb[:, :], pa[:, :])
                    nc.sync.dma_start(
                        ada_dr[n * 384:(n + 1) * 384]
                        .rearrange("(a b) -> a b", a=1),
                        asb[0:1, :],
                    )

                emit_bias_loads()
                if ADA_SPLIT:
                    emit_ada_split()
                    for i in range(8):
                        emit_transpose_block(i)
                else:
                    for i in range(8):
                        emit_transpose_block(i)
                        if i < 8:
                            emit_ada_chunk(2 * i)
                            emit_ada_chunk(2 * i + 1)
                    for n in range(16, 18):
                        emit_ada_chunk(n)
                    nc.sync.dma_start(
                        ada_pp[:, :, :],
                        ada_dr.opt().rearrange(
                            "(c k p) -> p c k", c=6, k=KT, p=128),
                    )
                nc.vector.tensor_add(ada_pp[:, :, :], ada_pp[:, :, :],
                                     bada_pp[:, :, :])
                nc.vector.tensor_scalar_add(ada_pp[:, 1, :], ada_pp[:, 1, :],
                                            1.0)
                nc.vector.tensor_scalar_add(ada_pp[:, 4, :], ada_pp[:, 4, :],
                                            1.0)

            if phase_limit > 0.6:
                with tc.tile_pool(name="ps_st", bufs=4, space="PSUM") as ps_st:
                    _ln_mod(tc, nc, xT, mod1T, ada_pp, 0, 1, ones_col,
                            pst, pln, ps_st)

        if phase_limit <= 1:
            es_mod1.close()
            return _truncate_out(tc, nc, out_dram)

        # ============ phase 2: qkv ==========================================
        es_qk = ExitStack()
        pqks = es_qk.enter_context(tc.tile_pool(name="pqks", bufs=1, side="right"))
        # chunks 0..15 = q heads, 16..31 = k heads; fp8 true values
        qk_st = pqks.tile([72, 32, NT], FP8, name="qk_st")
        pvaug = es_qk.enter_context(
            tc.tile_pool(name="pvaug", bufs=1, side="right"))
        # per head: cols 0..72 = v (true values), col 96 = ones, 72..96 zero
        v_aug = pvaug.tile([128, NT // 128, H, 97], FP8, name="v_aug")
        nc.gpsimd.memset(v_aug[:, :, :, HD:97], 0.0)
        nc.gpsimd.memset(v_aug[:, :, :, 96:97], 1.0)

        def emit_w2_convert(k):
            w2src = pw_s.tile([128, D], F32, tag="w2src", bufs=2,
                              name="w2src")
            nc.sync.dma_start(
                w2src[:, :], ins["w_fc2"][k * 128:(k + 1) * 128, :]
            )
            nc.gpsimd.tensor_scalar_mul(w2_sb[:, k, :], w2src[:, :], WS)

        def mm_group(psl, lhs8, rhs8, rhs_k_of, N):
            """accumulate 9 k-tiles: 4 DoubleRow pairs + 1 plain fp8."""
            for kk in range(4):
                nc.tensor.matmul(
                    psl, lhs8(2 * kk, 2), rhs_k_of(2 * kk, 2),
                    start=(kk == 0), stop=False, perf_mode=DR,
                )
            nc.tensor.matmul(psl, lhs8(8, 1), rhs_k_of(8, 1),
                             start=False, stop=True)

        with tc.tile_pool(name="p2w", bufs=1) as p2w, \
             tc.tile_pool(name="ps_mm", bufs=4, space="PSUM") as ps_mm:

            def emit_qk_dh(sec, j):
                # sec 0 = q (w cols j*144), sec 1 = k (w cols 1152 + j*144)
                wq_t = pqw.tile([128, KT, 144], F32, tag="wsq", bufs=2,
                                name="wq_t")
                nc.sync.dma_start(
                    wq_t[:, :, :],
                    ins["w_qkv"][:, sec * D + j * 144:sec * D + (j + 1) * 144]
                    .rearrange("(k p) m -> p k m", p=128),
                )
                wq_8 = pqw.tile([128, KT, 144], FP8, tag="wsbq", bufs=2,
                                name="wq_8")
                nc.gpsimd.tensor_scalar_mul(wq_8[:, :, :], wq_t[:, :, :], WS)
                for i in range(2):
                    ch = 16 * sec + 2 * j + i
                    hsl = slice(72 * i, 72 * i + 72)
                    ps2 = ps_mm.tile([72, 1024], F32, tag="mm", bufs=2,
                                     name="ps2")
                    for n in range(2):
                        nsl = slice(n * 512, (n + 1) * 512)

                        def lhs8(k0, kn, hsl=hsl):
                            w = wq_8[:, k0:k0 + kn, hsl]
                            return w if kn == 2 else wq_8[:, k0, hsl]

                        def rhs8(k0, kn, nsl=nsl):
                            m = mod1T[:, k0:k0 + kn, nsl]
                            return m if kn == 2 else mod1T[:, k0, nsl]

                        mm_group(ps2[:, nsl], lhs8, rhs8, rhs8, 512)
                    nc.scalar.activation(
                        qk_st[:, ch, :], ps2[:, :], AF.Identity,
                        bias=bqk72[:, ch:ch + 1], scale=WSI,
                    )

            def emit_v_slice(si):
                (c0, c1, h0, h1) = V_SLICES[si]
                wv_t = p2w.tile([128, KT, 432], F32, tag="wv", bufs=1,
                                name="wv_t")
                nc.sync.dma_start(
                    wv_t[:, :, 0:c1 - c0],
                    ins["w_qkv"][:, 2 * D + c0:2 * D + c1]
                    .rearrange("(k p) m -> p k m", p=128),
                )
                wv_8 = p2w.tile([128, KT, 432], FP8, tag="wvb", bufs=2,
                                name="wv_8")
                nc.scalar.activation(wv_8[:, :, 0:c1 - c0],
                                     wv_t[:, :, 0:c1 - c0], AF.Identity,
                                     scale=WS)
                for tt in range(NT // 128):
                    pmv = ps_mm.tile([128, 512], F32, tag="mmv", bufs=2,
                                     name="pmv")

                    def lhsv(k0, kn, tt=tt):
                        m = mod1T[:, k0:k0 + kn, tt * 128:(tt + 1) * 128]
                        return m if kn == 2 else mod1T[:, k0, tt * 128:(tt + 1) * 128]

                    def rhsv(k0, kn, c0=c0, c1=c1):
                        w = wv_8[:, k0:k0 + kn, 0:c1 - c0]
                        return w if kn == 2 else wv_8[:, k0, 0:c1 - c0]

                    mm_group(pmv[:, 0:c1 - c0], lhsv, rhsv, rhsv, c1 - c0)
                    for h in range(h0, h1):
                        nc.vector.tensor_scalar_mul(
                            v_aug[:, tt, h, 0:HD],
                            pmv[:, h * HD - c0:(h + 1) * HD - c0], WSI,
                        )

            # interleave q/k head-pairs and v slices so attention on early
            # heads can start while later chunks are still being produced
            for j in range(8):
                emit_qk_dh(0, j)
                emit_qk_dh(1, j)
                if j in (1, 3, 5):
                    emit_v_slice({1: 0, 3: 1, 5: 2}[j])
        es_mod1.close()
        es_qw.close()
        if phase_limit <= 2:
            es_qk.close()
            return _truncate_out(tc, nc, out_dram)

        # ============ phase 3: attention ====================================
        es_ao = ExitStack()
        pastk = es_ao.enter_context(tc.tile_pool(name="pastk", bufs=1))
        attn_st = pastk.tile([128, KT, NT], FP8, name="attn_st")

        with tc.tile_pool(name="pexp", bufs=2) as pexp, \
             tc.tile_pool(name="pattn", bufs=2) as pattn, \
             tc.tile_pool(name="ps_s", bufs=2, space="PSUM") as ps_s, \
             tc.tile_pool(name="ps_av", bufs=2, space="PSUM") as ps_av:
            for h in range(H):
                exp_h = pexp.tile([128, NT // 128, NT], FP8, tag="exp",
                                  name="exp_h")
                for kt_i in range(NT // 128):
                    pss2 = ps_s.tile([128, 1024], F32, tag="s", name="pss2")
                    for n in range(2):
                        nsl = slice(n * 512, (n + 1) * 512)
                        nc.tensor.matmul(
                            pss2[:, nsl],
                            qk_st[:, 16 + h, kt_i * 128:(kt_i + 1) * 128],
                            qk_st[:, h, nsl], start=True, stop=True,
                            skip_group_check=True,
                        )
                    nc.scalar.activation(
                        exp_h[:, kt_i, :], pss2[:, :], AF.Exp, scale=ISC
                    )
                attn_h = pattn.tile([72, NT], FP8, tag="attnh", name="attn_h")
                for n in range(2):
                    nsl = slice(n * 512, (n + 1) * 512)
                    pav = ps_av.tile([97, 512], F32, tag="av", name="pav")
                    for kk in range(4):
                        nc.tensor.matmul(
                            pav[:, :], v_aug[:, 2 * kk:2 * kk + 2, h, :],
                            exp_h[:, 2 * kk:2 * kk + 2, nsl],
                            start=(kk == 0), stop=(kk == 3), perf_mode=DR,
                        )
                    recip = pattn.tile([1, 512], F32, tag="recip", bufs=2,
                                       name="recip")
                    nc.vector.reciprocal(recip[:, :], pav[96:97, :])
                    bca = pattn.tile([72, 512], F32, tag="bca", name="bca")
                    nc.gpsimd.partition_broadcast(bca[:, :], recip[:, :])
                    atf = pattn.tile([72, 512], F32, tag="atf", bufs=2,
                                     name="atf")
                    nc.vector.tensor_mul(atf[:, :], pav[0:72, :], bca[:, :])
                    nc.vector.tensor_scalar_add(
                        attn_h[:, nsl], atf[:, :], bv_pp[:, h:h + 1]
                    )
                for (kt_i, p0, ln, off) in _head_segs(h * HD, HD):
                    nc.sync.dma_start(
                        attn_st[p0:p0 + ln, kt_i, :], attn_h[off:off + ln, :]
                    )
                with tc.tile_wait_until(0.150 + 0.006 * h):
                    for k2 in range((h * MH) // H, ((h + 1) * MH) // H):
                        emit_w2_convert(k2)

            # proj chases head completion: its k-accumulation consumes
            # attn_st k-tiles as the covering heads finish
            for mo in range(KT):
                wp_t = pw_s.tile([128, KT, 128], F32, tag="ws", bufs=3,
                                 name="wp_t")
                nc.sync.dma_start(
                    wp_t[:, :, :],
                    ins["w_proj"][:, mo * 128:(mo + 1) * 128]
                    .rearrange("(k p) m -> p k m", p=128),
                )
                wp_8 = pw_s.tile([128, KT, 128], FP8, tag="wsb", bufs=3,
                                 name="wp_8")
                nc.gpsimd.tensor_scalar_mul(wp_8[:, :, :], wp_t[:, :, :],
                                            WS)
                for n in range(2):
                    nsl = slice(n * 512, (n + 1) * 512)
                    ps2p = ps_s.tile([128, 512], F32, tag="mm2", bufs=2,
                                     name="ps2p")

                    def lhsp(k0, kn):
                        w = wp_8[:, k0:k0 + kn, :]
                        return w if kn == 2 else wp_8[:, k0, :]

                    def rhsp(k0, kn, nsl=nsl):
                        a = attn_st[:, k0:k0 + kn, nsl]
                        return a if kn == 2 else attn_st[:, k0, nsl]

                    mm_group(ps2p[:, :], lhsp, rhsp, rhsp, 512)
                    t_sb4 = pattn.tile([128, 512], F32, tag="tsb", bufs=3,
                                       name="t_sb4")
                    nc.scalar.activation(
                        t_sb4[:, :], ps2p[:, :], AF.Identity,
                        bias=bproj_pp[:, mo:mo + 1], scale=WSI,
                    )
                    nc.vector.scalar_tensor_tensor(
                        xT[:, mo, nsl], t_sb4[:, :],
                        ada_pp[:, 2, mo:mo + 1],
                        xT[:, mo, nsl], ALU.mult, ALU.add,
                    )
        es_qk.close()
        es_ao.close()
        if phase_limit <= 3:
            es_ao.close()
            return _truncate_out(tc, nc, out_dram)

        # ============ phase 4: proj + residual1 + LN2 ========================
        es_f1h = ExitStack()
        pf1h = es_f1h.enter_context(
            tc.tile_pool(name="pf1h", bufs=1, side="right"))
        F1H = 18   # fc1 m-tiles pre-converted during phase 4
        fc1_8 = pf1h.tile([128, KT, F1H * 128], FP8, name="fc1_8")
        es_mod2 = ExitStack()
        pmod2 = es_mod2.enter_context(
            tc.tile_pool(name="pmod2", bufs=1, side="right"))
        mod2T = pmod2.tile([128, KT, NT], FP8, name="mod2T")

        def emit_f1h_convert(p4f, mo):
            wfh_t = p4f.tile([128, KT, 128], F32, tag="wsf", bufs=6,
                             name="wfh_t")
            nc.sync.dma_start(
                wfh_t[:, :, :],
                ins["w_fc1"][:, mo * 128:(mo + 1) * 128]
                .rearrange("(k p) m -> p k m", p=128),
            )
            if mo % 3 == 2:
                nc.gpsimd.tensor_scalar_mul(
                    fc1_8[:, :, mo * 128:(mo + 1) * 128], wfh_t[:, :, :], WS)
            else:
                nc.scalar.activation(
                    fc1_8[:, :, mo * 128:(mo + 1) * 128], wfh_t[:, :, :],
                    AF.Identity, scale=WS)

        with tc.tile_pool(name="pst4", bufs=1) as pst4, \
             tc.tile_pool(name="pln4", bufs=1) as pln4, \
             tc.tile_pool(name="p4f", bufs=1) as p4f:
            for mo in range(F1H):
                emit_f1h_convert(p4f, mo)
            with tc.tile_pool(name="ps_st2", bufs=4, space="PSUM") as ps_st2:
                _ln_mod(tc, nc, xT, mod2T, ada_pp, 3, 4, ones_col,
                        pst4, pln4, ps_st2)
        if phase_limit <= 4:
            es_mod2.close()
            return _truncate_out(tc, nc, out_dram)

        # ============ phase 5: FFN + output =================================
        # half-granular pipeline: fc1 half-0 (prefetched weights) starts as
        # soon as mod2T half-0 exists; fc2 half-0 chases; streamed fc1
        # weights cover both halves between the two prefetched passes.
        with tc.tile_pool(name="p5w", bufs=1) as p5w, \
             tc.tile_pool(name="p5h", bufs=1) as p5h, \
             tc.tile_pool(name="ps_5", bufs=1, space="PSUM") as ps_5:
            hT_sb = p5h.tile([128, MH, NT], FP8, name="hT_sb")

            def fc1_group(mo, n, wf_8):
                nsl = slice(n * 512, (n + 1) * 512)
                pf1 = ps_5.tile([128, 512], F32, tag="f1", bufs=3,
                                name="pf1")

                def lhsf(k0, kn, mo=mo, wf_8=wf_8):
                    if wf_8 is None:
                        msl = slice(mo * 128, (mo + 1) * 128)
                        w = fc1_8[:, k0:k0 + kn, msl]
                        return w if kn == 2 else fc1_8[:, k0, msl]
                    w = wf_8[:, k0:k0 + kn, :]
                    return w if kn == 2 else wf_8[:, k0, :]

                def rhsf(k0, kn, nsl=nsl):
                    m = mod2T[:, k0:k0 + kn, nsl]
                    return m if kn == 2 else mod2T[:, k0, nsl]

                mm_group(pf1[:, :], lhsf, rhsf, rhsf, 512)
                nc.scalar.activation(
                    hT_sb[:, mo, nsl], pf1[:, :], AF.Gelu_apprx_tanh,
                    bias=bfc1_pp[:, mo:mo + 1], scale=WSI,
                )

            def fc2_m(m, n):
                nsl = slice(n * 512, (n + 1) * 512)
                pf2 = ps_5.tile([128, 512], F32, tag="f2", bufs=2,
                                name="pf2")
                for kk in range(MH // 2):
                    nc.tensor.matmul(
                        pf2[:, :],
                        w2_sb[:, 2 * kk:2 * kk + 2, m * 128:(m + 1) * 128],
                        hT_sb[:, 2 * kk:2 * kk + 2, nsl],
                        start=(kk == 0), stop=(kk == MH // 2 - 1),
                        perf_mode=DR,
                    )
                t2 = p5w.tile([128, 512], F32, tag="tsb", bufs=2, name="t2")
                nc.scalar.activation(
                    t2[:, :], pf2[:, :], AF.Identity,
                    bias=bfc2_pp[:, m:m + 1], scale=WSI,
                )
                nc.vector.scalar_tensor_tensor(
                    xT[:, m, nsl], t2[:, :],
                    ada_pp[:, 5, m:m + 1], xT[:, m, nsl],
                    ALU.mult, ALU.add,
                )

            # fc1 prefetched half-0, then streamed mo both halves, then
            # prefetched half-1 (fc2 half-0 can start during the latter)
            for mo in range(F1H):
                fc1_group(mo, 0, None)
            for mo in range(F1H, MH):
                wf_t = pw_s.tile([128, KT, 128], F32, tag="ws", bufs=3,
                                 name="wf_t")
                nc.sync.dma_start(
                    wf_t[:, :, :],
                    ins["w_fc1"][:, mo * 128:(mo + 1) * 128]
                    .rearrange("(k p) m -> p k m", p=128),
                )
                wf_8 = pw_s.tile([128, KT, 128], FP8, tag="wsb",
                                 bufs=3, name="wf_8")
                if mo % 2 == 0:
                    nc.gpsimd.tensor_scalar_mul(wf_8[:, :, :],
                                                wf_t[:, :, :], WS)
                else:
                    nc.vector.tensor_scalar_mul(wf_8[:, :, :],
                                                wf_t[:, :, :], WS)
                fc1_group(mo, 0, wf_8)
                fc1_group(mo, 1, wf_8)
            for mo in range(F1H):
                fc1_group(mo, 1, None)

            ot2 = {}
            for n in range(2):
                for m in range(KT):
                    fc2_m(m, n)
                    # transpose this half's token tiles; store [128,256]
                    # chunks once both m's of a pair are done
                    if m % 2 == 0:
                        ot2[n] = p5w.tile([128, 4, 256], F32, tag="ot2",
                                          bufs=2, name="ot2")
                    sl = slice(128 * (m % 2), 128 * (m % 2) + 128)
                    for tt in range(4 * n, 4 * n + 4):
                        pt = ps_5.tile([128, 128], F32, tag="tro", bufs=2,
                                       name="pt6")
                        nc.tensor.transpose(
                            pt[:, :], xT[:, m, tt * 128:(tt + 1) * 128],
                            ident[:, :],
                        )
                        nc.vector.tensor_copy(ot2[n][:, tt - 4 * n, sl],
                                              pt[:, :])
                        if m % 2 == 1 or m == KT - 1:
                            w = 128 * (m % 2) + 128
                            nc.sync.dma_start(
                                out_dram[tt * 128:(tt + 1) * 128,
                                         (m - m % 2) * 128:
                                         (m - m % 2) * 128 + w],
                                ot2[n][:, tt - 4 * n, 0:w],
                            )
        es_mod2.close()
        es_f1h.close()


_LOCK = threading.Lock()
_PROG = None


def _get_program():
    global _PROG
    with _LOCK:
        if _PROG is None:
            _PROG = _build_program()
    return _PROG


def _make_in_maps(inputs):
    arrs = {k: np.ascontiguousarray(np.asarray(v, dtype=np.float32))
            for k, v in inputs.items()}
    in_maps = []
    for c in range(NCORES):
        m = {k: v for k, v in arrs.items()
             if k not in ("x", "t_emb", "w_ada")}
        m["x"] = np.ascontiguousarray(arrs["x"][c])
        if ADA_SPLIT:
            m["t_emb"] = arrs["t_emb"]
            m["w_ada"] = np.ascontiguousarray(
                arrs["w_ada"][:, c * ADA_COLS:(c + 1) * ADA_COLS])
        else:
            m["t_emb"] = np.ascontiguousarray(arrs["t_emb"][c])
            m["w_ada"] = arrs["w_ada"]
        in_maps.append(m)
    return in_maps


def kernel(**inputs):
    nc = _get_program()
    res = run_bass_kernel_spmd(nc, _make_in_maps(inputs),
                               core_ids=list(range(NCORES)))
    return np.stack([r["out"] for r in res.results], axis=0)


def kernel_traced(inputs, **kw):
    """test-harness helper: returns full BassKernelResults with trace."""
    nc = _get_program()
    return run_bass_kernel_spmd(
        nc, _make_in_maps(inputs), core_ids=list(range(NCORES)), trace=True, **kw
    )


# revision 97
# speedup vs baseline: 1.7487x; 1.0243x over previous
"""DiT block kernel for Trainium2 (Bass/Tile), 8-core data parallel.

Shapes (hardcoded from the problem spec):
  x: (8, 1024, 1152), t_emb: (8, 1152)
  w_qkv (1152, 3456), w_proj (1152, 1152), w_fc1 (1152, 4608),
  w_fc2 (4608, 1152), w_ada (1152, 6912) + biases.

Strategy: batch-parallel across 8 cores (one batch element each).
Activations live transposed [D on partitions, tokens free]; projections are
out_T = W.T @ x_T with lhsT = W as stored.  All large GEMMs run in fp8e4
DoubleRow (two 128-deep k-tiles per PE pass); weights are scaled by 32 into
fp8 to stay in the normal range, compensated in the PSUM epilogue scale.
LayerNorm stats use f32r ones-column matmuls; softmax runs transposed with
no max subtraction and a ones-column appended to V for the denominators.
adaLN is contraction-split across the 8 cores (each core loads 1/8 of
w_ada, computes partial rows for all batches, then one small AllToAll
redistributes) -- gated by BASS_ADA_SPLIT.
"""

import os
import threading
from contextlib import ExitStack

import numpy as np

import concourse.bass as bass
import concourse.mybir as mybir
import concourse.tile as tile
from concourse import bacc
from concourse.bass_utils import run_bass_kernel_spmd
from concourse.masks import make_identity

F32 = mybir.dt.float32
F32R = mybir.dt.float32r
BF16 = mybir.dt.bfloat16
FP8 = mybir.dt.float8e4
AF = mybir.ActivationFunctionType
ALU = mybir.AluOpType
DR = mybir.MatmulPerfMode.DoubleRow

NCORES = 8
D = 1152
NT = 1024          # tokens per core (batch element)
KT = D // 128      # 9 partition-tiles of D
H = 16
HD = 72
HID = 4 * D        # 4608
MH = HID // 128    # 36
EPS = 1e-6
ISC = 1.0 / float(np.sqrt(HD))
WS = 32.0          # weight upscale into fp8e4 (avoids subnormals)
WSI = 1.0 / WS

ADA_SPLIT = os.environ.get("BASS_ADA_SPLIT", "1") == "1"
ADA_COLS = 6 * D // NCORES   # 864 columns of w_ada per core when split

# v output column slices aligned to head boundaries
V_SLICES = [(0, 432, 0, 6), (432, 864, 6, 12), (864, 1152, 12, 16)]


def _r(ap):
    return ap.bitcast(F32R)


def _head_segs(d0, n):
    """Split logical rows [d0, d0+n) of a [*,128]-tiled stacked tensor into
    (ktile, part0, length, dst_offset) segments within 128-partition tiles."""
    segs = []
    off = 0
    while n > 0:
        kt_i, p0 = divmod(d0, 128)
        ln = min(n, 128 - p0)
        segs.append((kt_i, p0, ln, off))
        d0 += ln
        off += ln
        n -= ln
    return segs


def _build_program():
    nc = bacc.Bacc(
        "TRN2", target_bir_lowering=False, debug=False, enable_asserts=False,
        num_devices=NCORES,
    )
    ins = {}
    ins["x"] = nc.dram_tensor("x", [NT, D], F32, kind="ExternalInput").ap()
    if ADA_SPLIT:
        ins["t_emb"] = nc.dram_tensor(
            "t_emb", [NCORES, D], F32, kind="ExternalInput").ap()
        ins["w_ada"] = nc.dram_tensor(
            "w_ada", [D, ADA_COLS], F32, kind="ExternalInput").ap()
    else:
        ins["t_emb"] = nc.dram_tensor(
            "t_emb", [D], F32, kind="ExternalInput").ap()
        ins["w_ada"] = nc.dram_tensor(
            "w_ada", [D, 6 * D], F32, kind="ExternalInput").ap()
    for name, shape in [
        ("w_qkv", [D, 3 * D]), ("b_qkv", [3 * D]),
        ("w_proj", [D, D]), ("b_proj", [D]),
        ("w_fc1", [D, HID]), ("b_fc1", [HID]),
        ("w_fc2", [HID, D]), ("b_fc2", [D]),
        ("b_ada", [6 * D]),
    ]:
        ins[name] = nc.dram_tensor(name, shape, F32, kind="ExternalInput").ap()
    out_dram = nc.dram_tensor("out", [NT, D], F32, kind="ExternalOutput").ap()

    with tile.TileContext(nc) as tc:
        _body(tc, ins, out_dram)
    nc.compile()
    return nc


def _ln_mod(tc, nc, src, dst, ada_pp, shift_c, scale_c, ones_col,
            pst, pln, ps_st, mul_pool=False):
    """dst[:,k,:] = fp8((src-mean)*rstd * ada[scale_c] + ada[shift_c]).
    Stats over the partition (D) axis per token via f32r ones matmuls."""
    ps_x, ps_q, st = {}, {}, {}
    for n in range(2):
        nsl = slice(n * 512, (n + 1) * 512)
        ps_x[n] = ps_st.tile([1, 512], F32, tag="st", name=f"psx{n}")
        ps_q[n] = ps_st.tile([1, 512], F32, tag="st", name=f"psq{n}")
        for k in range(KT):
            xb = pln.tile([128, 512], BF16, tag="xb", bufs=2, name="xb")
            nc.scalar.copy(xb[:, :], src[:, k, nsl])
            sq = pln.tile([128, 512], BF16, tag="sqb", bufs=2, name="sq")
            nc.vector.tensor_mul(sq[:, :], xb[:, :], xb[:, :])
            nc.tensor.matmul(
                ps_x[n][:, :], ones_col[:, :], xb[:, :],
                start=(k == 0), stop=(k == KT - 1), skip_group_check=True,
            )
            nc.tensor.matmul(
                ps_q[n][:, :], ones_col[:, :], sq[:, :],
                start=(k == 0), stop=(k == KT - 1), skip_group_check=True,
            )
    eps_sb = pst.tile([1, 1], F32, tag="eps", bufs=1, name="eps_sb")
    nc.vector.memset(eps_sb[:, :], EPS)
    for n in range(2):
        # rows: 0 = mean, 1 = E[x^2] -> rstd
        st[n] = pst.tile([1, 2, 512], F32, tag="lnst", bufs=2, name=f"st{n}")
        nc.vector.tensor_scalar_mul(st[n][:, 0, :], ps_x[n][:, :], 1.0 / D)
        work = pst.tile([1, 512], F32, tag="lnwork", bufs=2, name="work")
        nc.vector.tensor_mul(work[:, :], st[n][:, 0, :], st[n][:, 0, :])
        nc.vector.scalar_tensor_tensor(
            st[n][:, 1, :], ps_q[n][:, :], 1.0 / D, work[:, :],
            ALU.mult, ALU.subtract,
        )
        nc.scalar.activation(st[n][:, 1, :], st[n][:, 1, :], AF.Sqrt,
                             bias=eps_sb[:, :], scale=1.0)
        nc.vector.reciprocal(st[n][:, 1, :], st[n][:, 1, :])
    for n in range(2):
        nsl = slice(n * 512, (n + 1) * 512)
        meanB = pln.tile([128, 512], F32, tag="meanB", bufs=2, name="meanB")
        rstdB = pln.tile([128, 512], F32, tag="rstdB", bufs=2, name="rstdB")
        nc.gpsimd.partition_broadcast(meanB[:, :], st[n][:, 0, :])
        nc.gpsimd.partition_broadcast(rstdB[:, :], st[n][:, 1, :])
        for k in range(KT):
            tmp = pln.tile([128, 512], F32, tag="lnt", bufs=4, name="tmp")
            nc.vector.tensor_sub(tmp[:, :], src[:, k, nsl], meanB[:, :])
            meng = nc.gpsimd if (mul_pool and k % 2 == 0) else nc.vector
            meng.tensor_mul(tmp[:, :], tmp[:, :], rstdB[:, :])
            nc.gpsimd.tensor_scalar(
                dst[:, k, nsl], tmp[:, :],
                ada_pp[:, scale_c, k:k + 1], ada_pp[:, shift_c, k:k + 1],
                ALU.mult, ALU.add,
            )


def _truncate_out(tc, nc, out_dram):
    with tc.tile_pool(name="ptrunc", bufs=1) as p:
        z = p.tile([128, D], F32, name="z")
        nc.vector.memset(z[:, :], 0.0)
        for tt in range(NT // 128):
            nc.sync.dma_start(out_dram[tt * 128:(tt + 1) * 128, :], z[:, :])


def _body(tc, ins, out_dram):
    nc = tc.nc
    phase_limit = float(os.environ.get("BASS_PHASES", "6"))
    ctx = ExitStack()
    with ctx:
        dram = ctx.enter_context(tc.tile_pool(name="dram", bufs=1, space="DRAM"))
        if ADA_SPLIT:
            ada_part_dr = dram.tile([NCORES * ADA_COLS], F32)
            ada_gath_dr = dram.tile([NCORES * ADA_COLS], F32)
        else:
            ada_dr = dram.tile([6 * D], F32)

        pers = ctx.enter_context(tc.tile_pool(name="pers", bufs=1))
        ident = pers.tile([128, 128], F32)
        make_identity(nc, ident[:, :])
        ones_col = pers.tile([128, 1], BF16)
        nc.vector.memset(ones_col[:, :], 1.0)

        # silu(t_emb): split case holds all 8 batches, else just our own
        NB = NCORES if ADA_SPLIT else 1
        t_pp = pers.tile([128, NB, KT], F32)
        if ADA_SPLIT:
            nc.sync.dma_start(
                t_pp[:, :, :],
                ins["t_emb"].rearrange("b (k p) -> p b k", p=128))
        else:
            nc.sync.dma_start(
                t_pp[:, 0, :], ins["t_emb"].rearrange("(k p) -> p k", p=128))
        t_sb = pers.tile([128, NB, KT], BF16)
        nc.scalar.activation(t_sb[:, :, :], t_pp[:, :, :], AF.Silu)
        # preload activation tables so first exp/gelu hit no lazy table load
        warm = pers.tile([1, 2], F32)
        nc.scalar.activation(warm[:, 0:1], t_pp[0:1, 0, 0:1], AF.Exp)
        nc.scalar.activation(warm[:, 1:2], t_pp[0:1, 0, 0:1],
                             AF.Gelu_apprx_tanh)

        bqk72 = pers.tile([72, 32], F32)      # q,k bias per 72-head chunk
        bv_pp = pers.tile([72, H], F32)
        bproj_pp = pers.tile([128, KT], F32)
        bfc1_pp = pers.tile([128, MH], F32)
        bfc2_pp = pers.tile([128, KT], F32)
        bada_pp = pers.tile([128, 6, KT], F32)
        ada_pp = pers.tile([128, 6, KT], F32)

        def emit_bias_loads():
            nc.sync.dma_start(
                bqk72[:, :],
                ins["b_qkv"][0:2 * D].rearrange("(c p) -> p c", p=72))
            nc.sync.dma_start(
                bv_pp[:, :],
                ins["b_qkv"][2 * D:3 * D].rearrange("(h p) -> p h", p=72))
            nc.sync.dma_start(
                bproj_pp[:, :], ins["b_proj"].rearrange("(m p) -> p m", p=128))
            nc.sync.dma_start(
                bfc1_pp[:, :], ins["b_fc1"].rearrange("(m p) -> p m", p=128))
            nc.sync.dma_start(
                bfc2_pp[:, :], ins["b_fc2"].rearrange("(m p) -> p m", p=128))
            nc.sync.dma_start(
                bada_pp[:, :, :],
                ins["b_ada"].rearrange("(c k p) -> p c k", k=KT, p=128))

        xT = pers.tile([128, KT, NT], F32)   # becomes x2T, then out_T
        w2_sb = pers.tile([128, MH, D], FP8)  # fc2 weights, fp8*WS
        # weight-stream pool spanning phases 4-5 (proj/fc1 prefetch)
        pw_s = ctx.enter_context(tc.tile_pool(name="pw_s", bufs=1))
        # attn output store: allocated early so attention-phase pools do not
        # sit in freed qkv space (space-reuse would serialize the phases)
        es_ao = ExitStack()
        pastk = es_ao.enter_context(tc.tile_pool(name="pastk", bufs=1))
        attn_st = pastk.tile([128, KT, NT], FP8, name="attn_st")
        # qkv weight pool: lives through attention (closed at phase 4)
        es_qw = ExitStack()
        pqw = es_qw.enter_context(tc.tile_pool(name="pqw", bufs=1))

        # ============ phase 1: ada, x load+transpose, LN1 ====================
        es_mod1 = ExitStack()
        pmod1 = es_mod1.enter_context(tc.tile_pool(name="pmod1", bufs=1))
        mod1T = pmod1.tile([128, KT, NT], FP8, name="mod1T")

        with tc.tile_pool(name="p1w", bufs=1) as p1w, \
             tc.tile_pool(name="pst", bufs=1) as pst, \
             tc.tile_pool(name="pln", bufs=1) as pln:
            with tc.tile_pool(name="ps_pro", bufs=2, space="PSUM") as ps_pro, \
                 tc.tile_pool(name="pxin", bufs=3) as pxin, \
                 tc.tile_pool(name="ps_tr", bufs=3, space="PSUM") as ps_tr:

                def emit_transpose_block(tt):
                    xin = pxin.tile([128, D], F32, tag="xin", bufs=3,
                                    name="xin")
                    nc.sync.dma_start(
                        xin[:, :], ins["x"][tt * 128:(tt + 1) * 128, :])
                    for kd in range(KT):
                        pt = ps_tr.tile([128, 128], F32, tag="ptr", name="pt")
                        nc.tensor.transpose(
                            pt[:, :], xin[:, kd * 128:(kd + 1) * 128],
                            ident[:, :],
                        )
                        tsl = slice(tt * 128, (tt + 1) * 128)
                        if kd % 2 == 0:
                            nc.vector.tensor_copy(xT[:, kd, tsl], pt[:, :])
                        else:
                            nc.scalar.copy(xT[:, kd, tsl], pt[:, :])

                def emit_ada_split():
                    # partial rows for ALL batches over our 1/8 of w_ada,
                    # then AllToAll redistributes so each core gets its row.
                    ada_sb = pst.tile([NCORES, ADA_COLS], F32, tag="adasb",
                                      bufs=1, name="ada_sb")
                    for c in range(2):
                        csl = slice(c * 432, (c + 1) * 432)
                        wada_t = p1w.tile([128, KT, 432], F32, tag="wada",
                                          bufs=1, name="wada_t")
                        nc.sync.dma_start(
                            wada_t[:, :, :],
                            ins["w_ada"][:, csl]
                            .rearrange("(k p) m -> p k m", p=128),
                        )
                        wada_b = p1w.tile([128, KT, 432], BF16, tag="wadab",
                                          bufs=1, name="wada_b")
                        nc.gpsimd.tensor_copy(wada_b[:, :, :], wada_t[:, :, :])
                        pa = ps_pro.tile([NCORES, 432], F32, tag="psada",
                                         bufs=2, name="pa")
                        for k in range(KT):
                            nc.tensor.matmul(
                                pa[:, :], t_sb[:, :, k], wada_b[:, k, :],
                                start=(k == 0), stop=(k == KT - 1),
                            )
                        nc.vector.tensor_copy(ada_sb[:, csl], pa[:, :])
                    nc.sync.dma_start(
                        ada_part_dr.opt().rearrange("(b m) -> b m", b=NCORES),
                        ada_sb[:, :])
                    nc.gpsimd.collective_compute(
                        "AllToAll", ALU.bypass,
                        replica_groups=[list(range(NCORES))],
                        ins=[ada_part_dr.opt()],
                        outs=[ada_gath_dr.opt()],
                    )
                    nc.sync.dma_start(
                        ada_pp[:, :, :],
                        ada_gath_dr.opt().rearrange(
                            "(c k p) -> p c k", c=6, k=KT, p=128),
                    )

                def emit_ada_chunk(n):
                    # fallback: full w_ada on-core, M=1 bf16 matmuls
                    pa = ps_pro.tile([1, 384], F32, tag="psada", name="pa")
                    for k in range(KT):
                        wada_t = p1w.tile([128, 384], F32, tag="wsk", bufs=4,
                                          name="wada_t")
                        nc.sync.dma_start(
                            wada_t[:, :],
                            ins["w_ada"][k * 128:(k + 1) * 128,
                                         n * 384:(n + 1) * 384],
                        )
                        wada_b = p1w.tile([128, 384], BF16, tag="wskb",
                                          bufs=4, name="wada_b")
                        nc.gpsimd.tensor_copy(wada_b[:, :], wada_t[:, :])
                        nc.tensor.matmul(
                            pa[:, :], t_sb[:, 0, k:k + 1], wada_b[:, :],
                            start=(k == 0), stop=(k == KT - 1),
                        )
                    asb = pst.tile([1, 384], F32, tag="asb", bufs=2, name="asb")
                    nc.vector.tensor_copy(asb[:, :], pa[:, :])
                    nc.sync.dma_start(
                        ada_dr[n * 384:(n + 1) * 384]
                        .rearrange("(a b) -> a b", a=1),
                        asb[0:1, :],
                    )

                emit_bias_loads()
                if ADA_SPLIT:
                    emit_ada_split()
                    for i in range(8):
                        emit_transpose_block(i)
                else:
                    for i in range(8):
                        emit_transpose_block(i)
                        if i < 8:
                            emit_ada_chunk(2 * i)
                            emit_ada_chunk(2 * i + 1)
                    for n in range(16, 18):
                        emit_ada_chunk(n)
                    nc.sync.dma_start(
                        ada_pp[:, :, :],
                        ada_dr.opt().rearrange(
                            "(c k p) -> p c k", c=6, k=KT, p=128),
                    )
                nc.vector.tensor_add(ada_pp[:, :, :], ada_pp[:, :, :],
                                     bada_pp[:, :, :])
                nc.vector.tensor_scalar_add(ada_pp[:, 1, :], ada_pp[:, 1, :],
                                            1.0)
                nc.vector.tensor_scalar_add(ada_pp[:, 4, :], ada_pp[:, 4, :],
                                            1.0)

            if phase_limit > 0.6:
                with tc.tile_pool(name="ps_st", bufs=4, space="PSUM") as ps_st:
                    _ln_mod(tc, nc, xT, mod1T, ada_pp, 0, 1, ones_col,
                            pst, pln, ps_st)

        if phase_limit <= 1:
            es_mod1.close()
            return _truncate_out(tc, nc, out_dram)

        # ============ phase 2: qkv ==========================================
        es_qk = ExitStack()
        pqks = es_qk.enter_context(tc.tile_pool(name="pqks", bufs=1, side="right"))
        # chunks 0..15 = q heads, 16..31 = k heads; fp8 true values
        qk_st = pqks.tile([72, 32, NT], FP8, name="qk_st")
        pvaug = es_qk.enter_context(
            tc.tile_pool(name="pvaug", bufs=1, side="right"))
        # per head: cols 0..72 = v (true values), col 96 = ones, 72..96 zero
        v_aug = pvaug.tile([128, NT // 128, H, 97], FP8, name="v_aug")
        nc.gpsimd.memset(v_aug[:, :, :, HD:97], 0.0)
        nc.gpsimd.memset(v_aug[:, :, :, 96:97], 1.0)

        def emit_w2_convert(k):
            w2src = pw_s.tile([128, D], F32, tag="w2src", bufs=2,
                              name="w2src")
            nc.sync.dma_start(
                w2src[:, :], ins["w_fc2"][k * 128:(k + 1) * 128, :]
            )
            nc.gpsimd.tensor_scalar_mul(w2_sb[:, k, :], w2src[:, :], WS)

        def mm_group(psl, lhs8, rhs8, rhs_k_of, N):
            """accumulate 9 k-tiles: 4 DoubleRow pairs + 1 plain fp8."""
            for kk in range(4):
                nc.tensor.matmul(
                    psl, lhs8(2 * kk, 2), rhs_k_of(2 * kk, 2),
                    start=(kk == 0), stop=False, perf_mode=DR,
                )
            nc.tensor.matmul(psl, lhs8(8, 1), rhs_k_of(8, 1),
                             start=False, stop=True)

        with tc.tile_pool(name="p2w", bufs=1) as p2w, \
             tc.tile_pool(name="ps_mm", bufs=4, space="PSUM") as ps_mm:

            def emit_qk_dh(sec, j):
                # sec 0 = q (w cols j*144), sec 1 = k (w cols 1152 + j*144)
                wq_t = pqw.tile([128, KT, 144], F32, tag="wsq", bufs=2,
                                name="wq_t")
                nc.sync.dma_start(
                    wq_t[:, :, :],
                    ins["w_qkv"][:, sec * D + j * 144:sec * D + (j + 1) * 144]
                    .rearrange("(k p) m -> p k m", p=128),
                )
                wq_8 = pqw.tile([128, KT, 144], FP8, tag="wsbq", bufs=2,
                                name="wq_8")
                nc.gpsimd.tensor_scalar_mul(wq_8[:, :, :], wq_t[:, :, :], WS)
                for i in range(2):
                    ch = 16 * sec + 2 * j + i
                    hsl = slice(72 * i, 72 * i + 72)
                    ps2 = ps_mm.tile([72, 1024], F32, tag="mm", bufs=2,
                                     name="ps2")
                    for n in range(2):
                        nsl = slice(n * 512, (n + 1) * 512)

                        def lhs8(k0, kn, hsl=hsl):
                            w = wq_8[:, k0:k0 + kn, hsl]
                            return w if kn == 2 else wq_8[:, k0, hsl]

                        def rhs8(k0, kn, nsl=nsl):
                            m = mod1T[:, k0:k0 + kn, nsl]
                            return m if kn == 2 else mod1T[:, k0, nsl]

                        mm_group(ps2[:, nsl], lhs8, rhs8, rhs8, 512)
                    nc.scalar.activation(
                        qk_st[:, ch, :], ps2[:, :], AF.Identity,
                        bias=bqk72[:, ch:ch + 1], scale=WSI,
                    )

            def emit_v_slice(si):
                (c0, c1, h0, h1) = V_SLICES[si]
                wv_t = p2w.tile([128, KT, 432], F32, tag="wv", bufs=1,
                                name="wv_t")
                nc.sync.dma_start(
                    wv_t[:, :, 0:c1 - c0],
                    ins["w_qkv"][:, 2 * D + c0:2 * D + c1]
                    .rearrange("(k p) m -> p k m", p=128),
                )
                wv_8 = p2w.tile([128, KT, 432], FP8, tag="wvb", bufs=2,
                                name="wv_8")
                nc.scalar.activation(wv_8[:, :, 0:c1 - c0],
                                     wv_t[:, :, 0:c1 - c0], AF.Identity,
                                     scale=WS)
                for tt in range(NT // 128):
                    pmv = ps_mm.tile([128, 512], F32, tag="mmv", bufs=2,
                                     name="pmv")

                    def lhsv(k0, kn, tt=tt):
                        m = mod1T[:, k0:k0 + kn, tt * 128:(tt + 1) * 128]
                        return m if kn == 2 else mod1T[:, k0, tt * 128:(tt + 1) * 128]

                    def rhsv(k0, kn, c0=c0, c1=c1):
                        w = wv_8[:, k0:k0 + kn, 0:c1 - c0]
                        return w if kn == 2 else wv_8[:, k0, 0:c1 - c0]

                    mm_group(pmv[:, 0:c1 - c0], lhsv, rhsv, rhsv, c1 - c0)
                    for h in range(h0, h1):
                        nc.vector.tensor_scalar_mul(
                            v_aug[:, tt, h, 0:HD],
                            pmv[:, h * HD - c0:(h + 1) * HD - c0], WSI,
                        )

            # interleave q/k head-pairs and v slices so attention on early
            # heads can start while later chunks are still being produced
            for j in range(8):
                emit_qk_dh(0, j)
                emit_qk_dh(1, j)
                if j in (1, 3, 5):
                    emit_v_slice({1: 0, 3: 1, 5: 2}[j])
        es_mod1.close()
        es_qw.close()
        if phase_limit <= 2:
            es_qk.close()
            return _truncate_out(tc, nc, out_dram)

        # ============ phase 3: attention ====================================
        es_ao = ExitStack()
        pastk = es_ao.enter_context(tc.tile_pool(name="pastk", bufs=1))
        attn_st = pastk.tile([128, KT, NT], FP8, name="attn_st")

        with tc.tile_pool(name="pexp", bufs=2) as pexp, \
             tc.tile_pool(name="pattn", bufs=2) as pattn, \
             tc.tile_pool(name="ps_pj", bufs=2, space="PSUM") as ps_pj:
          with tc.tile_pool(name="ps_s", bufs=2, space="PSUM") as ps_s, \
               tc.tile_pool(name="ps_av", bufs=2, space="PSUM") as ps_av:
            for h in range(H):
                exp_h = pexp.tile([128, NT // 128, NT], FP8, tag="exp",
                                  name="exp_h")
                for kt_i in range(NT // 128):
                    pss2 = ps_s.tile([128, 1024], F32, tag="s", name="pss2")
                    for n in range(2):
                        nsl = slice(n * 512, (n + 1) * 512)
                        nc.tensor.matmul(
                            pss2[:, nsl],
                            qk_st[:, 16 + h, kt_i * 128:(kt_i + 1) * 128],
                            qk_st[:, h, nsl], start=True, stop=True,
                            skip_group_check=True,
                        )
                    nc.scalar.activation(
                        exp_h[:, kt_i, :], pss2[:, :], AF.Exp, scale=ISC
                    )
                attn_h = pattn.tile([72, NT], FP8, tag="attnh", name="attn_h")
                for n in range(2):
                    nsl = slice(n * 512, (n + 1) * 512)
                    pav = ps_av.tile([97, 512], F32, tag="av", name="pav")
                    for kk in range(4):
                        nc.tensor.matmul(
                            pav[:, :], v_aug[:, 2 * kk:2 * kk + 2, h, :],
                            exp_h[:, 2 * kk:2 * kk + 2, nsl],
                            start=(kk == 0), stop=(kk == 3), perf_mode=DR,
                        )
                    recip = pattn.tile([1, 512], F32, tag="recip", bufs=2,
                                       name="recip")
                    nc.vector.reciprocal(recip[:, :], pav[96:97, :])
                    bca = pattn.tile([72, 512], F32, tag="bca", name="bca")
                    nc.gpsimd.partition_broadcast(bca[:, :], recip[:, :])
                    atf = pattn.tile([72, 512], F32, tag="atf", bufs=2,
                                     name="atf")
                    nc.vector.tensor_mul(atf[:, :], pav[0:72, :], bca[:, :])
                    nc.vector.tensor_scalar_add(
                        attn_h[:, nsl], atf[:, :], bv_pp[:, h:h + 1]
                    )
                for (kt_i, p0, ln, off) in _head_segs(h * HD, HD):
                    nc.sync.dma_start(
                        attn_st[p0:p0 + ln, kt_i, :], attn_h[off:off + ln, :]
                    )
                with tc.tile_wait_until(0.150 + 0.006 * h):
                    for k2 in range((h * MH) // H, ((h + 1) * MH) // H):
                        emit_w2_convert(k2)

          if True:
            # proj chases head completion: its k-accumulation consumes
            # attn_st k-tiles as the covering heads finish
            for mo in range(KT):
                wp_t = pw_s.tile([128, KT, 128], F32, tag="ws", bufs=3,
                                 name="wp_t")
                nc.sync.dma_start(
                    wp_t[:, :, :],
                    ins["w_proj"][:, mo * 128:(mo + 1) * 128]
                    .rearrange("(k p) m -> p k m", p=128),
                )
                wp_8 = pw_s.tile([128, KT, 128], FP8, tag="wsb", bufs=3,
                                 name="wp_8")
                nc.gpsimd.tensor_scalar_mul(wp_8[:, :, :], wp_t[:, :, :],
                                            WS)
                for n in range(2):
                    nsl = slice(n * 512, (n + 1) * 512)
                    ps2p = ps_pj.tile([128, 512], F32, tag="mm2", bufs=2,
                                      name="ps2p")

                    def lhsp(k0, kn):
                        w = wp_8[:, k0:k0 + kn, :]
                        return w if kn == 2 else wp_8[:, k0, :]

                    def rhsp(k0, kn, nsl=nsl):
                        a = attn_st[:, k0:k0 + kn, nsl]
                        return a if kn == 2 else attn_st[:, k0, nsl]

                    mm_group(ps2p[:, :], lhsp, rhsp, rhsp, 512)
                    t_sb4 = pattn.tile([128, 512], F32, tag="tsb", bufs=3,
                                       name="t_sb4")
                    nc.scalar.activation(
                        t_sb4[:, :], ps2p[:, :], AF.Identity,
                        bias=bproj_pp[:, mo:mo + 1], scale=WSI,
                    )
                    nc.vector.scalar_tensor_tensor(
                        xT[:, mo, nsl], t_sb4[:, :],
                        ada_pp[:, 2, mo:mo + 1],
                        xT[:, mo, nsl], ALU.mult, ALU.add,
                    )
        es_qk.close()
        es_ao.close()
        if phase_limit <= 3:
            es_ao.close()
            return _truncate_out(tc, nc, out_dram)

        # ============ phase 4: proj + residual1 + LN2 ========================
        es_f1h = ExitStack()
        pf1h = es_f1h.enter_context(
            tc.tile_pool(name="pf1h", bufs=1, side="right"))
        F1H = 18   # fc1 m-tiles pre-converted during phase 4
        fc1_8 = pf1h.tile([128, KT, F1H * 128], FP8, name="fc1_8")
        es_mod2 = ExitStack()
        pmod2 = es_mod2.enter_context(
            tc.tile_pool(name="pmod2", bufs=1, side="right"))
        mod2T = pmod2.tile([128, KT, NT], FP8, name="mod2T")

        def emit_f1h_convert(p4f, mo):
            wfh_t = p4f.tile([128, KT, 128], F32, tag="wsf", bufs=6,
                             name="wfh_t")
            nc.sync.dma_start(
                wfh_t[:, :, :],
                ins["w_fc1"][:, mo * 128:(mo + 1) * 128]
                .rearrange("(k p) m -> p k m", p=128),
            )
            nc.scalar.activation(
                fc1_8[:, :, mo * 128:(mo + 1) * 128], wfh_t[:, :, :],
                AF.Identity, scale=WS)

        with tc.tile_pool(name="pst4", bufs=1) as pst4, \
             tc.tile_pool(name="pln4", bufs=1) as pln4, \
             tc.tile_pool(name="p4f", bufs=1) as p4f:
            with tc.tile_pool(name="ps_st2", bufs=4, space="PSUM") as ps_st2:
                _ln_mod(tc, nc, xT, mod2T, ada_pp, 3, 4, ones_col,
                        pst4, pln4, ps_st2)
            for mo in range(F1H):
                emit_f1h_convert(p4f, mo)
        if phase_limit <= 4:
            es_mod2.close()
            return _truncate_out(tc, nc, out_dram)

        # ============ phase 5: FFN + output =================================
        # half-granular pipeline: fc1 half-0 (prefetched weights) starts as
        # soon as mod2T half-0 exists; fc2 half-0 chases; streamed fc1
        # weights cover both halves between the two prefetched passes.
        with tc.tile_pool(name="p5w", bufs=1) as p5w, \
             tc.tile_pool(name="p5h", bufs=1) as p5h, \
             tc.tile_pool(name="ps_5", bufs=1, space="PSUM") as ps_5:
            hT_sb = p5h.tile([128, MH, NT], FP8, name="hT_sb")

            def fc1_group(mo, n, wf_8):
                nsl = slice(n * 512, (n + 1) * 512)
                pf1 = ps_5.tile([128, 512], F32, tag="f1", bufs=3,
                                name="pf1")

                def lhsf(k0, kn, mo=mo, wf_8=wf_8):
                    if wf_8 is None:
                        msl = slice(mo * 128, (mo + 1) * 128)
                        w = fc1_8[:, k0:k0 + kn, msl]
                        return w if kn == 2 else fc1_8[:, k0, msl]
                    w = wf_8[:, k0:k0 + kn, :]
                    return w if kn == 2 else wf_8[:, k0, :]

                def rhsf(k0, kn, nsl=nsl):
                    m = mod2T[:, k0:k0 + kn, nsl]
                    return m if kn == 2 else mod2T[:, k0, nsl]

                mm_group(pf1[:, :], lhsf, rhsf, rhsf, 512)
                nc.scalar.activation(
                    hT_sb[:, mo, nsl], pf1[:, :], AF.Gelu_apprx_tanh,
                    bias=bfc1_pp[:, mo:mo + 1], scale=WSI,
                )

            def fc2_m(m, n):
                nsl = slice(n * 512, (n + 1) * 512)
                pf2 = ps_5.tile([128, 512], F32, tag="f2", bufs=2,
                                name="pf2")
                for kk in range(MH // 2):
                    nc.tensor.matmul(
                        pf2[:, :],
                        w2_sb[:, 2 * kk:2 * kk + 2, m * 128:(m + 1) * 128],
                        hT_sb[:, 2 * kk:2 * kk + 2, nsl],
                        start=(kk == 0), stop=(kk == MH // 2 - 1),
                        perf_mode=DR,
                    )
                t2 = p5w.tile([128, 512], F32, tag="tsb", bufs=2, name="t2")
                nc.scalar.activation(
                    t2[:, :], pf2[:, :], AF.Identity,
                    bias=bfc2_pp[:, m:m + 1], scale=WSI,
                )
                nc.vector.scalar_tensor_tensor(
                    xT[:, m, nsl], t2[:, :],
                    ada_pp[:, 5, m:m + 1], xT[:, m, nsl],
                    ALU.mult, ALU.add,
                )

            # fc1 prefetched half-0, then streamed mo both halves, then
            # prefetched half-1 (fc2 half-0 can start during the latter)
            for mo in range(F1H):
                fc1_group(mo, 0, None)
            for mo in range(F1H, MH):
                wf_t = pw_s.tile([128, KT, 128], F32, tag="ws", bufs=3,
                                 name="wf_t")
                nc.sync.dma_start(
                    wf_t[:, :, :],
                    ins["w_fc1"][:, mo * 128:(mo + 1) * 128]
                    .rearrange("(k p) m -> p k m", p=128),
                )
                wf_8 = pw_s.tile([128, KT, 128], FP8, tag="wsb",
                                 bufs=3, name="wf_8")
                if mo % 2 == 0:
                    nc.gpsimd.tensor_scalar_mul(wf_8[:, :, :],
                                                wf_t[:, :, :], WS)
                else:
                    nc.vector.tensor_scalar_mul(wf_8[:, :, :],
                                                wf_t[:, :, :], WS)
                fc1_group(mo, 0, wf_8)
                fc1_group(mo, 1, wf_8)
            for mo in range(F1H):
                fc1_group(mo, 1, None)

            ot2 = {}
            for n in range(2):
                for m in range(KT):
                    fc2_m(m, n)
                    # transpose this half's token tiles; store [128,256]
                    # chunks once both m's of a pair are done
                    if m % 2 == 0:
                        ot2[n] = p5w.tile([128, 4, 256], F32, tag="ot2",
                                          bufs=2, name="ot2")
                    sl = slice(128 * (m % 2), 128 * (m % 2) + 128)
                    for tt in range(4 * n, 4 * n + 4):
                        pt = ps_5.tile([128, 128], F32, tag="tro", bufs=2,
                                       name="pt6")
                        nc.tensor.transpose(
                            pt[:, :], xT[:, m, tt * 128:(tt + 1) * 128],
                            ident[:, :],
                        )
                        nc.vector.tensor_copy(ot2[n][:, tt - 4 * n, sl],
                                              pt[:, :])
                        if m % 2 == 1 or m == KT - 1:
                            w = 128 * (m % 2) + 128
                            nc.sync.dma_start(
                                out_dram[tt * 128:(tt + 1) * 128,
                                         (m - m % 2) * 128:
                                         (m - m % 2) * 128 + w],
                                ot2[n][:, tt - 4 * n, 0:w],
                            )
        es_mod2.close()
        es_f1h.close()


_LOCK = threading.Lock()
_PROG = None


def _get_program():
    global _PROG
    with _LOCK:
        if _PROG is None:
            _PROG = _build_program()
    return _PROG


def _make_in_maps(inputs):
    arrs = {k: np.ascontiguousarray(np.asarray(v, dtype=np.float32))
            for k, v in inputs.items()}
    in_maps = []
    for c in range(NCORES):
        m = {k: v for k, v in arrs.items()
             if k not in ("x", "t_emb", "w_ada")}
        m["x"] = np.ascontiguousarray(arrs["x"][c])
        if ADA_SPLIT:
            m["t_emb"] = arrs["t_emb"]
            m["w_ada"] = np.ascontiguousarray(
                arrs["w_ada"][:, c * ADA_COLS:(c + 1) * ADA_COLS])
        else:
            m["t_emb"] = np.ascontiguousarray(arrs["t_emb"][c])
            m["w_ada"] = arrs["w_ada"]
        in_maps.append(m)
    return in_maps


def kernel(**inputs):
    nc = _get_program()
    res = run_bass_kernel_spmd(nc, _make_in_maps(inputs),
                               core_ids=list(range(NCORES)))
    return np.stack([r["out"] for r in res.results], axis=0)


def kernel_traced(inputs, **kw):
    """test-harness helper: returns full BassKernelResults with trace."""
    nc = _get_program()
    return run_bass_kernel_spmd(
        nc, _make_in_maps(inputs), core_ids=list(range(NCORES)), trace=True, **kw
    )


# revision 102
# speedup vs baseline: 1.7499x; 1.0007x over previous
"""DiT block kernel for Trainium2 (Bass/Tile), 8-core data parallel.

Shapes (hardcoded from the problem spec):
  x: (8, 1024, 1152), t_emb: (8, 1152)
  w_qkv (1152, 3456), w_proj (1152, 1152), w_fc1 (1152, 4608),
  w_fc2 (4608, 1152), w_ada (1152, 6912) + biases.

Strategy: batch-parallel across 8 cores (one batch element each).
Activations live transposed [D on partitions, tokens free]; projections are
out_T = W.T @ x_T with lhsT = W as stored.  All large GEMMs run in fp8e4
DoubleRow (two 128-deep k-tiles per PE pass); weights are scaled by 32 into
fp8 to stay in the normal range, compensated in the PSUM epilogue scale.
LayerNorm stats use f32r ones-column matmuls; softmax runs transposed with
no max subtraction and a ones-column appended to V for the denominators.
adaLN is contraction-split across the 8 cores (each core loads 1/8 of
w_ada, computes partial rows for all batches, then one small AllToAll
redistributes) -- gated by BASS_ADA_SPLIT.
"""

import os
import threading
from contextlib import ExitStack

import numpy as np

import concourse.bass as bass
import concourse.mybir as mybir
import concourse.tile as tile
from concourse import bacc
from concourse.bass_utils import run_bass_kernel_spmd
from concourse.masks import make_identity

F32 = mybir.dt.float32
F32R = mybir.dt.float32r
BF16 = mybir.dt.bfloat16
FP8 = mybir.dt.float8e4
AF = mybir.ActivationFunctionType
ALU = mybir.AluOpType
DR = mybir.MatmulPerfMode.DoubleRow

NCORES = 8
D = 1152
NT = 1024          # tokens per core (batch element)
KT = D // 128      # 9 partition-tiles of D
H = 16
HD = 72
HID = 4 * D        # 4608
MH = HID // 128    # 36
EPS = 1e-6
ISC = 1.0 / float(np.sqrt(HD))
WS = 32.0          # weight upscale into fp8e4 (avoids subnormals)
WSI = 1.0 / WS

ADA_SPLIT = os.environ.get("BASS_ADA_SPLIT", "1") == "1"
ADA_COLS = 6 * D // NCORES   # 864 columns of w_ada per core when split

# v output column slices aligned to head boundaries
V_SLICES = [(0, 432, 0, 6), (432, 864, 6, 12), (864, 1152, 12, 16)]


def _r(ap):
    return ap.bitcast(F32R)


def _head_segs(d0, n):
    """Split logical rows [d0, d0+n) of a [*,128]-tiled stacked tensor into
    (ktile, part0, length, dst_offset) segments within 128-partition tiles."""
    segs = []
    off = 0
    while n > 0:
        kt_i, p0 = divmod(d0, 128)
        ln = min(n, 128 - p0)
        segs.append((kt_i, p0, ln, off))
        d0 += ln
        off += ln
        n -= ln
    return segs


def _build_program():
    nc = bacc.Bacc(
        "TRN2", target_bir_lowering=False, debug=False, enable_asserts=False,
        num_devices=NCORES,
    )
    ins = {}
    ins["x"] = nc.dram_tensor("x", [NT, D], F32, kind="ExternalInput").ap()
    if ADA_SPLIT:
        ins["t_emb"] = nc.dram_tensor(
            "t_emb", [NCORES, D], F32, kind="ExternalInput").ap()
        ins["w_ada"] = nc.dram_tensor(
            "w_ada", [D, ADA_COLS], F32, kind="ExternalInput").ap()
    else:
        ins["t_emb"] = nc.dram_tensor(
            "t_emb", [D], F32, kind="ExternalInput").ap()
        ins["w_ada"] = nc.dram_tensor(
            "w_ada", [D, 6 * D], F32, kind="ExternalInput").ap()
    for name, shape in [
        ("w_qkv", [D, 3 * D]), ("b_qkv", [3 * D]),
        ("w_proj", [D, D]), ("b_proj", [D]),
        ("w_fc1", [D, HID]), ("b_fc1", [HID]),
        ("w_fc2", [HID, D]), ("b_fc2", [D]),
        ("b_ada", [6 * D]),
    ]:
        ins[name] = nc.dram_tensor(name, shape, F32, kind="ExternalInput").ap()
    out_dram = nc.dram_tensor("out", [NT, D], F32, kind="ExternalOutput").ap()

    with tile.TileContext(nc) as tc:
        _body(tc, ins, out_dram)
    nc.compile()
    return nc


def _ln_mod(tc, nc, src, dst, ada_pp, shift_c, scale_c, ones_col,
            pst, pln, ps_st, mul_pool=False):
    """dst[:,k,:] = fp8((src-mean)*rstd * ada[scale_c] + ada[shift_c]).
    Stats over the partition (D) axis per token via f32r ones matmuls."""
    ps_x, ps_q, st = {}, {}, {}
    for n in range(2):
        nsl = slice(n * 512, (n + 1) * 512)
        ps_x[n] = ps_st.tile([1, 512], F32, tag="st", name=f"psx{n}")
        ps_q[n] = ps_st.tile([1, 512], F32, tag="st", name=f"psq{n}")
        for k in range(KT):
            xb = pln.tile([128, 512], BF16, tag="xb", bufs=2, name="xb")
            nc.scalar.copy(xb[:, :], src[:, k, nsl])
            sq = pln.tile([128, 512], BF16, tag="sqb", bufs=2, name="sq")
            nc.vector.tensor_mul(sq[:, :], xb[:, :], xb[:, :])
            nc.tensor.matmul(
                ps_x[n][:, :], ones_col[:, :], xb[:, :],
                start=(k == 0), stop=(k == KT - 1), skip_group_check=True,
            )
            nc.tensor.matmul(
                ps_q[n][:, :], ones_col[:, :], sq[:, :],
                start=(k == 0), stop=(k == KT - 1), skip_group_check=True,
            )
    eps_sb = pst.tile([1, 1], F32, tag="eps", bufs=1, name="eps_sb")
    nc.vector.memset(eps_sb[:, :], EPS)
    for n in range(2):
        # rows: 0 = mean, 1 = E[x^2] -> rstd
        st[n] = pst.tile([1, 2, 512], F32, tag="lnst", bufs=2, name=f"st{n}")
        nc.vector.tensor_scalar_mul(st[n][:, 0, :], ps_x[n][:, :], 1.0 / D)
        work = pst.tile([1, 512], F32, tag="lnwork", bufs=2, name="work")
        nc.vector.tensor_mul(work[:, :], st[n][:, 0, :], st[n][:, 0, :])
        nc.vector.scalar_tensor_tensor(
            st[n][:, 1, :], ps_q[n][:, :], 1.0 / D, work[:, :],
            ALU.mult, ALU.subtract,
        )
        nc.scalar.activation(st[n][:, 1, :], st[n][:, 1, :], AF.Sqrt,
                             bias=eps_sb[:, :], scale=1.0)
        nc.vector.reciprocal(st[n][:, 1, :], st[n][:, 1, :])
    for n in range(2):
        nsl = slice(n * 512, (n + 1) * 512)
        meanB = pln.tile([128, 512], F32, tag="meanB", bufs=2, name="meanB")
        rstdB = pln.tile([128, 512], F32, tag="rstdB", bufs=2, name="rstdB")
        nc.gpsimd.partition_broadcast(meanB[:, :], st[n][:, 0, :])
        nc.gpsimd.partition_broadcast(rstdB[:, :], st[n][:, 1, :])
        for k in range(KT):
            tmp = pln.tile([128, 512], F32, tag="lnt", bufs=4, name="tmp")
            nc.vector.tensor_sub(tmp[:, :], src[:, k, nsl], meanB[:, :])
            meng = nc.gpsimd if (mul_pool and k % 2 == 0) else nc.vector
            meng.tensor_mul(tmp[:, :], tmp[:, :], rstdB[:, :])
            nc.gpsimd.tensor_scalar(
                dst[:, k, nsl], tmp[:, :],
                ada_pp[:, scale_c, k:k + 1], ada_pp[:, shift_c, k:k + 1],
                ALU.mult, ALU.add,
            )


def _truncate_out(tc, nc, out_dram):
    with tc.tile_pool(name="ptrunc", bufs=1) as p:
        z = p.tile([128, D], F32, name="z")
        nc.vector.memset(z[:, :], 0.0)
        for tt in range(NT // 128):
            nc.sync.dma_start(out_dram[tt * 128:(tt + 1) * 128, :], z[:, :])


def _body(tc, ins, out_dram):
    nc = tc.nc
    phase_limit = float(os.environ.get("BASS_PHASES", "6"))
    ctx = ExitStack()
    with ctx:
        dram = ctx.enter_context(tc.tile_pool(name="dram", bufs=1, space="DRAM"))
        if ADA_SPLIT:
            ada_part_dr = dram.tile([NCORES * ADA_COLS], F32)
            ada_gath_dr = dram.tile([NCORES * ADA_COLS], F32)
        else:
            ada_dr = dram.tile([6 * D], F32)

        pers = ctx.enter_context(tc.tile_pool(name="pers", bufs=1))
        ident = pers.tile([128, 128], F32)
        make_identity(nc, ident[:, :])
        ones_col = pers.tile([128, 1], BF16)
        nc.vector.memset(ones_col[:, :], 1.0)

        # silu(t_emb): split case holds all 8 batches, else just our own
        NB = NCORES if ADA_SPLIT else 1
        t_pp = pers.tile([128, NB, KT], F32)
        if ADA_SPLIT:
            nc.sync.dma_start(
                t_pp[:, :, :],
                ins["t_emb"].rearrange("b (k p) -> p b k", p=128))
        else:
            nc.sync.dma_start(
                t_pp[:, 0, :], ins["t_emb"].rearrange("(k p) -> p k", p=128))
        t_sb = pers.tile([128, NB, KT], BF16)
        nc.scalar.activation(t_sb[:, :, :], t_pp[:, :, :], AF.Silu)
        # preload activation tables so first exp/gelu hit no lazy table load
        warm = pers.tile([1, 2], F32)
        nc.scalar.activation(warm[:, 0:1], t_pp[0:1, 0, 0:1], AF.Exp)
        nc.scalar.activation(warm[:, 1:2], t_pp[0:1, 0, 0:1],
                             AF.Gelu_apprx_tanh)

        bqk72 = pers.tile([72, 32], F32)      # q,k bias per 72-head chunk
        bv_pp = pers.tile([72, H], F32)
        bproj_pp = pers.tile([128, KT], F32)
        bfc1_pp = pers.tile([128, MH], F32)
        bfc2_pp = pers.tile([128, KT], F32)
        bada_pp = pers.tile([128, 6, KT], F32)
        ada_pp = pers.tile([128, 6, KT], F32)

        def emit_bias_loads():
            nc.sync.dma_start(
                bqk72[:, :],
                ins["b_qkv"][0:2 * D].rearrange("(c p) -> p c", p=72))
            nc.sync.dma_start(
                bv_pp[:, :],
                ins["b_qkv"][2 * D:3 * D].rearrange("(h p) -> p h", p=72))
            nc.sync.dma_start(
                bproj_pp[:, :], ins["b_proj"].rearrange("(m p) -> p m", p=128))
            nc.sync.dma_start(
                bfc1_pp[:, :], ins["b_fc1"].rearrange("(m p) -> p m", p=128))
            nc.sync.dma_start(
                bfc2_pp[:, :], ins["b_fc2"].rearrange("(m p) -> p m", p=128))
            nc.sync.dma_start(
                bada_pp[:, :, :],
                ins["b_ada"].rearrange("(c k p) -> p c k", k=KT, p=128))

        xT = pers.tile([128, KT, NT], F32)   # becomes x2T, then out_T
        w2_sb = pers.tile([128, MH, D], FP8)  # fc2 weights, fp8*WS
        # weight-stream pool spanning phases 4-5 (proj/fc1 prefetch)
        pw_s = ctx.enter_context(tc.tile_pool(name="pw_s", bufs=1))
        # attn output store: allocated early so attention-phase pools do not
        # sit in freed qkv space (space-reuse would serialize the phases)
        es_ao = ExitStack()
        pastk = es_ao.enter_context(tc.tile_pool(name="pastk", bufs=1))
        attn_st = pastk.tile([128, KT, NT], FP8, name="attn_st")
        # qkv weight pool: lives through attention (closed at phase 4)
        es_qw = ExitStack()
        pqw = es_qw.enter_context(tc.tile_pool(name="pqw", bufs=1))

        # ============ phase 1: ada, x load+transpose, LN1 ====================
        es_mod1 = ExitStack()
        pmod1 = es_mod1.enter_context(tc.tile_pool(name="pmod1", bufs=1))
        mod1T = pmod1.tile([128, KT, NT], FP8, name="mod1T")

        with tc.tile_pool(name="p1w", bufs=1) as p1w, \
             tc.tile_pool(name="pst", bufs=1) as pst, \
             tc.tile_pool(name="pln", bufs=1) as pln:
            with tc.tile_pool(name="ps_pro", bufs=2, space="PSUM") as ps_pro, \
                 tc.tile_pool(name="pxin", bufs=3) as pxin, \
                 tc.tile_pool(name="ps_tr", bufs=3, space="PSUM") as ps_tr:

                def emit_transpose_block(tt):
                    xin = pxin.tile([128, D], F32, tag="xin", bufs=3,
                                    name="xin")
                    nc.sync.dma_start(
                        xin[:, :], ins["x"][tt * 128:(tt + 1) * 128, :])
                    for kd in range(KT):
                        pt = ps_tr.tile([128, 128], F32, tag="ptr", name="pt")
                        nc.tensor.transpose(
                            pt[:, :], xin[:, kd * 128:(kd + 1) * 128],
                            ident[:, :],
                        )
                        tsl = slice(tt * 128, (tt + 1) * 128)
                        if kd % 2 == 0:
                            nc.vector.tensor_copy(xT[:, kd, tsl], pt[:, :])
                        else:
                            nc.scalar.copy(xT[:, kd, tsl], pt[:, :])

                def emit_ada_split():
                    # partial rows for ALL batches over our 1/8 of w_ada,
                    # then AllToAll redistributes so each core gets its row.
                    ada_sb = pst.tile([NCORES, ADA_COLS], F32, tag="adasb",
                                      bufs=1, name="ada_sb")
                    for c in range(2):
                        csl = slice(c * 432, (c + 1) * 432)
                        wada_t = p1w.tile([128, KT, 432], F32, tag="wada",
                                          bufs=1, name="wada_t")
                        nc.sync.dma_start(
                            wada_t[:, :, :],
                            ins["w_ada"][:, csl]
                            .rearrange("(k p) m -> p k m", p=128),
                        )
                        wada_b = p1w.tile([128, KT, 432], BF16, tag="wadab",
                                          bufs=1, name="wada_b")
                        nc.gpsimd.tensor_copy(wada_b[:, :, :], wada_t[:, :, :])
                        pa = ps_pro.tile([NCORES, 432], F32, tag="psada",
                                         bufs=2, name="pa")
                        for k in range(KT):
                            nc.tensor.matmul(
                                pa[:, :], t_sb[:, :, k], wada_b[:, k, :],
                                start=(k == 0), stop=(k == KT - 1),
                            )
                        nc.vector.tensor_copy(ada_sb[:, csl], pa[:, :])
                    nc.sync.dma_start(
                        ada_part_dr.opt().rearrange("(b m) -> b m", b=NCORES),
                        ada_sb[:, :])
                    nc.gpsimd.collective_compute(
                        "AllToAll", ALU.bypass,
                        replica_groups=[list(range(NCORES))],
                        ins=[ada_part_dr.opt()],
                        outs=[ada_gath_dr.opt()],
                    )
                    nc.sync.dma_start(
                        ada_pp[:, :, :],
                        ada_gath_dr.opt().rearrange(
                            "(c k p) -> p c k", c=6, k=KT, p=128),
                    )

                def emit_ada_chunk(n):
                    # fallback: full w_ada on-core, M=1 bf16 matmuls
                    pa = ps_pro.tile([1, 384], F32, tag="psada", name="pa")
                    for k in range(KT):
                        wada_t = p1w.tile([128, 384], F32, tag="wsk", bufs=4,
                                          name="wada_t")
                        nc.sync.dma_start(
                            wada_t[:, :],
                            ins["w_ada"][k * 128:(k + 1) * 128,
                                         n * 384:(n + 1) * 384],
                        )
                        wada_b = p1w.tile([128, 384], BF16, tag="wskb",
                                          bufs=4, name="wada_b")
                        nc.gpsimd.tensor_copy(wada_b[:, :], wada_t[:, :])
                        nc.tensor.matmul(
                            pa[:, :], t_sb[:, 0, k:k + 1], wada_b[:, :],
                            start=(k == 0), stop=(k == KT - 1),
                        )
                    asb = pst.tile([1, 384], F32, tag="asb", bufs=2, name="asb")
                    nc.vector.tensor_copy(asb[:, :], pa[:, :])
                    nc.sync.dma_start(
                        ada_dr[n * 384:(n + 1) * 384]
                        .rearrange("(a b) -> a b", a=1),
                        asb[0:1, :],
                    )

                emit_bias_loads()
                if ADA_SPLIT:
                    emit_ada_split()
                    for i in range(8):
                        emit_transpose_block(i)
                else:
                    for i in range(8):
                        emit_transpose_block(i)
                        if i < 8:
                            emit_ada_chunk(2 * i)
                            emit_ada_chunk(2 * i + 1)
                    for n in range(16, 18):
                        emit_ada_chunk(n)
                    nc.sync.dma_start(
                        ada_pp[:, :, :],
                        ada_dr.opt().rearrange(
                            "(c k p) -> p c k", c=6, k=KT, p=128),
                    )
                nc.vector.tensor_add(ada_pp[:, :, :], ada_pp[:, :, :],
                                     bada_pp[:, :, :])
                nc.vector.tensor_scalar_add(ada_pp[:, 1, :], ada_pp[:, 1, :],
                                            1.0)
                nc.vector.tensor_scalar_add(ada_pp[:, 4, :], ada_pp[:, 4, :],
                                            1.0)

            if phase_limit > 0.6:
                with tc.tile_pool(name="ps_st", bufs=4, space="PSUM") as ps_st:
                    _ln_mod(tc, nc, xT, mod1T, ada_pp, 0, 1, ones_col,
                            pst, pln, ps_st)

        if phase_limit <= 1:
            es_mod1.close()
            return _truncate_out(tc, nc, out_dram)

        # ============ phase 2: qkv ==========================================
        es_qk = ExitStack()
        pqks = es_qk.enter_context(tc.tile_pool(name="pqks", bufs=1, side="right"))
        # chunks 0..15 = q heads, 16..31 = k heads; fp8 true values
        qk_st = pqks.tile([72, 32, NT], FP8, name="qk_st")
        pvaug = es_qk.enter_context(
            tc.tile_pool(name="pvaug", bufs=1, side="right"))
        # per head: cols 0..72 = v (true values), col 96 = ones, 72..96 zero
        v_aug = pvaug.tile([128, NT // 128, H, 97], FP8, name="v_aug")
        nc.gpsimd.memset(v_aug[:, :, :, HD:97], 0.0)
        nc.gpsimd.memset(v_aug[:, :, :, 96:97], 1.0)

        def emit_w2_convert(k):
            w2src = pw_s.tile([128, D], F32, tag="w2src", bufs=2,
                              name="w2src")
            nc.sync.dma_start(
                w2src[:, :], ins["w_fc2"][k * 128:(k + 1) * 128, :]
            )
            nc.gpsimd.tensor_scalar_mul(w2_sb[:, k, :], w2src[:, :], WS)

        def mm_group(psl, lhs8, rhs8, rhs_k_of, N):
            """accumulate 9 k-tiles: 4 DoubleRow pairs + 1 plain fp8."""
            for kk in range(4):
                nc.tensor.matmul(
                    psl, lhs8(2 * kk, 2), rhs_k_of(2 * kk, 2),
                    start=(kk == 0), stop=False, perf_mode=DR,
                )
            nc.tensor.matmul(psl, lhs8(8, 1), rhs_k_of(8, 1),
                             start=False, stop=True)

        with tc.tile_pool(name="p2w", bufs=1) as p2w, \
             tc.tile_pool(name="ps_mm", bufs=4, space="PSUM") as ps_mm:

            def emit_qk_dh(sec, j):
                # sec 0 = q (w cols j*144), sec 1 = k (w cols 1152 + j*144)
                wq_t = pqw.tile([128, KT, 144], F32, tag="wsq", bufs=2,
                                name="wq_t")
                nc.sync.dma_start(
                    wq_t[:, :, :],
                    ins["w_qkv"][:, sec * D + j * 144:sec * D + (j + 1) * 144]
                    .rearrange("(k p) m -> p k m", p=128),
                )
                wq_8 = pqw.tile([128, KT, 144], FP8, tag="wsbq", bufs=2,
                                name="wq_8")
                nc.gpsimd.tensor_scalar_mul(wq_8[:, :, :], wq_t[:, :, :], WS)
                for i in range(2):
                    ch = 16 * sec + 2 * j + i
                    hsl = slice(72 * i, 72 * i + 72)
                    ps2 = ps_mm.tile([72, 1024], F32, tag="mm", bufs=2,
                                     name="ps2")
                    for n in range(2):
                        nsl = slice(n * 512, (n + 1) * 512)

                        def lhs8(k0, kn, hsl=hsl):
                            w = wq_8[:, k0:k0 + kn, hsl]
                            return w if kn == 2 else wq_8[:, k0, hsl]

                        def rhs8(k0, kn, nsl=nsl):
                            m = mod1T[:, k0:k0 + kn, nsl]
                            return m if kn == 2 else mod1T[:, k0, nsl]

                        mm_group(ps2[:, nsl], lhs8, rhs8, rhs8, 512)
                    nc.scalar.activation(
                        qk_st[:, ch, :], ps2[:, :], AF.Identity,
                        bias=bqk72[:, ch:ch + 1], scale=WSI,
                    )

            def emit_v_slice(si):
                (c0, c1, h0, h1) = V_SLICES[si]
                wv_t = p2w.tile([128, KT, 432], F32, tag="wv", bufs=1,
                                name="wv_t")
                nc.sync.dma_start(
                    wv_t[:, :, 0:c1 - c0],
                    ins["w_qkv"][:, 2 * D + c0:2 * D + c1]
                    .rearrange("(k p) m -> p k m", p=128),
                )
                wv_8 = p2w.tile([128, KT, 432], FP8, tag="wvb", bufs=2,
                                name="wv_8")
                nc.vector.tensor_scalar_mul(wv_8[:, :, 0:c1 - c0],
                                            wv_t[:, :, 0:c1 - c0], WS)
                for tt in range(NT // 128):
                    pmv = ps_mm.tile([128, 512], F32, tag="mmv", bufs=2,
                                     name="pmv")

                    def lhsv(k0, kn, tt=tt):
                        m = mod1T[:, k0:k0 + kn, tt * 128:(tt + 1) * 128]
                        return m if kn == 2 else mod1T[:, k0, tt * 128:(tt + 1) * 128]

                    def rhsv(k0, kn, c0=c0, c1=c1):
                        w = wv_8[:, k0:k0 + kn, 0:c1 - c0]
                        return w if kn == 2 else wv_8[:, k0, 0:c1 - c0]

                    mm_group(pmv[:, 0:c1 - c0], lhsv, rhsv, rhsv, c1 - c0)
                    for h in range(h0, h1):
                        nc.vector.tensor_scalar_mul(
                            v_aug[:, tt, h, 0:HD],
                            pmv[:, h * HD - c0:(h + 1) * HD - c0], WSI,
                        )

            # interleave q/k head-pairs and v slices so attention on early
            # heads can start while later chunks are still being produced
            for j in range(8):
                emit_qk_dh(0, j)
                emit_qk_dh(1, j)
                if j in (1, 3, 5):
                    emit_v_slice({1: 0, 3: 1, 5: 2}[j])
        es_mod1.close()
        es_qw.close()
        if phase_limit <= 2:
            es_qk.close()
            return _truncate_out(tc, nc, out_dram)

        # ============ phase 3: attention ====================================
        es_ao = ExitStack()
        pastk = es_ao.enter_context(tc.tile_pool(name="pastk", bufs=1))
        attn_st = pastk.tile([128, KT, NT], FP8, name="attn_st")

        with tc.tile_pool(name="pexp", bufs=2) as pexp, \
             tc.tile_pool(name="pattn", bufs=2) as pattn, \
             tc.tile_pool(name="ps_pj", bufs=2, space="PSUM") as ps_pj:
          with tc.tile_pool(name="ps_s", bufs=2, space="PSUM") as ps_s, \
               tc.tile_pool(name="ps_av", bufs=2, space="PSUM") as ps_av:
            for h in range(H):
                exp_h = pexp.tile([128, NT // 128, NT], FP8, tag="exp",
                                  name="exp_h")
                for kt_i in range(NT // 128):
                    pss2 = ps_s.tile([128, 1024], F32, tag="s", name="pss2")
                    for n in range(2):
                        nsl = slice(n * 512, (n + 1) * 512)
                        nc.tensor.matmul(
                            pss2[:, nsl],
                            qk_st[:, 16 + h, kt_i * 128:(kt_i + 1) * 128],
                            qk_st[:, h, nsl], start=True, stop=True,
                            skip_group_check=True,
                        )
                    nc.scalar.activation(
                        exp_h[:, kt_i, :], pss2[:, :], AF.Exp, scale=ISC
                    )
                attn_h = pattn.tile([72, NT], FP8, tag="attnh", name="attn_h")
                for n in range(2):
                    nsl = slice(n * 512, (n + 1) * 512)
                    pav = ps_av.tile([97, 512], F32, tag="av", name="pav")
                    for kk in range(4):
                        nc.tensor.matmul(
                            pav[:, :], v_aug[:, 2 * kk:2 * kk + 2, h, :],
                            exp_h[:, 2 * kk:2 * kk + 2, nsl],
                            start=(kk == 0), stop=(kk == 3), perf_mode=DR,
                        )
                    recip = pattn.tile([1, 512], F32, tag="recip", bufs=2,
                                       name="recip")
                    nc.vector.reciprocal(recip[:, :], pav[96:97, :])
                    bca = pattn.tile([72, 512], F32, tag="bca", name="bca")
                    nc.gpsimd.partition_broadcast(bca[:, :], recip[:, :])
                    atf = pattn.tile([72, 512], F32, tag="atf", bufs=2,
                                     name="atf")
                    nc.vector.tensor_mul(atf[:, :], pav[0:72, :], bca[:, :])
                    nc.vector.tensor_scalar_add(
                        attn_h[:, nsl], atf[:, :], bv_pp[:, h:h + 1]
                    )
                for (kt_i, p0, ln, off) in _head_segs(h * HD, HD):
                    nc.sync.dma_start(
                        attn_st[p0:p0 + ln, kt_i, :], attn_h[off:off + ln, :]
                    )
                with tc.tile_wait_until(0.150 + 0.006 * h):
                    for k2 in range((h * MH) // H, ((h + 1) * MH) // H):
                        emit_w2_convert(k2)

          if True:
            # proj chases head completion: its k-accumulation consumes
            # attn_st k-tiles as the covering heads finish
            for mo in range(KT):
                wp_t = pw_s.tile([128, KT, 128], F32, tag="ws", bufs=3,
                                 name="wp_t")
                nc.sync.dma_start(
                    wp_t[:, :, :],
                    ins["w_proj"][:, mo * 128:(mo + 1) * 128]
                    .rearrange("(k p) m -> p k m", p=128),
                )
                wp_8 = pw_s.tile([128, KT, 128], FP8, tag="wsb", bufs=3,
                                 name="wp_8")
                nc.gpsimd.tensor_scalar_mul(wp_8[:, :, :], wp_t[:, :, :],
                                            WS)
                for n in range(2):
                    nsl = slice(n * 512, (n + 1) * 512)
                    ps2p = ps_pj.tile([128, 512], F32, tag="mm2", bufs=2,
                                      name="ps2p")

                    def lhsp(k0, kn):
                        w = wp_8[:, k0:k0 + kn, :]
                        return w if kn == 2 else wp_8[:, k0, :]

                    def rhsp(k0, kn, nsl=nsl):
                        a = attn_st[:, k0:k0 + kn, nsl]
                        return a if kn == 2 else attn_st[:, k0, nsl]

                    mm_group(ps2p[:, :], lhsp, rhsp, rhsp, 512)
                    t_sb4 = pattn.tile([128, 512], F32, tag="tsb", bufs=3,
                                       name="t_sb4")
                    nc.scalar.activation(
                        t_sb4[:, :], ps2p[:, :], AF.Identity,
                        bias=bproj_pp[:, mo:mo + 1], scale=WSI,
                    )
                    nc.vector.scalar_tensor_tensor(
                        xT[:, mo, nsl], t_sb4[:, :],
                        ada_pp[:, 2, mo:mo + 1],
                        xT[:, mo, nsl], ALU.mult, ALU.add,
                    )
        es_qk.close()
        es_ao.close()
        if phase_limit <= 3:
            es_ao.close()
            return _truncate_out(tc, nc, out_dram)

        # ============ phase 4: proj + residual1 + LN2 ========================
        es_f1h = ExitStack()
        pf1h = es_f1h.enter_context(
            tc.tile_pool(name="pf1h", bufs=1, side="right"))
        F1H = 18   # fc1 m-tiles pre-converted during phase 4
        fc1_8 = pf1h.tile([128, KT, F1H * 128], FP8, name="fc1_8")
        es_mod2 = ExitStack()
        pmod2 = es_mod2.enter_context(
            tc.tile_pool(name="pmod2", bufs=1, side="right"))
        mod2T = pmod2.tile([128, KT, NT], FP8, name="mod2T")

        def emit_f1h_convert(p4f, mo):
            wfh_t = p4f.tile([128, KT, 128], F32, tag="wsf", bufs=6,
                             name="wfh_t")
            nc.sync.dma_start(
                wfh_t[:, :, :],
                ins["w_fc1"][:, mo * 128:(mo + 1) * 128]
                .rearrange("(k p) m -> p k m", p=128),
            )
            nc.scalar.activation(
                fc1_8[:, :, mo * 128:(mo + 1) * 128], wfh_t[:, :, :],
                AF.Identity, scale=WS)

        with tc.tile_pool(name="pst4", bufs=1) as pst4, \
             tc.tile_pool(name="pln4", bufs=1) as pln4, \
             tc.tile_pool(name="p4f", bufs=1) as p4f:
            with tc.tile_pool(name="ps_st2", bufs=4, space="PSUM") as ps_st2:
                _ln_mod(tc, nc, xT, mod2T, ada_pp, 3, 4, ones_col,
                        pst4, pln4, ps_st2)
            for mo in range(F1H):
                emit_f1h_convert(p4f, mo)
        if phase_limit <= 4:
            es_mod2.close()
            return _truncate_out(tc, nc, out_dram)

        # ============ phase 5: FFN + output =================================
        # half-granular pipeline: fc1 half-0 (prefetched weights) starts as
        # soon as mod2T half-0 exists; fc2 half-0 chases; streamed fc1
        # weights cover both halves between the two prefetched passes.
        with tc.tile_pool(name="p5w", bufs=1) as p5w, \
             tc.tile_pool(name="p5h", bufs=1) as p5h, \
             tc.tile_pool(name="ps_5", bufs=1, space="PSUM") as ps_5:
            hT_sb = p5h.tile([128, MH, NT], FP8, name="hT_sb")

            def fc1_group(mo, n, wf_8):
                nsl = slice(n * 512, (n + 1) * 512)
                pf1 = ps_5.tile([128, 512], F32, tag="f1", bufs=3,
                                name="pf1")

                def lhsf(k0, kn, mo=mo, wf_8=wf_8):
                    if wf_8 is None:
                        msl = slice(mo * 128, (mo + 1) * 128)
                        w = fc1_8[:, k0:k0 + kn, msl]
                        return w if kn == 2 else fc1_8[:, k0, msl]
                    w = wf_8[:, k0:k0 + kn, :]
                    return w if kn == 2 else wf_8[:, k0, :]

                def rhsf(k0, kn, nsl=nsl):
                    m = mod2T[:, k0:k0 + kn, nsl]
                    return m if kn == 2 else mod2T[:, k0, nsl]

                mm_group(pf1[:, :], lhsf, rhsf, rhsf, 512)
                nc.scalar.activation(
                    hT_sb[:, mo, nsl], pf1[:, :], AF.Gelu_apprx_tanh,
                    bias=bfc1_pp[:, mo:mo + 1], scale=WSI,
                )

            def fc2_m(m, n):
                nsl = slice(n * 512, (n + 1) * 512)
                pf2 = ps_5.tile([128, 512], F32, tag="f2", bufs=2,
                                name="pf2")
                for kk in range(MH // 2):
                    nc.tensor.matmul(
                        pf2[:, :],
                        w2_sb[:, 2 * kk:2 * kk + 2, m * 128:(m + 1) * 128],
                        hT_sb[:, 2 * kk:2 * kk + 2, nsl],
                        start=(kk == 0), stop=(kk == MH // 2 - 1),
                        perf_mode=DR,
                    )
                t2 = p5w.tile([128, 512], F32, tag="tsb", bufs=2, name="t2")
                nc.scalar.activation(
                    t2[:, :], pf2[:, :], AF.Identity,
                    bias=bfc2_pp[:, m:m + 1], scale=WSI,
                )
                nc.vector.scalar_tensor_tensor(
                    xT[:, m, nsl], t2[:, :],
                    ada_pp[:, 5, m:m + 1], xT[:, m, nsl],
                    ALU.mult, ALU.add,
                )

            # fc1 prefetched half-0, then streamed mo both halves, then
            # prefetched half-1 (fc2 half-0 can start during the latter)
            for mo in range(F1H):
                fc1_group(mo, 0, None)
            for mo in range(F1H, MH):
                wf_t = pw_s.tile([128, KT, 128], F32, tag="ws", bufs=3,
                                 name="wf_t")
                nc.sync.dma_start(
                    wf_t[:, :, :],
                    ins["w_fc1"][:, mo * 128:(mo + 1) * 128]
                    .rearrange("(k p) m -> p k m", p=128),
                )
                wf_8 = pw_s.tile([128, KT, 128], FP8, tag="wsb",
                                 bufs=3, name="wf_8")
                if mo % 2 == 0:
                    nc.gpsimd.tensor_scalar_mul(wf_8[:, :, :],
                                                wf_t[:, :, :], WS)
                else:
                    nc.vector.tensor_scalar_mul(wf_8[:, :, :],
                                                wf_t[:, :, :], WS)
                fc1_group(mo, 0, wf_8)
                fc1_group(mo, 1, wf_8)
            for mo in range(F1H):
                fc1_group(mo, 1, None)

            ot2 = {}
            for n in range(2):
                for m in range(KT):
                    fc2_m(m, n)
                    # transpose this half's token tiles; store [128,256]
                    # chunks once both m's of a pair are done
                    if m % 2 == 0:
                        ot2[n] = p5w.tile([128, 4, 256], F32, tag="ot2",
                                          bufs=2, name="ot2")
                    sl = slice(128 * (m % 2), 128 * (m % 2) + 128)
                    for tt in range(4 * n, 4 * n + 4):
                        pt = ps_5.tile([128, 128], F32, tag="tro", bufs=2,
                                       name="pt6")
                        nc.tensor.transpose(
                            pt[:, :], xT[:, m, tt * 128:(tt + 1) * 128],
                            ident[:, :],
                        )
                        nc.vector.tensor_copy(ot2[n][:, tt - 4 * n, sl],
                                              pt[:, :])
                        if m % 2 == 1 or m == KT - 1:
                            w = 128 * (m % 2) + 128
                            nc.sync.dma_start(
                                out_dram[tt * 128:(tt + 1) * 128,
                                         (m - m % 2) * 128:
                                         (m - m % 2) * 128 + w],
                                ot2[n][:, tt - 4 * n, 0:w],
                            )
        es_mod2.close()
        es_f1h.close()


_LOCK = threading.Lock()
_PROG = None


def _get_program():
    global _PROG
    with _LOCK:
        if _PROG is None:
            _PROG = _build_program()
    return _PROG


def _make_in_maps(inputs):
    arrs = {k: np.ascontiguousarray(np.asarray(v, dtype=np.float32))
            for k, v in inputs.items()}
    in_maps = []
    for c in range(NCORES):
        m = {k: v for k, v in arrs.items()
             if k not in ("x", "t_emb", "w_ada")}
        m["x"] = np.ascontiguousarray(arrs["x"][c])
        if ADA_SPLIT:
            m["t_emb"] = arrs["t_emb"]
            m["w_ada"] = np.ascontiguousarray(
                arrs["w_ada"][:, c * ADA_COLS:(c + 1) * ADA_COLS])
        else:
            m["t_emb"] = np.ascontiguousarray(arrs["t_emb"][c])
            m["w_ada"] = arrs["w_ada"]
        in_maps.append(m)
    return in_maps


def kernel(**inputs):
    nc = _get_program()
    res = run_bass_kernel_spmd(nc, _make_in_maps(inputs),
                               core_ids=list(range(NCORES)))
    return np.stack([r["out"] for r in res.results], axis=0)


def kernel_traced(inputs, **kw):
    """test-harness helper: returns full BassKernelResults with trace."""
    nc = _get_program()
    return run_bass_kernel_spmd(
        nc, _make_in_maps(inputs), core_ids=list(range(NCORES)), trace=True, **kw
    )


# revision 107
# speedup vs baseline: 1.7655x; 1.0089x over previous
"""DiT block kernel for Trainium2 (Bass/Tile), 8-core data parallel.

Shapes (hardcoded from the problem spec):
  x: (8, 1024, 1152), t_emb: (8, 1152)
  w_qkv (1152, 3456), w_proj (1152, 1152), w_fc1 (1152, 4608),
  w_fc2 (4608, 1152), w_ada (1152, 6912) + biases.

Strategy: batch-parallel across 8 cores (one batch element each).
Activations live transposed [D on partitions, tokens free]; projections are
out_T = W.T @ x_T with lhsT = W as stored.  All large GEMMs run in fp8e4
DoubleRow (two 128-deep k-tiles per PE pass); weights are scaled by 32 into
fp8 to stay in the normal range, compensated in the PSUM epilogue scale.
LayerNorm stats use f32r ones-column matmuls; softmax runs transposed with
no max subtraction and a ones-column appended to V for the denominators.
adaLN is contraction-split across the 8 cores (each core loads 1/8 of
w_ada, computes partial rows for all batches, then one small AllToAll
redistributes) -- gated by BASS_ADA_SPLIT.
"""

import os
import threading
from contextlib import ExitStack

import numpy as np

import concourse.bass as bass
import concourse.mybir as mybir
import concourse.tile as tile
from concourse import bacc
from concourse.bass_utils import run_bass_kernel_spmd
from concourse.masks import make_identity

F32 = mybir.dt.float32
F32R = mybir.dt.float32r
BF16 = mybir.dt.bfloat16
FP8 = mybir.dt.float8e4
AF = mybir.ActivationFunctionType
ALU = mybir.AluOpType
DR = mybir.MatmulPerfMode.DoubleRow

NCORES = 8
D = 1152
NT = 1024          # tokens per core (batch element)
KT = D // 128      # 9 partition-tiles of D
H = 16
HD = 72
HID = 4 * D        # 4608
MH = HID // 128    # 36
EPS = 1e-6
ISC = 1.0 / float(np.sqrt(HD))
# Schraudolph exp: e^(ISC*x) ~= bitcast_f32(int32(EXPA*x + EXPB)); rel err
# <~4%, used for a few heads to offload softmax exp from ACT to DVE
EXPA = float((1 << 23) * 1.4426950408889634 * (1.0 / np.sqrt(HD)))
EXPB = float((1 << 23) * (127.0 - 0.04367))
DVE_EXP_HEADS = (5, 10, 15)
WS = 32.0          # weight upscale into fp8e4 (avoids subnormals)
WSI = 1.0 / WS

ADA_SPLIT = os.environ.get("BASS_ADA_SPLIT", "1") == "1"
ADA_COLS = 6 * D // NCORES   # 864 columns of w_ada per core when split

# v output column slices aligned to head boundaries
V_SLICES = [(0, 432, 0, 6), (432, 864, 6, 12), (864, 1152, 12, 16)]


def _r(ap):
    return ap.bitcast(F32R)


def _head_segs(d0, n):
    """Split logical rows [d0, d0+n) of a [*,128]-tiled stacked tensor into
    (ktile, part0, length, dst_offset) segments within 128-partition tiles."""
    segs = []
    off = 0
    while n > 0:
        kt_i, p0 = divmod(d0, 128)
        ln = min(n, 128 - p0)
        segs.append((kt_i, p0, ln, off))
        d0 += ln
        off += ln
        n -= ln
    return segs


def _build_program():
    nc = bacc.Bacc(
        "TRN2", target_bir_lowering=False, debug=False, enable_asserts=False,
        num_devices=NCORES,
    )
    ins = {}
    ins["x"] = nc.dram_tensor("x", [NT, D], F32, kind="ExternalInput").ap()
    if ADA_SPLIT:
        ins["t_emb"] = nc.dram_tensor(
            "t_emb", [NCORES, D], F32, kind="ExternalInput").ap()
        ins["w_ada"] = nc.dram_tensor(
            "w_ada", [D, ADA_COLS], F32, kind="ExternalInput").ap()
    else:
        ins["t_emb"] = nc.dram_tensor(
            "t_emb", [D], F32, kind="ExternalInput").ap()
        ins["w_ada"] = nc.dram_tensor(
            "w_ada", [D, 6 * D], F32, kind="ExternalInput").ap()
    for name, shape in [
        ("w_qkv", [D, 3 * D]), ("b_qkv", [3 * D]),
        ("w_proj", [D, D]), ("b_proj", [D]),
        ("w_fc1", [D, HID]), ("b_fc1", [HID]),
        ("w_fc2", [HID, D]), ("b_fc2", [D]),
        ("b_ada", [6 * D]),
    ]:
        ins[name] = nc.dram_tensor(name, shape, F32, kind="ExternalInput").ap()
    out_dram = nc.dram_tensor("out", [NT, D], F32, kind="ExternalOutput").ap()

    with tile.TileContext(nc) as tc:
        _body(tc, ins, out_dram)
    nc.compile()
    return nc


def _ln_mod(tc, nc, src, dst, ada_pp, shift_c, scale_c, ones_col,
            pst, pln, ps_st, mul_pool=False):
    """dst[:,k,:] = fp8((src-mean)*rstd * ada[scale_c] + ada[shift_c]).
    Stats over the partition (D) axis per token via f32r ones matmuls."""
    ps_x, ps_q, st = {}, {}, {}
    for n in range(2):
        nsl = slice(n * 512, (n + 1) * 512)
        ps_x[n] = ps_st.tile([1, 512], F32, tag="st", name=f"psx{n}")
        ps_q[n] = ps_st.tile([1, 512], F32, tag="st", name=f"psq{n}")
        for k in range(KT):
            xb = pln.tile([128, 512], BF16, tag="xb", bufs=2, name="xb")
            nc.scalar.copy(xb[:, :], src[:, k, nsl])
            sq = pln.tile([128, 512], BF16, tag="sqb", bufs=2, name="sq")
            nc.vector.tensor_mul(sq[:, :], xb[:, :], xb[:, :])
            nc.tensor.matmul(
                ps_x[n][:, :], ones_col[:, :], xb[:, :],
                start=(k == 0), stop=(k == KT - 1), skip_group_check=True,
            )
            nc.tensor.matmul(
                ps_q[n][:, :], ones_col[:, :], sq[:, :],
                start=(k == 0), stop=(k == KT - 1), skip_group_check=True,
            )
    eps_sb = pst.tile([1, 1], F32, tag="eps", bufs=1, name="eps_sb")
    nc.vector.memset(eps_sb[:, :], EPS)
    for n in range(2):
        # rows: 0 = mean, 1 = E[x^2] -> rstd
        st[n] = pst.tile([1, 2, 512], F32, tag="lnst", bufs=2, name=f"st{n}")
        nc.vector.tensor_scalar_mul(st[n][:, 0, :], ps_x[n][:, :], 1.0 / D)
        work = pst.tile([1, 512], F32, tag="lnwork", bufs=2, name="work")
        nc.vector.tensor_mul(work[:, :], st[n][:, 0, :], st[n][:, 0, :])
        nc.vector.scalar_tensor_tensor(
            st[n][:, 1, :], ps_q[n][:, :], 1.0 / D, work[:, :],
            ALU.mult, ALU.subtract,
        )
        nc.scalar.activation(st[n][:, 1, :], st[n][:, 1, :], AF.Sqrt,
                             bias=eps_sb[:, :], scale=1.0)
        nc.vector.reciprocal(st[n][:, 1, :], st[n][:, 1, :])
    for n in range(2):
        nsl = slice(n * 512, (n + 1) * 512)
        meanB = pln.tile([128, 512], F32, tag="meanB", bufs=2, name="meanB")
        rstdB = pln.tile([128, 512], F32, tag="rstdB", bufs=2, name="rstdB")
        nc.gpsimd.partition_broadcast(meanB[:, :], st[n][:, 0, :])
        nc.gpsimd.partition_broadcast(rstdB[:, :], st[n][:, 1, :])
        for k in range(KT):
            tmp = pln.tile([128, 512], F32, tag="lnt", bufs=4, name="tmp")
            nc.vector.tensor_sub(tmp[:, :], src[:, k, nsl], meanB[:, :])
            meng = nc.gpsimd if (mul_pool and k % 2 == 0) else nc.vector
            meng.tensor_mul(tmp[:, :], tmp[:, :], rstdB[:, :])
            nc.gpsimd.tensor_scalar(
                dst[:, k, nsl], tmp[:, :],
                ada_pp[:, scale_c, k:k + 1], ada_pp[:, shift_c, k:k + 1],
                ALU.mult, ALU.add,
            )


def _truncate_out(tc, nc, out_dram):
    with tc.tile_pool(name="ptrunc", bufs=1) as p:
        z = p.tile([128, D], F32, name="z")
        nc.vector.memset(z[:, :], 0.0)
        for tt in range(NT // 128):
            nc.sync.dma_start(out_dram[tt * 128:(tt + 1) * 128, :], z[:, :])


def _body(tc, ins, out_dram):
    nc = tc.nc
    phase_limit = float(os.environ.get("BASS_PHASES", "6"))
    ctx = ExitStack()
    with ctx:
        dram = ctx.enter_context(tc.tile_pool(name="dram", bufs=1, space="DRAM"))
        if ADA_SPLIT:
            ada_part_dr = dram.tile([NCORES * ADA_COLS], F32)
            ada_gath_dr = dram.tile([NCORES * ADA_COLS], F32)
        else:
            ada_dr = dram.tile([6 * D], F32)

        pers = ctx.enter_context(tc.tile_pool(name="pers", bufs=1))
        ident = pers.tile([128, 128], F32)
        make_identity(nc, ident[:, :])
        ones_col = pers.tile([128, 1], BF16)
        nc.vector.memset(ones_col[:, :], 1.0)

        # silu(t_emb): split case holds all 8 batches, else just our own
        NB = NCORES if ADA_SPLIT else 1
        t_pp = pers.tile([128, NB, KT], F32)
        if ADA_SPLIT:
            nc.sync.dma_start(
                t_pp[:, :, :],
                ins["t_emb"].rearrange("b (k p) -> p b k", p=128))
        else:
            nc.sync.dma_start(
                t_pp[:, 0, :], ins["t_emb"].rearrange("(k p) -> p k", p=128))
        t_sb = pers.tile([128, NB, KT], BF16)
        nc.scalar.activation(t_sb[:, :, :], t_pp[:, :, :], AF.Silu)
        # preload activation tables so first exp/gelu hit no lazy table load
        warm = pers.tile([1, 2], F32)
        nc.scalar.activation(warm[:, 0:1], t_pp[0:1, 0, 0:1], AF.Exp)
        nc.scalar.activation(warm[:, 1:2], t_pp[0:1, 0, 0:1],
                             AF.Gelu_apprx_tanh)

        bqk72 = pers.tile([72, 32], F32)      # q,k bias per 72-head chunk
        bv_pp = pers.tile([72, H], F32)
        bproj_pp = pers.tile([128, KT], F32)
        bfc1_pp = pers.tile([128, MH], F32)
        bfc2_pp = pers.tile([128, KT], F32)
        bada_pp = pers.tile([128, 6, KT], F32)
        ada_pp = pers.tile([128, 6, KT], F32)

        def emit_bias_loads():
            nc.sync.dma_start(
                bqk72[:, :],
                ins["b_qkv"][0:2 * D].rearrange("(c p) -> p c", p=72))
            nc.sync.dma_start(
                bv_pp[:, :],
                ins["b_qkv"][2 * D:3 * D].rearrange("(h p) -> p h", p=72))
            nc.sync.dma_start(
                bproj_pp[:, :], ins["b_proj"].rearrange("(m p) -> p m", p=128))
            nc.sync.dma_start(
                bfc1_pp[:, :], ins["b_fc1"].rearrange("(m p) -> p m", p=128))
            nc.sync.dma_start(
                bfc2_pp[:, :], ins["b_fc2"].rearrange("(m p) -> p m", p=128))
            nc.sync.dma_start(
                bada_pp[:, :, :],
                ins["b_ada"].rearrange("(c k p) -> p c k", k=KT, p=128))

        xT = pers.tile([128, KT, NT], F32)   # becomes x2T, then out_T
        w2_sb = pers.tile([128, MH, D], FP8)  # fc2 weights, fp8*WS
        # weight-stream pool spanning phases 4-5 (proj/fc1 prefetch)
        pw_s = ctx.enter_context(tc.tile_pool(name="pw_s", bufs=1))
        # attn output store: allocated early so attention-phase pools do not
        # sit in freed qkv space (space-reuse would serialize the phases)
        es_ao = ExitStack()
        pastk = es_ao.enter_context(tc.tile_pool(name="pastk", bufs=1))
        attn_st = pastk.tile([128, KT, NT], FP8, name="attn_st")
        # qkv weight pool: lives through attention (closed at phase 4)
        es_qw = ExitStack()
        pqw = es_qw.enter_context(tc.tile_pool(name="pqw", bufs=1))

        # ============ phase 1: ada, x load+transpose, LN1 ====================
        es_mod1 = ExitStack()
        pmod1 = es_mod1.enter_context(tc.tile_pool(name="pmod1", bufs=1))
        mod1T = pmod1.tile([128, KT, NT], FP8, name="mod1T")

        with tc.tile_pool(name="p1w", bufs=1) as p1w, \
             tc.tile_pool(name="pst", bufs=1) as pst, \
             tc.tile_pool(name="pln", bufs=1) as pln:
            with tc.tile_pool(name="ps_pro", bufs=2, space="PSUM") as ps_pro, \
                 tc.tile_pool(name="pxin", bufs=3) as pxin, \
                 tc.tile_pool(name="ps_tr", bufs=3, space="PSUM") as ps_tr:

                def emit_transpose_block(tt):
                    xin = pxin.tile([128, D], F32, tag="xin", bufs=3,
                                    name="xin")
                    nc.sync.dma_start(
                        xin[:, :], ins["x"][tt * 128:(tt + 1) * 128, :])
                    for kd in range(KT):
                        pt = ps_tr.tile([128, 128], F32, tag="ptr", name="pt")
                        nc.tensor.transpose(
                            pt[:, :], xin[:, kd * 128:(kd + 1) * 128],
                            ident[:, :],
                        )
                        tsl = slice(tt * 128, (tt + 1) * 128)
                        if kd % 2 == 0:
                            nc.vector.tensor_copy(xT[:, kd, tsl], pt[:, :])
                        else:
                            nc.scalar.copy(xT[:, kd, tsl], pt[:, :])

                def emit_ada_split():
                    # partial rows for ALL batches over our 1/8 of w_ada,
                    # then AllToAll redistributes so each core gets its row.
                    ada_sb = pst.tile([NCORES, ADA_COLS], F32, tag="adasb",
                                      bufs=1, name="ada_sb")
                    for c in range(2):
                        csl = slice(c * 432, (c + 1) * 432)
                        wada_t = p1w.tile([128, KT, 432], F32, tag="wada",
                                          bufs=1, name="wada_t")
                        nc.sync.dma_start(
                            wada_t[:, :, :],
                            ins["w_ada"][:, csl]
                            .rearrange("(k p) m -> p k m", p=128),
                        )
                        wada_b = p1w.tile([128, KT, 432], BF16, tag="wadab",
                                          bufs=1, name="wada_b")
                        nc.gpsimd.tensor_copy(wada_b[:, :, :], wada_t[:, :, :])
                        pa = ps_pro.tile([NCORES, 432], F32, tag="psada",
                                         bufs=2, name="pa")
                        for k in range(KT):
                            nc.tensor.matmul(
                                pa[:, :], t_sb[:, :, k], wada_b[:, k, :],
                                start=(k == 0), stop=(k == KT - 1),
                            )
                        nc.vector.tensor_copy(ada_sb[:, csl], pa[:, :])
                    nc.sync.dma_start(
                        ada_part_dr.opt().rearrange("(b m) -> b m", b=NCORES),
                        ada_sb[:, :])
                    nc.gpsimd.collective_compute(
                        "AllToAll", ALU.bypass,
                        replica_groups=[list(range(NCORES))],
                        ins=[ada_part_dr.opt()],
                        outs=[ada_gath_dr.opt()],
                    )
                    nc.sync.dma_start(
                        ada_pp[:, :, :],
                        ada_gath_dr.opt().rearrange(
                            "(c k p) -> p c k", c=6, k=KT, p=128),
                    )

                def emit_ada_chunk(n):
                    # fallback: full w_ada on-core, M=1 bf16 matmuls
                    pa = ps_pro.tile([1, 384], F32, tag="psada", name="pa")
                    for k in range(KT):
                        wada_t = p1w.tile([128, 384], F32, tag="wsk", bufs=4,
                                          name="wada_t")
                        nc.sync.dma_start(
                            wada_t[:, :],
                            ins["w_ada"][k * 128:(k + 1) * 128,
                                         n * 384:(n + 1) * 384],
                        )
                        wada_b = p1w.tile([128, 384], BF16, tag="wskb",
                                          bufs=4, name="wada_b")
                        nc.gpsimd.tensor_copy(wada_b[:, :], wada_t[:, :])
                        nc.tensor.matmul(
                            pa[:, :], t_sb[:, 0, k:k + 1], wada_b[:, :],
                            start=(k == 0), stop=(k == KT - 1),
                        )
                    asb = pst.tile([1, 384], F32, tag="asb", bufs=2, name="asb")
                    nc.vector.tensor_copy(asb[:, :], pa[:, :])
                    nc.sync.dma_start(
                        ada_dr[n * 384:(n + 1) * 384]
                        .rearrange("(a b) -> a b", a=1),
                        asb[0:1, :],
                    )

                emit_bias_loads()
                if ADA_SPLIT:
                    emit_ada_split()
                    for i in range(8):
                        emit_transpose_block(i)
                else:
                    for i in range(8):
                        emit_transpose_block(i)
                        if i < 8:
                            emit_ada_chunk(2 * i)
                            emit_ada_chunk(2 * i + 1)
                    for n in range(16, 18):
                        emit_ada_chunk(n)
                    nc.sync.dma_start(
                        ada_pp[:, :, :],
                        ada_dr.opt().rearrange(
                            "(c k p) -> p c k", c=6, k=KT, p=128),
                    )
                nc.vector.tensor_add(ada_pp[:, :, :], ada_pp[:, :, :],
                                     bada_pp[:, :, :])
                nc.vector.tensor_scalar_add(ada_pp[:, 1, :], ada_pp[:, 1, :],
                                            1.0)
                nc.vector.tensor_scalar_add(ada_pp[:, 4, :], ada_pp[:, 4, :],
                                            1.0)

            if phase_limit > 0.6:
                with tc.tile_pool(name="ps_st", bufs=4, space="PSUM") as ps_st:
                    _ln_mod(tc, nc, xT, mod1T, ada_pp, 0, 1, ones_col,
                            pst, pln, ps_st)

        if phase_limit <= 1:
            es_mod1.close()
            return _truncate_out(tc, nc, out_dram)

        # ============ phase 2: qkv ==========================================
        es_qk = ExitStack()
        pqks = es_qk.enter_context(tc.tile_pool(name="pqks", bufs=1, side="right"))
        # chunks 0..15 = q heads, 16..31 = k heads; fp8 true values
        qk_st = pqks.tile([72, 32, NT], FP8, name="qk_st")
        pvaug = es_qk.enter_context(
            tc.tile_pool(name="pvaug", bufs=1, side="right"))
        # per head: cols 0..72 = v (true values), col 96 = ones, 72..96 zero
        v_aug = pvaug.tile([128, NT // 128, H, 97], FP8, name="v_aug")
        nc.gpsimd.memset(v_aug[:, :, :, HD:97], 0.0)
        nc.gpsimd.memset(v_aug[:, :, :, 96:97], 1.0)

        def emit_w2_convert(k):
            w2src = pw_s.tile([128, D], F32, tag="w2src", bufs=2,
                              name="w2src")
            nc.sync.dma_start(
                w2src[:, :], ins["w_fc2"][k * 128:(k + 1) * 128, :]
            )
            nc.gpsimd.tensor_scalar_mul(w2_sb[:, k, :], w2src[:, :], WS)

        def mm_group(psl, lhs8, rhs8, rhs_k_of, N):
            """accumulate 9 k-tiles: 4 DoubleRow pairs + 1 plain fp8."""
            for kk in range(4):
                nc.tensor.matmul(
                    psl, lhs8(2 * kk, 2), rhs_k_of(2 * kk, 2),
                    start=(kk == 0), stop=False, perf_mode=DR,
                )
            nc.tensor.matmul(psl, lhs8(8, 1), rhs_k_of(8, 1),
                             start=False, stop=True)

        with tc.tile_pool(name="p2w", bufs=1) as p2w, \
             tc.tile_pool(name="ps_mm", bufs=4, space="PSUM") as ps_mm:

            def emit_qk_dh(sec, j):
                # sec 0 = q (w cols j*144), sec 1 = k (w cols 1152 + j*144)
                wq_t = pqw.tile([128, KT, 144], F32, tag="wsq", bufs=2,
                                name="wq_t")
                nc.sync.dma_start(
                    wq_t[:, :, :],
                    ins["w_qkv"][:, sec * D + j * 144:sec * D + (j + 1) * 144]
                    .rearrange("(k p) m -> p k m", p=128),
                )
                wq_8 = pqw.tile([128, KT, 144], FP8, tag="wsbq", bufs=2,
                                name="wq_8")
                nc.gpsimd.tensor_scalar_mul(wq_8[:, :, :], wq_t[:, :, :], WS)
                for i in range(2):
                    ch = 16 * sec + 2 * j + i
                    hsl = slice(72 * i, 72 * i + 72)
                    ps2 = ps_mm.tile([72, 1024], F32, tag="mm", bufs=2,
                                     name="ps2")
                    for n in range(2):
                        nsl = slice(n * 512, (n + 1) * 512)

                        def lhs8(k0, kn, hsl=hsl):
                            w = wq_8[:, k0:k0 + kn, hsl]
                            return w if kn == 2 else wq_8[:, k0, hsl]

                        def rhs8(k0, kn, nsl=nsl):
                            m = mod1T[:, k0:k0 + kn, nsl]
                            return m if kn == 2 else mod1T[:, k0, nsl]

                        mm_group(ps2[:, nsl], lhs8, rhs8, rhs8, 512)
                    nc.scalar.activation(
                        qk_st[:, ch, :], ps2[:, :], AF.Identity,
                        bias=bqk72[:, ch:ch + 1], scale=WSI,
                    )

            def emit_v_slice(si):
                (c0, c1, h0, h1) = V_SLICES[si]
                wv_t = p2w.tile([128, KT, 432], F32, tag="wv", bufs=1,
                                name="wv_t")
                nc.sync.dma_start(
                    wv_t[:, :, 0:c1 - c0],
                    ins["w_qkv"][:, 2 * D + c0:2 * D + c1]
                    .rearrange("(k p) m -> p k m", p=128),
                )
                wv_8 = p2w.tile([128, KT, 432], FP8, tag="wvb", bufs=2,
                                name="wv_8")
                nc.vector.tensor_scalar_mul(wv_8[:, :, 0:c1 - c0],
                                            wv_t[:, :, 0:c1 - c0], WS)
                for tt in range(NT // 128):
                    pmv = ps_mm.tile([128, 512], F32, tag="mmv", bufs=2,
                                     name="pmv")

                    def lhsv(k0, kn, tt=tt):
                        m = mod1T[:, k0:k0 + kn, tt * 128:(tt + 1) * 128]
                        return m if kn == 2 else mod1T[:, k0, tt * 128:(tt + 1) * 128]

                    def rhsv(k0, kn, c0=c0, c1=c1):
                        w = wv_8[:, k0:k0 + kn, 0:c1 - c0]
                        return w if kn == 2 else wv_8[:, k0, 0:c1 - c0]

                    mm_group(pmv[:, 0:c1 - c0], lhsv, rhsv, rhsv, c1 - c0)
                    for h in range(h0, h1):
                        nc.vector.tensor_scalar_mul(
                            v_aug[:, tt, h, 0:HD],
                            pmv[:, h * HD - c0:(h + 1) * HD - c0], WSI,
                        )

            # interleave q/k head-pairs and v slices so attention on early
            # heads can start while later chunks are still being produced
            for j in range(8):
                emit_qk_dh(0, j)
                emit_qk_dh(1, j)
                if j in (1, 3, 5):
                    emit_v_slice({1: 0, 3: 1, 5: 2}[j])
        es_mod1.close()
        es_qw.close()
        if phase_limit <= 2:
            es_qk.close()
            return _truncate_out(tc, nc, out_dram)

        # ============ phase 3: attention ====================================
        es_ao = ExitStack()
        pastk = es_ao.enter_context(tc.tile_pool(name="pastk", bufs=1))
        attn_st = pastk.tile([128, KT, NT], FP8, name="attn_st")

        with tc.tile_pool(name="pexp", bufs=2) as pexp, \
             tc.tile_pool(name="pattn", bufs=2) as pattn, \
             tc.tile_pool(name="ps_pj", bufs=2, space="PSUM") as ps_pj:
          with tc.tile_pool(name="ps_s", bufs=2, space="PSUM") as ps_s, \
               tc.tile_pool(name="ps_av", bufs=2, space="PSUM") as ps_av:
            for h in range(H):
                exp_h = pexp.tile([128, NT // 128, NT], FP8, tag="exp",
                                  name="exp_h")
                for kt_i in range(NT // 128):
                    pss2 = ps_s.tile([128, 1024], F32, tag="s", name="pss2")
                    for n in range(2):
                        nsl = slice(n * 512, (n + 1) * 512)
                        nc.tensor.matmul(
                            pss2[:, nsl],
                            qk_st[:, 16 + h, kt_i * 128:(kt_i + 1) * 128],
                            qk_st[:, h, nsl], start=True, stop=True,
                            skip_group_check=True,
                        )
                    if kt_i == 4:
                        # Schraudolph exp on DVE: one tile per head runs in
                        # parallel with ACT's seven, shaving the ACT span
                        zi = pexp.tile([128, 1024], mybir.dt.int32,
                                       tag="zi", bufs=2, name="zi")
                        nc.vector.tensor_scalar(
                            zi[:, :], pss2[:, :], EXPA, EXPB,
                            ALU.mult, ALU.add,
                        )
                        nc.vector.tensor_copy(exp_h[:, kt_i, :],
                                              zi[:, :].bitcast(F32))
                    else:
                        nc.scalar.activation(
                            exp_h[:, kt_i, :], pss2[:, :], AF.Exp, scale=ISC
                        )
                attn_h = pattn.tile([72, NT], FP8, tag="attnh", name="attn_h")
                for n in range(2):
                    nsl = slice(n * 512, (n + 1) * 512)
                    pav = ps_av.tile([97, 512], F32, tag="av", name="pav")
                    for kk in range(4):
                        nc.tensor.matmul(
                            pav[:, :], v_aug[:, 2 * kk:2 * kk + 2, h, :],
                            exp_h[:, 2 * kk:2 * kk + 2, nsl],
                            start=(kk == 0), stop=(kk == 3), perf_mode=DR,
                        )
                    recip = pattn.tile([1, 512], F32, tag="recip", bufs=2,
                                       name="recip")
                    nc.vector.reciprocal(recip[:, :], pav[96:97, :])
                    bca = pattn.tile([72, 512], F32, tag="bca", name="bca")
                    nc.gpsimd.partition_broadcast(bca[:, :], recip[:, :])
                    atf = pattn.tile([72, 512], F32, tag="atf", bufs=2,
                                     name="atf")
                    nc.vector.tensor_mul(atf[:, :], pav[0:72, :], bca[:, :])
                    nc.vector.tensor_scalar_add(
                        attn_h[:, nsl], atf[:, :], bv_pp[:, h:h + 1]
                    )
                for (kt_i, p0, ln, off) in _head_segs(h * HD, HD):
                    nc.sync.dma_start(
                        attn_st[p0:p0 + ln, kt_i, :], attn_h[off:off + ln, :]
                    )
                with tc.tile_wait_until(0.150 + 0.006 * h):
                    for k2 in range((h * MH) // H, ((h + 1) * MH) // H):
                        emit_w2_convert(k2)

          if True:
            # proj chases head completion: its k-accumulation consumes
            # attn_st k-tiles as the covering heads finish
            for mo in range(KT):
                wp_t = pw_s.tile([128, KT, 128], F32, tag="ws", bufs=3,
                                 name="wp_t")
                nc.sync.dma_start(
                    wp_t[:, :, :],
                    ins["w_proj"][:, mo * 128:(mo + 1) * 128]
                    .rearrange("(k p) m -> p k m", p=128),
                )
                wp_8 = pw_s.tile([128, KT, 128], FP8, tag="wsb", bufs=3,
                                 name="wp_8")
                nc.gpsimd.tensor_scalar_mul(wp_8[:, :, :], wp_t[:, :, :],
                                            WS)
                for n in range(2):
                    nsl = slice(n * 512, (n + 1) * 512)
                    ps2p = ps_pj.tile([128, 512], F32, tag="mm2", bufs=2,
                                      name="ps2p")

                    def lhsp(k0, kn):
                        w = wp_8[:, k0:k0 + kn, :]
                        return w if kn == 2 else wp_8[:, k0, :]

                    def rhsp(k0, kn, nsl=nsl):
                        a = attn_st[:, k0:k0 + kn, nsl]
                        return a if kn == 2 else attn_st[:, k0, nsl]

                    mm_group(ps2p[:, :], lhsp, rhsp, rhsp, 512)
                    t_sb4 = pattn.tile([128, 512], F32, tag="tsb", bufs=3,
                                       name="t_sb4")
                    nc.scalar.activation(
                        t_sb4[:, :], ps2p[:, :], AF.Identity,
                        bias=bproj_pp[:, mo:mo + 1], scale=WSI,
                    )
                    nc.vector.scalar_tensor_tensor(
                        xT[:, mo, nsl], t_sb4[:, :],
                        ada_pp[:, 2, mo:mo + 1],
                        xT[:, mo, nsl], ALU.mult, ALU.add,
                    )
        es_qk.close()
        es_ao.close()
        if phase_limit <= 3:
            es_ao.close()
            return _truncate_out(tc, nc, out_dram)

        # ============ phase 4: proj + residual1 + LN2 ========================
        es_f1h = ExitStack()
        pf1h = es_f1h.enter_context(
            tc.tile_pool(name="pf1h", bufs=1, side="right"))
        F1H = 18   # fc1 m-tiles pre-converted during phase 4
        fc1_8 = pf1h.tile([128, KT, F1H * 128], FP8, name="fc1_8")
        es_mod2 = ExitStack()
        pmod2 = es_mod2.enter_context(
            tc.tile_pool(name="pmod2", bufs=1, side="right"))
        mod2T = pmod2.tile([128, KT, NT], FP8, name="mod2T")

        def emit_f1h_convert(p4f, mo):
            wfh_t = p4f.tile([128, KT, 128], F32, tag="wsf", bufs=6,
                             name="wfh_t")
            nc.sync.dma_start(
                wfh_t[:, :, :],
                ins["w_fc1"][:, mo * 128:(mo + 1) * 128]
                .rearrange("(k p) m -> p k m", p=128),
            )
            nc.scalar.activation(
                fc1_8[:, :, mo * 128:(mo + 1) * 128], wfh_t[:, :, :],
                AF.Identity, scale=WS)

        with tc.tile_pool(name="pst4", bufs=1) as pst4, \
             tc.tile_pool(name="pln4", bufs=1) as pln4, \
             tc.tile_pool(name="p4f", bufs=1) as p4f:
            with tc.tile_pool(name="ps_st2", bufs=4, space="PSUM") as ps_st2:
                _ln_mod(tc, nc, xT, mod2T, ada_pp, 3, 4, ones_col,
                        pst4, pln4, ps_st2)
            for mo in range(F1H):
                emit_f1h_convert(p4f, mo)
        if phase_limit <= 4:
            es_mod2.close()
            return _truncate_out(tc, nc, out_dram)

        # ============ phase 5: FFN + output =================================
        # half-granular pipeline: fc1 half-0 (prefetched weights) starts as
        # soon as mod2T half-0 exists; fc2 half-0 chases; streamed fc1
        # weights cover both halves between the two prefetched passes.
        with tc.tile_pool(name="p5w", bufs=1) as p5w, \
             tc.tile_pool(name="p5h", bufs=1) as p5h, \
             tc.tile_pool(name="ps_5", bufs=1, space="PSUM") as ps_5:
            hT_sb = p5h.tile([128, MH, NT], FP8, name="hT_sb")

            def fc1_group(mo, n, wf_8):
                nsl = slice(n * 512, (n + 1) * 512)
                pf1 = ps_5.tile([128, 512], F32, tag="f1", bufs=3,
                                name="pf1")

                def lhsf(k0, kn, mo=mo, wf_8=wf_8):
                    if wf_8 is None:
                        msl = slice(mo * 128, (mo + 1) * 128)
                        w = fc1_8[:, k0:k0 + kn, msl]
                        return w if kn == 2 else fc1_8[:, k0, msl]
                    w = wf_8[:, k0:k0 + kn, :]
                    return w if kn == 2 else wf_8[:, k0, :]

                def rhsf(k0, kn, nsl=nsl):
                    m = mod2T[:, k0:k0 + kn, nsl]
                    return m if kn == 2 else mod2T[:, k0, nsl]

                mm_group(pf1[:, :], lhsf, rhsf, rhsf, 512)
                nc.scalar.activation(
                    hT_sb[:, mo, nsl], pf1[:, :], AF.Gelu_apprx_tanh,
                    bias=bfc1_pp[:, mo:mo + 1], scale=WSI,
                )

            def fc2_m(m, n):
                nsl = slice(n * 512, (n + 1) * 512)
                pf2 = ps_5.tile([128, 512], F32, tag="f2", bufs=2,
                                name="pf2")
                for kk in range(MH // 2):
                    nc.tensor.matmul(
                        pf2[:, :],
                        w2_sb[:, 2 * kk:2 * kk + 2, m * 128:(m + 1) * 128],
                        hT_sb[:, 2 * kk:2 * kk + 2, nsl],
                        start=(kk == 0), stop=(kk == MH // 2 - 1),
                        perf_mode=DR,
                    )
                t2 = p5w.tile([128, 512], F32, tag="tsb", bufs=2, name="t2")
                nc.scalar.activation(
                    t2[:, :], pf2[:, :], AF.Identity,
                    bias=bfc2_pp[:, m:m + 1], scale=WSI,
                )
                nc.vector.scalar_tensor_tensor(
                    xT[:, m, nsl], t2[:, :],
                    ada_pp[:, 5, m:m + 1], xT[:, m, nsl],
                    ALU.mult, ALU.add,
                )

            # fc1 prefetched half-0, then streamed mo both halves, then
            # prefetched half-1 (fc2 half-0 can start during the latter)
            for mo in range(F1H):
                fc1_group(mo, 0, None)
            for mo in range(F1H, MH):
                wf_t = pw_s.tile([128, KT, 128], F32, tag="ws", bufs=3,
                                 name="wf_t")
                nc.sync.dma_start(
                    wf_t[:, :, :],
                    ins["w_fc1"][:, mo * 128:(mo + 1) * 128]
                    .rearrange("(k p) m -> p k m", p=128),
                )
                wf_8 = pw_s.tile([128, KT, 128], FP8, tag="wsb",
                                 bufs=3, name="wf_8")
                if mo % 2 == 0:
                    nc.gpsimd.tensor_scalar_mul(wf_8[:, :, :],
                                                wf_t[:, :, :], WS)
                else:
                    nc.vector.tensor_scalar_mul(wf_8[:, :, :],
                                                wf_t[:, :, :], WS)
                fc1_group(mo, 0, wf_8)
                fc1_group(mo, 1, wf_8)
            for mo in range(F1H):
                fc1_group(mo, 1, None)

            ot2 = {}
            for n in range(2):
                for m in range(KT):
                    fc2_m(m, n)
                    # transpose this half's token tiles; store [128,256]
                    # chunks once both m's of a pair are done
                    if m % 2 == 0:
                        ot2[n] = p5w.tile([128, 4, 256], F32, tag="ot2",
                                          bufs=2, name="ot2")
                    sl = slice(128 * (m % 2), 128 * (m % 2) + 128)
                    for tt in range(4 * n, 4 * n + 4):
                        pt = ps_5.tile([128, 128], F32, tag="tro", bufs=2,
                                       name="pt6")
                        nc.tensor.transpose(
                            pt[:, :], xT[:, m, tt * 128:(tt + 1) * 128],
                            ident[:, :],
                        )
                        nc.vector.tensor_copy(ot2[n][:, tt - 4 * n, sl],
                                              pt[:, :])
                        if m % 2 == 1 or m == KT - 1:
                            w = 128 * (m % 2) + 128
                            nc.sync.dma_start(
                                out_dram[tt * 128:(tt + 1) * 128,
                                         (m - m % 2) * 128:
                                         (m - m % 2) * 128 + w],
                                ot2[n][:, tt - 4 * n, 0:w],
                            )
        es_mod2.close()
        es_f1h.close()


_LOCK = threading.Lock()
_PROG = None


def _get_program():
    global _PROG
    with _LOCK:
        if _PROG is None:
            _PROG = _build_program()
    return _PROG


def _make_in_maps(inputs):
    arrs = {k: np.ascontiguousarray(np.asarray(v, dtype=np.float32))
            for k, v in inputs.items()}
    in_maps = []
    for c in range(NCORES):
        m = {k: v for k, v in arrs.items()
             if k not in ("x", "t_emb", "w_ada")}
        m["x"] = np.ascontiguousarray(arrs["x"][c])
        if ADA_SPLIT:
            m["t_emb"] = arrs["t_emb"]
            m["w_ada"] = np.ascontiguousarray(
                arrs["w_ada"][:, c * ADA_COLS:(c + 1) * ADA_COLS])
        else:
            m["t_emb"] = np.ascontiguousarray(arrs["t_emb"][c])
            m["w_ada"] = arrs["w_ada"]
        in_maps.append(m)
    return in_maps


def kernel(**inputs):
    nc = _get_program()
    res = run_bass_kernel_spmd(nc, _make_in_maps(inputs),
                               core_ids=list(range(NCORES)))
    return np.stack([r["out"] for r in res.results], axis=0)


def kernel_traced(inputs, **kw):
    """test-harness helper: returns full BassKernelResults with trace."""
    nc = _get_program()
    return run_bass_kernel_spmd(
        nc, _make_in_maps(inputs), core_ids=list(range(NCORES)), trace=True, **kw
    )
